# revision 20
# baseline (speedup 1.0000x reference)
"""Trainium2 Bass kernel for AlignmentContrastiveLoss (8 NeuronCores, SPMD).

Reference semantics:
  im = im_set[:, 1:, :]           [256, 36, 1024]
  s  = s_seq[:, 1:-2, :]          [256, 32, 1024]
  align[i,j,n,m] = im[i,n] . s[j,m], zeroed where n >= im_len[i]-1 or m >= s_len[j]-3
  scores[i,j] = sum_m max_n align[i,j,n,m]
  loss = sum_i relu(M + max_{j!=i} scores[i,j] - scores[i,i])
       + sum_j relu(M + max_{i!=j} scores[i,j] - scores[j,j])

Sharding: data-parallel over images (32 per core), s replicated.

v2 design:
  - f32 tokens are cast once to fp8e4 (im-mask fused as activation scale),
    bitcast to 16-bit fp8-pairs and transposed via the DMA XBAR (pure bit
    movement) into a packed layout: partition p of q-chunk q holds the d
    pair (256q+2p, 256q+2p+1) interleaved per token (HW-verified mapping).
  - s feeds the PE as RAW packed bytes via MatmulPerfMode.DoubleRowSwInterleave
    (stationary side accepts interleaved pairs; output partitions come out
    token-REVERSED, verified on HW). im (small) is deinterleaved to planar
    [128, q, 2, tok] fp8 by one gpsimd 4D copy per tile, which also compacts
    away the XBAR pad columns. No PE transposes, no PSUM->SBUF staging copies.
  - The s-token reversal is compensated in the word-sum weights (w4 built
    from a reversed word mask), so scoresT and the loss tail are unchanged.
  - wt-outer loop: per s-tile, 3 region-chunk matmul groups + vector MAX
    reduce; word-sum is a tiny PE matmul against s-mask-weighted block-ones
    (applies the s word mask for free and directly yields scoresT[img,sent]).
  - im staged as 11 tiles of 3 images (112-partition windows, 4-col overlap
    into the next tile's range which is later overwritten with real data).
  - Cross-core traffic: one 520-float AllGather of per-core column-max
    partials + scattered diagonals + local cost_s sum.
"""

import numpy as np

MARGIN = 0.2
B = 256          # global batch (images == sentences)
NCORES = 8
BI = B // NCORES  # images per core = 32
NREG = 36        # regions per image after stripping
NWORD = 32       # words per sentence after stripping
D = 1024
IM_TOK = BI * NREG      # 1152 dense im tokens
S_TOK = B * NWORD       # 8192 s tokens
S_TILES = S_TOK // 128  # 64
BIG = 1.0e30
# region chunks for the main matmul: (token offset, ntok, nimg)
RCHUNKS = [(0, 432, 12), (432, 432, 12), (864, 288, 8)]
# im staging tiles: (dense token offset, window (mult of 16), first image, n images)
IM_STAGE = [(108 * t, 112, 3 * t, 3) for t in range(10)] + [(1080, 80, 30, 2)]
IM_TP_COLS = 112 * 11  # padded XBAR destination: disjoint 112-col windows
BLK = 520  # allgather block floats: 256 colmax | 256 diag-scatter | 1 cost_s | pad


def fix_multiwaits(nc, mybir):
    """This toolchain's walrus accepts 1 wait per instruction (2 for
    EventSemaphore); Tile can emit more. Offload surplus waits onto
    inserted same-engine NoOps placed immediately before the instruction."""
    n_fix = 0
    for fn in nc.m.functions:
        for blk in fn.blocks:
            insts = blk.instructions
            i = 0
            while i < len(insts):
                inst = insts[i]
                si = inst.sync_info
                waits = list(si.on_wait) if si is not None and si.on_wait else []
                cap = 2 if isinstance(inst, mybir.InstEventSemaphore) else 1
                if len(waits) > cap:
                    surplus, keep = waits[:-cap], waits[-cap:]
                    si.on_wait = keep
                    for w in surplus:
                        nop = mybir.InstNoOp(
                            name=f"{inst.name}_wsplit{n_fix}",
                            engine=inst.engine,
                            ins=[],
                            outs=[],
                            sync_info=mybir.SyncInfo(on_wait=[w], on_update=[]),
                        )
                        insts.insert(i, nop)
                        n_fix += 1
                        i += 1
                i += 1
    return n_fix


DEFAULT_OPTS = {
    "sf_bufs": 12,     # f32 staging tiles
    "pk_bufs": 12,     # packed fp8-as-bf16 staging tiles
    "alp_bufs": 6,     # PSUM align buffers
    "gpsimd_cast": 3,  # every Nth s cast on gpsimd (0 = all scalar)
    "im_head": 4,      # im tiles staged before the first s tile
}


def build_graph(opts=None):
    import concourse.bass as bass
    import concourse.mybir as mybir
    import concourse.tile as tile
    from concourse.masks import make_identity
    from contextlib import ExitStack

    opts = {**DEFAULT_OPTS, **(opts or {})}

    f32 = mybir.dt.float32
    bf16 = mybir.dt.bfloat16
    fp8 = mybir.dt.float8e4
    i32 = mybir.dt.int32
    ALU = mybir.AluOpType
    AX = mybir.AxisListType
    ACTF = mybir.ActivationFunctionType
    SWI = mybir.MatmulPerfMode.DoubleRowSwInterleave

    nc = bass.Bass()

    im_ext = nc.declare_dram_parameter("im_set", [BI, 37, D], f32, isOutput=False)
    s_ext = nc.declare_dram_parameter("s_seq", [B, 35, D], f32, isOutput=False)
    imlen_ext = nc.declare_dram_parameter("im_len", [BI], i32, isOutput=False)
    slen_ext = nc.declare_dram_parameter("s_len", [B], i32, isOutput=False)
    dmask_ext = nc.declare_dram_parameter("diag_mask", [B, BI], f32, isOutput=False)
    dmaskT_ext = nc.declare_dram_parameter("diag_maskT", [BI, B], f32, isOutput=False)
    out_ext = nc.declare_dram_parameter("out", [1], f32, isOutput=True)
    if opts.get("debug"):
        dbg_sel = nc.declare_dram_parameter("dbg_sel", [128, 128], f32, isOutput=True)
        dbg_pm = nc.declare_dram_parameter("dbg_pm", [128, 128], f32, isOutput=True)
        dbg_rb = nc.declare_dram_parameter("dbg_rb", [128, 128], f32, isOutput=True)
        dbg_tc = nc.declare_dram_parameter("dbg_tc", [128, 2], f32, isOutput=True)
        dbg_bm = nc.declare_dram_parameter("dbg_bm", [2, 128, 128], f32, isOutput=True)
        dbg_w4 = nc.declare_dram_parameter("dbg_w4", [128, S_TILES, 4], f32, isOutput=True)
        dbg_scT = nc.declare_dram_parameter("dbg_scT", [BI, S_TILES, 4], f32, isOutput=True)
        dbg_mx = nc.declare_dram_parameter("dbg_mx", [128, S_TILES, BI], f32, isOutput=True)

    with tile.TileContext(nc) as tc, ExitStack() as top:
        # ---------------- constants ----------------
        const = top.enter_context(tc.tile_pool(name="const", bufs=1))
        ident_f32 = const.tile([128, 128], f32)
        make_identity(nc, ident_f32)
        ones32 = const.tile([32, 1], f32)
        nc.gpsimd.memset(ones32, 1.0)
        ones128 = const.tile([128, 1], f32)
        nc.gpsimd.memset(ones128, 1.0)

        # ---------------- token masks (device-side) ----------------
        mpool = top.enter_context(tc.tile_pool(name="masks", bufs=1))
        dram = top.enter_context(tc.tile_pool(name="dram", bufs=1, space="DRAM"))

        # per-image region mask [BI, NREG]: n < im_len-1
        imlen_sb = mpool.tile([BI, 1], i32)
        nc.sync.dma_start(imlen_sb, imlen_ext.rearrange("(p o) -> p o", o=1))
        il_sb = mpool.tile([BI, 1], f32)
        nc.gpsimd.tensor_scalar(il_sb, imlen_sb, 1, None, op0=ALU.subtract)
        iota_r = mpool.tile([BI, NREG], f32)
        nc.gpsimd.iota(
            iota_r, pattern=[[1, NREG]], base=0, channel_multiplier=0,
            allow_small_or_imprecise_dtypes=True,
        )
        mask_im = mpool.tile([BI, NREG], f32)
        nc.gpsimd.tensor_scalar(mask_im, iota_r, il_sb, None, op0=ALU.is_lt)
        # maskcol_im [128, 11]: per (partition, im stage tile) in REGION-major
        # stage order (p = ni*n + i); pad rows -> 0
        mask_im_dram = dram.tile([BI, NREG], f32)
        nc.sync.dma_start(mask_im_dram[:, :], mask_im)
        maskcol_im = mpool.tile([128, len(IM_STAGE)], f32)
        nc.gpsimd.memset(maskcol_im, 0.0)
        for t, (toff, win, i0, ni) in enumerate(IM_STAGE):
            nc.sync.dma_start(
                maskcol_im[0:36 * ni, t:t + 1],
                mask_im_dram[i0:i0 + ni, :].rearrange("i n -> n i"),
            )

        # s word masks -> w4 block-ones weights [128, 64, 4] bf16:
        #   w4[32*jj + m, wt, jj] = (m < s_len[4*wt+jj] - 3)
        slen_sb = mpool.tile([128, 2], i32)
        nc.sync.dma_start(slen_sb, slen_ext.rearrange("(t p) -> p t", p=128))
        sl_sb = mpool.tile([128, 2], f32)
        nc.gpsimd.tensor_scalar(sl_sb, slen_sb, 3, None, op0=ALU.subtract)
        iota_w = mpool.tile([128, NWORD], f32)
        nc.gpsimd.iota(
            iota_w, pattern=[[1, NWORD]], base=0, channel_multiplier=0,
            allow_small_or_imprecise_dtypes=True,
        )
        # Word-sum weights for WORD-major stage order (p = 4w + j) combined
        # with the SwInterleave token reversal (partition p <-> raw col 127-p):
        #   w4[p, wt, jj] = [ (127-p)%4 == jj ] * ( (127-p)//4 < sl[4*wt+jj] )
        # Built transposed (partition c = sentence-within-half, free p) then
        # PE-transposed into place.
        rb = mpool.tile([128, 128], f32)     # rb[c, p] = (127-p)//4
        nc.gpsimd.iota(rb, pattern=[[-1, 32], [0, 4]], base=31,
                       channel_multiplier=0, allow_small_or_imprecise_dtypes=True)
        # sel[c, p] = (p%4 == 3 - c%4)  <=>  ((c + p + 1) & 3 == 0)
        cp_i = mpool.tile([128, 128], i32)
        nc.gpsimd.iota(cp_i, pattern=[[1, 128]], base=1, channel_multiplier=1)
        cp_a = mpool.tile([128, 128], i32)
        nc.vector.tensor_scalar(cp_a, cp_i, 3, None, op0=ALU.bitwise_and)
        sel = mpool.tile([128, 128], f32)
        nc.vector.tensor_scalar(sel, cp_a, 0, None, op0=ALU.is_equal)
        w4 = mpool.tile([128, S_TILES, 4], bf16)
        with tc.tile_pool(name="w4ps", bufs=2, space="PSUM") as wps:
            for h in range(2):
                bh = mpool.tile([128, 128], f32, tag=f"w4bh{h}")
                nc.vector.tensor_scalar(
                    bh, rb, sl_sb[:, h:h + 1], None, op0=ALU.is_lt
                )
                bm = mpool.tile([128, 128], f32, tag=f"w4bm{h}")
                nc.vector.tensor_mul(bm, bh, sel)
                wt_ps = wps.tile([128, 128], f32, tag=f"w4t{h}")
                nc.tensor.transpose(wt_ps, bm, ident_f32)
                nc.vector.tensor_copy(
                    w4[:, 32 * h:32 * (h + 1), :].rearrange("p a b -> p (a b)"),
                    wt_ps,
                )
                if opts.get("debug"):
                    nc.sync.dma_start(dbg_bm[h, :, :], bm)
        if opts.get("debug"):
            nc.sync.dma_start(dbg_sel[:, :], sel)
            nc.sync.dma_start(dbg_rb[:, :], rb)

        # diag masks (sharding metadata inputs)
        dmask_sb = mpool.tile([128, 2, BI], f32)
        nc.sync.dma_start(dmask_sb, dmask_ext.rearrange("(t p) i -> p t i", p=128))
        dmaskT_sb = mpool.tile([BI, 2, 128], f32)
        nc.sync.dma_start(dmaskT_sb, dmaskT_ext.rearrange("p (t f) -> p t f", f=128))

        # ---------------- persistent big buffers ----------------
        big = top.enter_context(tc.tile_pool(name="big", bufs=1))
        # packed-transposed fp8 pairs, stored as bf16 bit containers.
        # One tile per staging unit so the (whole-tile) dependency tracking
        # of the XBAR DMA writes stays exactly per-tile.
        imTp_t = [big.tile([128, 4, 112], bf16, name=f"imTp{t}") for t in range(len(IM_STAGE))]
        imP = big.tile([128, 4, 2, IM_TOK], fp8)      # dense planar im
        sTp_t = [big.tile([128, 4, 128], bf16, name=f"sTp{i}") for i in range(S_TILES)]
        maxima = big.tile([128, S_TILES, BI], bf16)  # per (word, wtile, img) region-max
        scoresT_sb = big.tile([BI, S_TILES, 4], f32)  # [img, wt, jj] == [img, sent]
        scores_sb = big.tile([128, 2, BI], f32)       # [sent%128, sent//128, img]

        # fp8 views: im pair-split for the deinterleave, s raw for SwInterleave
        imTp8_t = [
            t.bitcast(fp8).rearrange("p q (t b) -> p q b t", b=2) for t in imTp_t
        ]
        sTraw_t = [t.bitcast(fp8) for t in sTp_t]     # each [128, 4, 256]

        with ExitStack() as mid:
            stage = mid.enter_context(
                tc.tile_pool(name="stage", bufs=opts["sf_bufs"])
            )
            pk = mid.enter_context(tc.tile_pool(name="pk", bufs=opts["pk_bufs"]))
            alp = mid.enter_context(
                tc.tile_pool(name="alp", bufs=opts["alp_bufs"], space="PSUM")
            )
            scp = mid.enter_context(tc.tile_pool(name="scp", bufs=1, space="PSUM"))
            scoresT_ps = scp.tile([BI, S_TILES, 4], f32)

            def stage_im(t):
                toff, win, i0, ni = IM_STAGE[t]
                nreal = 36 * ni
                tf32 = stage.tile([128, D], f32, tag="sf32")
                nc.sync.dma_start(
                    tf32[0:nreal, :],
                    im_ext[i0:i0 + ni, 1:1 + NREG, :].rearrange("i n d -> n i d"),
                )
                tbf = pk.tile([128, D // 2], bf16, tag="spk")
                nc.scalar.activation(
                    tbf.bitcast(fp8)[0:win, :], tf32[0:win, :],
                    ACTF.Copy, scale=maskcol_im[0:win, t:t + 1],
                )
                # XBAR into the tile's private buffer
                nc.scalar.dma_start(
                    imTp_t[t][:, :, 0:win], tbf[0:win, :], transpose=True
                )
                # deinterleave + compact + un-permute (region-major stage
                # order ni*n + i -> dense 36*i + n) in one strided copy
                nc.gpsimd.tensor_copy(
                    imP[:, :, :, toff:toff + nreal].rearrange(
                        "p q b (i n) -> p q b i n", n=NREG
                    ),
                    imTp8_t[t][:, :, :, 0:nreal].rearrange(
                        "p q b (n i) -> p q b n i", i=ni
                    ).rearrange("p q b n i -> p q b i n"),
                )

            def stage_s(i):
                tf32 = stage.tile([128, D], f32, tag="sf32")
                nc.sync.dma_start(
                    tf32,
                    s_ext[4 * i:4 * i + 4, 1:1 + NWORD, :].rearrange(
                        "j w d -> w j d"
                    ),
                )
                tbf = pk.tile([128, D // 2], bf16, tag="spk")
                g = opts["gpsimd_cast"]
                if g and (i % g == 0):
                    nc.gpsimd.tensor_copy(tbf.bitcast(fp8), tf32)
                else:
                    nc.scalar.activation(tbf.bitcast(fp8), tf32, ACTF.Copy)
                # alternate transpose dispatch between sync and scalar queues
                eng = nc.sync if (i % 2 == 0) else nc.scalar
                eng.dma_start(sTp_t[i], tbf, transpose=True)

            # stage im head first (needed by rc0), then stream s with the rest
            head = opts["im_head"]
            for t in range(head):
                stage_im(t)
            stage_s(0)
            for t in range(head, len(IM_STAGE)):
                stage_im(t)
            for i in range(1, S_TILES):
                stage_s(i)

            # ---------------- main matmul + region-max + word-sum ----------------
            def word_sum(wt):
                # scoresT[img, 4wt+jj] = sum_m maxima[(jj,m), wt, img] * wmask
                nc.tensor.matmul(
                    scoresT_ps[:, wt, :],
                    lhsT=maxima[:, wt, :],
                    rhs=w4[:, wt, :],
                    start=True, stop=True,
                )

            for wt in range(S_TILES):
                for rci, (toff, ntok, nimg) in enumerate(RCHUNKS):
                    pal = alp.tile([128, nimg, NREG], f32, tag="align")
                    for q in range(4):
                        nc.tensor.matmul(
                            pal.rearrange("p a b -> p (a b)"),
                            lhsT=sTraw_t[wt][:, q, :],
                            rhs=imP[:, q, :, toff:toff + ntok],
                            start=(q == 0),
                            stop=(q == 3),
                            perf_mode=SWI,
                        )
                    nc.vector.tensor_reduce(
                        maxima[:, wt, toff // NREG:toff // NREG + nimg],
                        pal, axis=AX.X, op=ALU.max,
                    )
                    # emit the previous tile's word-sum between rc chunks so the
                    # PE never waits on the vector MAX of the current tile
                    if rci == 0 and wt > 0:
                        word_sum(wt - 1)
            word_sum(S_TILES - 1)

            # scoresT -> SBUF, then transpose back to [sent, img]
            nc.vector.tensor_copy(scoresT_sb, scoresT_ps)
            if opts.get("debug"):
                w4d = mpool.tile([128, S_TILES, 4], f32)
                nc.vector.tensor_copy(w4d, w4)
                nc.sync.dma_start(dbg_w4[:, :, :], w4d)
                nc.sync.dma_start(dbg_scT[:, :, :], scoresT_sb)
                mxd = mpool.tile([128, S_TILES, BI], f32)
                nc.vector.tensor_copy(mxd, maxima)
                nc.sync.dma_start(dbg_mx[:, :, :], mxd)
            sc_ps = scp.tile([128, 2, BI], f32)
            for t in range(2):
                nc.tensor.transpose(
                    sc_ps[:, t, :],
                    scoresT_sb[:, 32 * t:32 * (t + 1), :].rearrange(
                        "p a b -> p (a b)"
                    ),
                    ident_f32[:BI, :BI],
                )
                nc.vector.tensor_copy(scores_sb[:, t, :], sc_ps[:, t, :])

        # ---------------- loss tail ----------------
        with ExitStack() as tail:
            tp = tail.enter_context(tc.tile_pool(name="tailp", bufs=1, space="PSUM"))
            ts = tail.enter_context(tc.tile_pool(name="tails", bufs=1))

            # col-max over local images (diag excluded) + scattered diag
            masked = ts.tile([128, 2, BI], f32)
            nc.vector.scalar_tensor_tensor(
                masked, dmask_sb, -BIG, scores_sb, op0=ALU.mult, op1=ALU.add
            )
            colmax_p = ts.tile([128, 2], f32)
            nc.vector.tensor_reduce(colmax_p, masked, axis=AX.X, op=ALU.max)
            dtmp = ts.tile([128, 2, BI], f32)
            nc.vector.tensor_mul(dtmp, dmask_sb, scores_sb)
            dscat = ts.tile([128, 2], f32)
            nc.vector.tensor_reduce(dscat, dtmp, axis=AX.X, op=ALU.add)

            # row-max over sentences (diag excluded); scoresT_sb is [img, sent]
            scT_flat = scoresT_sb.rearrange("p a b -> p (a b)")
            dmaskT_flat = dmaskT_sb.rearrange("p a b -> p (a b)")
            maskedT = ts.tile([BI, B], f32)
            nc.vector.scalar_tensor_tensor(
                maskedT, dmaskT_flat, -BIG, scT_flat, op0=ALU.mult, op1=ALU.add
            )
            rowmax = ts.tile([BI, 1], f32)
            nc.vector.tensor_reduce(rowmax, maskedT, axis=AX.X, op=ALU.max)
            dT_tmp = ts.tile([BI, B], f32)
            nc.vector.tensor_mul(dT_tmp, dmaskT_flat, scT_flat)
            d_row = ts.tile([BI, 1], f32)
            nc.vector.tensor_reduce(d_row, dT_tmp, axis=AX.X, op=ALU.add)

            cost_s = ts.tile([BI, 1], f32)
            nc.vector.tensor_sub(cost_s, rowmax, d_row)
            nc.vector.tensor_scalar(
                cost_s, cost_s, MARGIN, 0.0, op0=ALU.add, op1=ALU.max
            )
            cs_ps = tp.tile([1, 1], f32)
            nc.tensor.matmul(cs_ps, lhsT=ones32, rhs=cost_s, start=True, stop=True)
            cs_sb = ts.tile([1, 8], f32)
            nc.gpsimd.memset(cs_sb, 0.0)
            nc.vector.tensor_copy(cs_sb[:, 0:1], cs_ps)

            # pack allgather block: [0:256) colmax | [256:512) dscat | 512 cost_s
            blk = dram.tile([BLK], f32)
            nc.sync.dma_start(
                blk[0:256].rearrange("(t p) -> p t", p=128), colmax_p
            )
            nc.sync.dma_start(
                blk[256:512].rearrange("(t p) -> p t", p=128), dscat
            )
            nc.sync.dma_start(blk[512:520], cs_sb[0, :])
            gath = dram.tile([NCORES, BLK], f32, addr_space="Shared")
            nc.gpsimd.collective_compute(
                "AllGather",
                ALU.bypass,
                ins=[blk.opt()],
                outs=[gath.opt()],
                replica_groups=[list(range(NCORES))],
            )

            # redundant final reduction on every core
            g_cm = ts.tile([128, 2, NCORES], f32)
            g_d = ts.tile([128, 2, NCORES], f32)
            for t in range(2):
                nc.sync.dma_start(
                    g_cm[:, t, :],
                    gath[:, 128 * t:128 * (t + 1)].rearrange("c p -> p c"),
                )
                nc.sync.dma_start(
                    g_d[:, t, :],
                    gath[:, 256 + 128 * t:256 + 128 * (t + 1)].rearrange("c p -> p c"),
                )
            g_cs = ts.tile([1, NCORES], f32)
            nc.sync.dma_start(g_cs, gath[:, 512:513].rearrange("a b -> b a"))

            colmax_g = ts.tile([128, 2], f32)
            nc.vector.tensor_reduce(colmax_g, g_cm, axis=AX.X, op=ALU.max)
            d_all = ts.tile([128, 2], f32)
            nc.vector.tensor_reduce(d_all, g_d, axis=AX.X, op=ALU.add)
            cim = ts.tile([128, 2], f32)
            nc.vector.tensor_sub(cim, colmax_g, d_all)
            nc.vector.tensor_scalar(cim, cim, MARGIN, 0.0, op0=ALU.add, op1=ALU.max)
            cim_r = ts.tile([128, 1], f32)
            nc.vector.tensor_reduce(cim_r, cim, axis=AX.X, op=ALU.add)
            tot_ps = tp.tile([1, 1], f32)
            nc.tensor.matmul(tot_ps, lhsT=ones128, rhs=cim_r, start=True, stop=True)
            cs_tot = ts.tile([1, 1], f32)
            nc.vector.tensor_reduce(cs_tot, g_cs, axis=AX.X, op=ALU.add)
            total = ts.tile([1, 1], f32)
            nc.vector.tensor_add(total, tot_ps, cs_tot)
            nc.sync.dma_start(out_ext[0:1], total[0, :])

    fix_multiwaits(nc, mybir)
    return nc


_CACHE = {}


def _get_nc():
    if "nc" not in _CACHE:
        _CACHE["nc"] = build_graph()
    return _CACHE["nc"]


def make_in_maps(im_set, s_seq, im_len, s_len):
    im_set = np.ascontiguousarray(im_set, dtype=np.float32)
    s_seq = np.ascontiguousarray(s_seq, dtype=np.float32)
    im_len = np.ascontiguousarray(im_len, dtype=np.int32)
    s_len = np.ascontiguousarray(s_len, dtype=np.int32)
    in_maps = []
    for c in range(NCORES):
        dm = np.zeros((B, BI), dtype=np.float32)
        for i in range(BI):
            dm[BI * c + i, i] = 1.0
        in_maps.append({
            "im_set": im_set[BI * c:BI * (c + 1)],
            "s_seq": s_seq,
            "im_len": im_len[BI * c:BI * (c + 1)],
            "s_len": s_len,
            "diag_mask": dm,
            "diag_maskT": np.ascontiguousarray(dm.T),
        })
    return in_maps


def kernel(im_set, s_seq, im_len, s_len):
    import time
    from concourse.bass_utils import run_bass_kernel_spmd

    nc = _get_nc()
    in_maps = make_in_maps(im_set, s_seq, im_len, s_len)
    last = None
    for attempt in range(3):
        try:
            res = run_bass_kernel_spmd(nc, in_maps, core_ids=list(range(NCORES)))
            return np.asarray(
                res.results[0]["out"], dtype=np.float32
            ).reshape(())[()]
        except Exception as e:  # transient device-unrecoverable happens
            last = e
            time.sleep(30 * (attempt + 1))
    raise last


# revision 21
# speedup vs baseline: 1.0868x; 1.0868x over previous
"""Trainium2 Bass kernel for AlignmentContrastiveLoss (8 NeuronCores, SPMD).

Reference semantics:
  im = im_set[:, 1:, :]           [256, 36, 1024]
  s  = s_seq[:, 1:-2, :]          [256, 32, 1024]
  align[i,j,n,m] = im[i,n] . s[j,m], zeroed where n >= im_len[i]-1 or m >= s_len[j]-3
  scores[i,j] = sum_m max_n align[i,j,n,m]
  loss = sum_i relu(M + max_{j!=i} scores[i,j] - scores[i,i])
       + sum_j relu(M + max_{i!=j} scores[i,j] - scores[j,j])

Sharding: data-parallel over images (32 per core), s replicated.

v2 design:
  - f32 tokens are cast once to fp8e4 (im-mask fused as activation scale),
    bitcast to 16-bit fp8-pairs and transposed via the DMA XBAR (pure bit
    movement) into a packed layout: partition p of q-chunk q holds the d
    pair (256q+2p, 256q+2p+1) interleaved per token (HW-verified mapping).
  - s feeds the PE as RAW packed bytes via MatmulPerfMode.DoubleRowSwInterleave
    (stationary side accepts interleaved pairs; output partitions come out
    token-REVERSED, verified on HW). im (small) is deinterleaved to planar
    [128, q, 2, tok] fp8 by one gpsimd 4D copy per tile, which also compacts
    away the XBAR pad columns. No PE transposes, no PSUM->SBUF staging copies.
  - The s-token reversal is compensated in the word-sum weights (w4 built
    from a reversed word mask), so scoresT and the loss tail are unchanged.
  - wt-outer loop: per s-tile, 3 region-chunk matmul groups + vector MAX
    reduce; word-sum is a tiny PE matmul against s-mask-weighted block-ones
    (applies the s word mask for free and directly yields scoresT[img,sent]).
  - im staged as 11 tiles of 3 images (112-partition windows, 4-col overlap
    into the next tile's range which is later overwritten with real data).
  - Cross-core traffic: one 520-float AllGather of per-core column-max
    partials + scattered diagonals + local cost_s sum.
"""

import numpy as np

MARGIN = 0.2
B = 256          # global batch (images == sentences)
NCORES = 8
BI = B // NCORES  # images per core = 32
NREG = 36        # regions per image after stripping
NWORD = 32       # words per sentence after stripping
D = 1024
IM_TOK = BI * NREG      # 1152 dense im tokens
S_TOK = B * NWORD       # 8192 s tokens
S_TILES = S_TOK // 128  # 64
BIG = 1.0e30
# region chunks for the main matmul: (token offset, ntok, nimg)
RCHUNKS = [(0, 432, 12), (432, 432, 12), (864, 288, 8)]
# im staging tiles: (dense token offset, window (mult of 16), first image, n images)
IM_STAGE = [(108 * t, 112, 3 * t, 3) for t in range(10)] + [(1080, 80, 30, 2)]
IM_TP_COLS = 112 * 11  # padded XBAR destination: disjoint 112-col windows
BLK = 520  # allgather block floats: 256 colmax | 256 diag-scatter | 1 cost_s | pad


def fix_multiwaits(nc, mybir):
    """This toolchain's walrus accepts 1 wait per instruction (2 for
    EventSemaphore); Tile can emit more. Offload surplus waits onto
    inserted same-engine NoOps placed immediately before the instruction."""
    n_fix = 0
    for fn in nc.m.functions:
        for blk in fn.blocks:
            insts = blk.instructions
            i = 0
            while i < len(insts):
                inst = insts[i]
                si = inst.sync_info
                waits = list(si.on_wait) if si is not None and si.on_wait else []
                cap = 2 if isinstance(inst, mybir.InstEventSemaphore) else 1
                if len(waits) > cap:
                    surplus, keep = waits[:-cap], waits[-cap:]
                    si.on_wait = keep
                    for w in surplus:
                        nop = mybir.InstNoOp(
                            name=f"{inst.name}_wsplit{n_fix}",
                            engine=inst.engine,
                            ins=[],
                            outs=[],
                            sync_info=mybir.SyncInfo(on_wait=[w], on_update=[]),
                        )
                        insts.insert(i, nop)
                        n_fix += 1
                        i += 1
                i += 1
    return n_fix


DEFAULT_OPTS = {
    "sf_bufs": 12,     # f32 staging tiles
    "pk_bufs": 12,     # packed fp8-as-bf16 staging tiles
    "alp_bufs": 6,     # PSUM align buffers
    "gpsimd_cast": 3,  # every Nth s cast on vector (0 = all scalar)
    "s_prefetch": 8,   # s loads dispatched ahead of their pack stage
    "im_head": 4,      # im tiles staged before the first s tile
}


def build_graph(opts=None):
    import concourse.bass as bass
    import concourse.mybir as mybir
    import concourse.tile as tile
    from concourse.masks import make_identity
    from contextlib import ExitStack

    opts = {**DEFAULT_OPTS, **(opts or {})}

    f32 = mybir.dt.float32
    bf16 = mybir.dt.bfloat16
    fp8 = mybir.dt.float8e4
    i32 = mybir.dt.int32
    ALU = mybir.AluOpType
    AX = mybir.AxisListType
    ACTF = mybir.ActivationFunctionType
    SWI = mybir.MatmulPerfMode.DoubleRowSwInterleave

    nc = bass.Bass()

    im_ext = nc.declare_dram_parameter("im_set", [BI, 37, D], f32, isOutput=False)
    s_ext = nc.declare_dram_parameter("s_seq", [B, 35, D], f32, isOutput=False)
    imlen_ext = nc.declare_dram_parameter("im_len", [BI], i32, isOutput=False)
    slen_ext = nc.declare_dram_parameter("s_len", [B], i32, isOutput=False)
    dmask_ext = nc.declare_dram_parameter("diag_mask", [B, BI], f32, isOutput=False)
    dmaskT_ext = nc.declare_dram_parameter("diag_maskT", [BI, B], f32, isOutput=False)
    out_ext = nc.declare_dram_parameter("out", [1], f32, isOutput=True)
    if opts.get("debug"):
        dbg_sel = nc.declare_dram_parameter("dbg_sel", [128, 128], f32, isOutput=True)
        dbg_pm = nc.declare_dram_parameter("dbg_pm", [128, 128], f32, isOutput=True)
        dbg_rb = nc.declare_dram_parameter("dbg_rb", [128, 128], f32, isOutput=True)
        dbg_tc = nc.declare_dram_parameter("dbg_tc", [128, 2], f32, isOutput=True)
        dbg_bm = nc.declare_dram_parameter("dbg_bm", [2, 128, 128], f32, isOutput=True)
        dbg_w4 = nc.declare_dram_parameter("dbg_w4", [128, S_TILES, 4], f32, isOutput=True)
        dbg_scT = nc.declare_dram_parameter("dbg_scT", [BI, S_TILES, 4], f32, isOutput=True)
        dbg_mx = nc.declare_dram_parameter("dbg_mx", [128, S_TILES, BI], f32, isOutput=True)

    with tile.TileContext(nc) as tc, ExitStack() as top:
        # ---------------- constants ----------------
        const = top.enter_context(tc.tile_pool(name="const", bufs=1))
        ident_f32 = const.tile([128, 128], f32)
        make_identity(nc, ident_f32)
        ones32 = const.tile([32, 1], f32)
        nc.gpsimd.memset(ones32, 1.0)
        ones128 = const.tile([128, 1], f32)
        nc.gpsimd.memset(ones128, 1.0)

        # ---------------- token masks (device-side) ----------------
        mpool = top.enter_context(tc.tile_pool(name="masks", bufs=1))
        dram = top.enter_context(tc.tile_pool(name="dram", bufs=1, space="DRAM"))

        # per-image region mask [BI, NREG]: n < im_len-1
        imlen_sb = mpool.tile([BI, 1], i32)
        nc.sync.dma_start(imlen_sb, imlen_ext.rearrange("(p o) -> p o", o=1))
        il_sb = mpool.tile([BI, 1], f32)
        nc.gpsimd.tensor_scalar(il_sb, imlen_sb, 1, None, op0=ALU.subtract)
        iota_r = mpool.tile([BI, NREG], f32)
        nc.gpsimd.iota(
            iota_r, pattern=[[1, NREG]], base=0, channel_multiplier=0,
            allow_small_or_imprecise_dtypes=True,
        )
        mask_im = mpool.tile([BI, NREG], f32)
        nc.gpsimd.tensor_scalar(mask_im, iota_r, il_sb, None, op0=ALU.is_lt)
        # maskcol_im [128, 11]: per (partition, im stage tile) in REGION-major
        # stage order (p = ni*n + i); pad rows -> 0
        mask_im_dram = dram.tile([BI, NREG], f32)
        nc.sync.dma_start(mask_im_dram[:, :], mask_im)
        maskcol_im = mpool.tile([128, len(IM_STAGE)], f32)
        nc.gpsimd.memset(maskcol_im, 0.0)
        for t, (toff, win, i0, ni) in enumerate(IM_STAGE):
            nc.sync.dma_start(
                maskcol_im[0:36 * ni, t:t + 1],
                mask_im_dram[i0:i0 + ni, :].rearrange("i n -> n i"),
            )

        # s word masks -> w4 block-ones weights [128, 64, 4] bf16:
        #   w4[32*jj + m, wt, jj] = (m < s_len[4*wt+jj] - 3)
        slen_sb = mpool.tile([128, 2], i32)
        nc.sync.dma_start(slen_sb, slen_ext.rearrange("(t p) -> p t", p=128))
        sl_sb = mpool.tile([128, 2], f32)
        nc.gpsimd.tensor_scalar(sl_sb, slen_sb, 3, None, op0=ALU.subtract)
        iota_w = mpool.tile([128, NWORD], f32)
        nc.gpsimd.iota(
            iota_w, pattern=[[1, NWORD]], base=0, channel_multiplier=0,
            allow_small_or_imprecise_dtypes=True,
        )
        # Word-sum weights for WORD-major stage order (p = 4w + j) combined
        # with the SwInterleave token reversal (partition p <-> raw col 127-p):
        #   w4[p, wt, jj] = [ (127-p)%4 == jj ] * ( (127-p)//4 < sl[4*wt+jj] )
        # Built transposed (partition c = sentence-within-half, free p) then
        # PE-transposed into place.
        rb = mpool.tile([128, 128], f32)     # rb[c, p] = (127-p)//4
        nc.gpsimd.iota(rb, pattern=[[-1, 32], [0, 4]], base=31,
                       channel_multiplier=0, allow_small_or_imprecise_dtypes=True)
        # sel[c, p] = (p%4 == 3 - c%4)  <=>  ((c + p + 1) & 3 == 0)
        cp_i = mpool.tile([128, 128], i32)
        nc.gpsimd.iota(cp_i, pattern=[[1, 128]], base=1, channel_multiplier=1)
        cp_a = mpool.tile([128, 128], i32)
        nc.vector.tensor_scalar(cp_a, cp_i, 3, None, op0=ALU.bitwise_and)
        sel = mpool.tile([128, 128], f32)
        nc.vector.tensor_scalar(sel, cp_a, 0, None, op0=ALU.is_equal)
        w4 = mpool.tile([128, S_TILES, 4], bf16)
        with tc.tile_pool(name="w4ps", bufs=2, space="PSUM") as wps:
            for h in range(2):
                bh = mpool.tile([128, 128], f32, tag=f"w4bh{h}")
                nc.vector.tensor_scalar(
                    bh, rb, sl_sb[:, h:h + 1], None, op0=ALU.is_lt
                )
                bm = mpool.tile([128, 128], f32, tag=f"w4bm{h}")
                nc.vector.tensor_mul(bm, bh, sel)
                wt_ps = wps.tile([128, 128], f32, tag=f"w4t{h}")
                nc.tensor.transpose(wt_ps, bm, ident_f32)
                nc.vector.tensor_copy(
                    w4[:, 32 * h:32 * (h + 1), :].rearrange("p a b -> p (a b)"),
                    wt_ps,
                )
                if opts.get("debug"):
                    nc.sync.dma_start(dbg_bm[h, :, :], bm)
        if opts.get("debug"):
            nc.sync.dma_start(dbg_sel[:, :], sel)
            nc.sync.dma_start(dbg_rb[:, :], rb)

        # diag masks (sharding metadata inputs)
        dmask_sb = mpool.tile([128, 2, BI], f32)
        nc.sync.dma_start(dmask_sb, dmask_ext.rearrange("(t p) i -> p t i", p=128))
        dmaskT_sb = mpool.tile([BI, 2, 128], f32)
        nc.sync.dma_start(dmaskT_sb, dmaskT_ext.rearrange("p (t f) -> p t f", f=128))

        # ---------------- persistent big buffers ----------------
        big = top.enter_context(tc.tile_pool(name="big", bufs=1))
        # packed-transposed fp8 pairs, stored as bf16 bit containers.
        # One tile per staging unit so the (whole-tile) dependency tracking
        # of the XBAR DMA writes stays exactly per-tile.
        imTp_t = [big.tile([128, 4, 112], bf16, name=f"imTp{t}") for t in range(len(IM_STAGE))]
        imP = big.tile([128, 4, 2, IM_TOK], fp8)      # dense planar im
        sTp_t = [big.tile([128, 4, 128], bf16, name=f"sTp{i}") for i in range(S_TILES)]
        maxima = big.tile([128, S_TILES, BI], bf16)  # per (word, wtile, img) region-max
        scoresT_sb = big.tile([BI, S_TILES, 4], f32)  # [img, wt, jj] == [img, sent]
        scores_sb = big.tile([128, 2, BI], f32)       # [sent%128, sent//128, img]

        # fp8 views: im pair-split for the deinterleave, s raw for SwInterleave
        imTp8_t = [
            t.bitcast(fp8).rearrange("p q (t b) -> p q b t", b=2) for t in imTp_t
        ]
        sTraw_t = [t.bitcast(fp8) for t in sTp_t]     # each [128, 4, 256]

        with ExitStack() as mid:
            stage = mid.enter_context(
                tc.tile_pool(name="stage", bufs=opts["sf_bufs"])
            )
            pk = mid.enter_context(tc.tile_pool(name="pk", bufs=opts["pk_bufs"]))
            alp = mid.enter_context(
                tc.tile_pool(name="alp", bufs=opts["alp_bufs"], space="PSUM")
            )
            scp = mid.enter_context(tc.tile_pool(name="scp", bufs=1, space="PSUM"))
            scoresT_ps = scp.tile([BI, S_TILES, 4], f32)

            def stage_im(t):
                toff, win, i0, ni = IM_STAGE[t]
                nreal = 36 * ni
                tf32 = stage.tile([128, D], f32, tag="sf32")
                nc.sync.dma_start(
                    tf32[0:nreal, :],
                    im_ext[i0:i0 + ni, 1:1 + NREG, :].rearrange("i n d -> n i d"),
                )
                tbf = pk.tile([128, D // 2], bf16, tag="spk")
                nc.scalar.activation(
                    tbf.bitcast(fp8)[0:win, :], tf32[0:win, :],
                    ACTF.Copy, scale=maskcol_im[0:win, t:t + 1],
                )
                # XBAR into the tile's private buffer
                nc.scalar.dma_start(
                    imTp_t[t][:, :, 0:win], tbf[0:win, :], transpose=True
                )
                # deinterleave + compact + un-permute (region-major stage
                # order ni*n + i -> dense 36*i + n) in one strided copy
                nc.gpsimd.tensor_copy(
                    imP[:, :, :, toff:toff + nreal].rearrange(
                        "p q b (i n) -> p q b i n", n=NREG
                    ),
                    imTp8_t[t][:, :, :, 0:nreal].rearrange(
                        "p q b (n i) -> p q b n i", i=ni
                    ).rearrange("p q b n i -> p q b i n"),
                )

            def s_load(i):
                tf32 = stage.tile([128, D], f32, tag="sf32")
                nc.sync.dma_start(
                    tf32,
                    s_ext[4 * i:4 * i + 4, 1:1 + NWORD, :].rearrange(
                        "j w d -> w j d"
                    ),
                )
                return tf32

            def s_pack(i, tf32):
                tbf = pk.tile([128, D // 2], bf16, tag="spk")
                g = opts["gpsimd_cast"]
                if g and (i % g == g - 1):
                    # vector-cast tiles get their transpose dispatched from
                    # sync (loads are prefetched ahead, so no head-of-line
                    # blocking); scalar-cast tiles pair cast+T on scalar
                    nc.vector.tensor_copy(tbf.bitcast(fp8), tf32)
                    nc.sync.dma_start(sTp_t[i], tbf, transpose=True)
                else:
                    nc.scalar.activation(tbf.bitcast(fp8), tf32, ACTF.Copy)
                    nc.scalar.dma_start(sTp_t[i], tbf, transpose=True)

            # im head first (needed by rc0); s loads run ahead of their
            # cast+transpose by PRE tiles so no dispatch queue ever blocks
            head = opts["im_head"]
            PRE = opts["s_prefetch"]
            for t in range(head):
                stage_im(t)
            pending = []
            for i in range(S_TILES):
                pending.append((i, s_load(i)))
                if i == PRE - 1:
                    for t in range(head, len(IM_STAGE)):
                        stage_im(t)
                if len(pending) > PRE:
                    s_pack(*pending.pop(0))
            for it in pending:
                s_pack(*it)

            # ---------------- main matmul + region-max + word-sum ----------------
            def word_sum(wt):
                # scoresT[img, 4wt+jj] = sum_m maxima[(jj,m), wt, img] * wmask
                nc.tensor.matmul(
                    scoresT_ps[:, wt, :],
                    lhsT=maxima[:, wt, :],
                    rhs=w4[:, wt, :],
                    start=True, stop=True,
                )

            for wt in range(S_TILES):
                for rci, (toff, ntok, nimg) in enumerate(RCHUNKS):
                    pal = alp.tile([128, nimg, NREG], f32, tag="align")
                    for q in range(4):
                        nc.tensor.matmul(
                            pal.rearrange("p a b -> p (a b)"),
                            lhsT=sTraw_t[wt][:, q, :],
                            rhs=imP[:, q, :, toff:toff + ntok],
                            start=(q == 0),
                            stop=(q == 3),
                            perf_mode=SWI,
                        )
                    nc.vector.tensor_reduce(
                        maxima[:, wt, toff // NREG:toff // NREG + nimg],
                        pal, axis=AX.X, op=ALU.max,
                    )
                    # emit the previous tile's word-sum between rc chunks so the
                    # PE never waits on the vector MAX of the current tile
                    if rci == 0 and wt > 0:
                        word_sum(wt - 1)
            word_sum(S_TILES - 1)

            # scoresT -> SBUF, then transpose back to [sent, img]
            nc.vector.tensor_copy(scoresT_sb, scoresT_ps)
            if opts.get("debug"):
                w4d = mpool.tile([128, S_TILES, 4], f32)
                nc.vector.tensor_copy(w4d, w4)
                nc.sync.dma_start(dbg_w4[:, :, :], w4d)
                nc.sync.dma_start(dbg_scT[:, :, :], scoresT_sb)
                mxd = mpool.tile([128, S_TILES, BI], f32)
                nc.vector.tensor_copy(mxd, maxima)
                nc.sync.dma_start(dbg_mx[:, :, :], mxd)
            sc_ps = scp.tile([128, 2, BI], f32)
            for t in range(2):
                nc.tensor.transpose(
                    sc_ps[:, t, :],
                    scoresT_sb[:, 32 * t:32 * (t + 1), :].rearrange(
                        "p a b -> p (a b)"
                    ),
                    ident_f32[:BI, :BI],
                )
                nc.vector.tensor_copy(scores_sb[:, t, :], sc_ps[:, t, :])

        # ---------------- loss tail ----------------
        with ExitStack() as tail:
            tp = tail.enter_context(tc.tile_pool(name="tailp", bufs=1, space="PSUM"))
            ts = tail.enter_context(tc.tile_pool(name="tails", bufs=1))

            # col-max over local images (diag excluded) + scattered diag
            masked = ts.tile([128, 2, BI], f32)
            nc.vector.scalar_tensor_tensor(
                masked, dmask_sb, -BIG, scores_sb, op0=ALU.mult, op1=ALU.add
            )
            colmax_p = ts.tile([128, 2], f32)
            nc.vector.tensor_reduce(colmax_p, masked, axis=AX.X, op=ALU.max)
            dtmp = ts.tile([128, 2, BI], f32)
            nc.vector.tensor_mul(dtmp, dmask_sb, scores_sb)
            dscat = ts.tile([128, 2], f32)
            nc.vector.tensor_reduce(dscat, dtmp, axis=AX.X, op=ALU.add)

            # row-max over sentences (diag excluded); scoresT_sb is [img, sent]
            scT_flat = scoresT_sb.rearrange("p a b -> p (a b)")
            dmaskT_flat = dmaskT_sb.rearrange("p a b -> p (a b)")
            maskedT = ts.tile([BI, B], f32)
            nc.vector.scalar_tensor_tensor(
                maskedT, dmaskT_flat, -BIG, scT_flat, op0=ALU.mult, op1=ALU.add
            )
            rowmax = ts.tile([BI, 1], f32)
            nc.vector.tensor_reduce(rowmax, maskedT, axis=AX.X, op=ALU.max)
            dT_tmp = ts.tile([BI, B], f32)
            nc.vector.tensor_mul(dT_tmp, dmaskT_flat, scT_flat)
            d_row = ts.tile([BI, 1], f32)
            nc.vector.tensor_reduce(d_row, dT_tmp, axis=AX.X, op=ALU.add)

            cost_s = ts.tile([BI, 1], f32)
            nc.vector.tensor_sub(cost_s, rowmax, d_row)
            nc.vector.tensor_scalar(
                cost_s, cost_s, MARGIN, 0.0, op0=ALU.add, op1=ALU.max
            )
            cs_ps = tp.tile([1, 1], f32)
            nc.tensor.matmul(cs_ps, lhsT=ones32, rhs=cost_s, start=True, stop=True)
            cs_sb = ts.tile([1, 8], f32)
            nc.gpsimd.memset(cs_sb, 0.0)
            nc.vector.tensor_copy(cs_sb[:, 0:1], cs_ps)

            # pack allgather block: [0:256) colmax | [256:512) dscat | 512 cost_s
            blk = dram.tile([BLK], f32)
            nc.sync.dma_start(
                blk[0:256].rearrange("(t p) -> p t", p=128), colmax_p
            )
            nc.sync.dma_start(
                blk[256:512].rearrange("(t p) -> p t", p=128), dscat
            )
            nc.sync.dma_start(blk[512:520], cs_sb[0, :])
            gath = dram.tile([NCORES, BLK], f32, addr_space="Shared")
            nc.gpsimd.collective_compute(
                "AllGather",
                ALU.bypass,
                ins=[blk.opt()],
                outs=[gath.opt()],
                replica_groups=[list(range(NCORES))],
            )

            # redundant final reduction on every core
            g_cm = ts.tile([128, 2, NCORES], f32)
            g_d = ts.tile([128, 2, NCORES], f32)
            for t in range(2):
                nc.sync.dma_start(
                    g_cm[:, t, :],
                    gath[:, 128 * t:128 * (t + 1)].rearrange("c p -> p c"),
                )
                nc.sync.dma_start(
                    g_d[:, t, :],
                    gath[:, 256 + 128 * t:256 + 128 * (t + 1)].rearrange("c p -> p c"),
                )
            g_cs = ts.tile([1, NCORES], f32)
            nc.sync.dma_start(g_cs, gath[:, 512:513].rearrange("a b -> b a"))

            colmax_g = ts.tile([128, 2], f32)
            nc.vector.tensor_reduce(colmax_g, g_cm, axis=AX.X, op=ALU.max)
            d_all = ts.tile([128, 2], f32)
            nc.vector.tensor_reduce(d_all, g_d, axis=AX.X, op=ALU.add)
            cim = ts.tile([128, 2], f32)
            nc.vector.tensor_sub(cim, colmax_g, d_all)
            nc.vector.tensor_scalar(cim, cim, MARGIN, 0.0, op0=ALU.add, op1=ALU.max)
            cim_r = ts.tile([128, 1], f32)
            nc.vector.tensor_reduce(cim_r, cim, axis=AX.X, op=ALU.add)
            tot_ps = tp.tile([1, 1], f32)
            nc.tensor.matmul(tot_ps, lhsT=ones128, rhs=cim_r, start=True, stop=True)
            cs_tot = ts.tile([1, 1], f32)
            nc.vector.tensor_reduce(cs_tot, g_cs, axis=AX.X, op=ALU.add)
            total = ts.tile([1, 1], f32)
            nc.vector.tensor_add(total, tot_ps, cs_tot)
            nc.sync.dma_start(out_ext[0:1], total[0, :])

    fix_multiwaits(nc, mybir)
    return nc


_CACHE = {}


def _get_nc():
    if "nc" not in _CACHE:
        _CACHE["nc"] = build_graph()
    return _CACHE["nc"]


def make_in_maps(im_set, s_seq, im_len, s_len):
    im_set = np.ascontiguousarray(im_set, dtype=np.float32)
    s_seq = np.ascontiguousarray(s_seq, dtype=np.float32)
    im_len = np.ascontiguousarray(im_len, dtype=np.int32)
    s_len = np.ascontiguousarray(s_len, dtype=np.int32)
    in_maps = []
    for c in range(NCORES):
        dm = np.zeros((B, BI), dtype=np.float32)
        for i in range(BI):
            dm[BI * c + i, i] = 1.0
        in_maps.append({
            "im_set": im_set[BI * c:BI * (c + 1)],
            "s_seq": s_seq,
            "im_len": im_len[BI * c:BI * (c + 1)],
            "s_len": s_len,
            "diag_mask": dm,
            "diag_maskT": np.ascontiguousarray(dm.T),
        })
    return in_maps


def kernel(im_set, s_seq, im_len, s_len):
    import time
    from concourse.bass_utils import run_bass_kernel_spmd

    nc = _get_nc()
    in_maps = make_in_maps(im_set, s_seq, im_len, s_len)
    last = None
    for attempt in range(3):
        try:
            res = run_bass_kernel_spmd(nc, in_maps, core_ids=list(range(NCORES)))
            return np.asarray(
                res.results[0]["out"], dtype=np.float32
            ).reshape(())[()]
        except Exception as e:  # transient device-unrecoverable happens
            last = e
            time.sleep(30 * (attempt + 1))
    raise last


# revision 23
# speedup vs baseline: 1.4297x; 1.3155x over previous
"""Trainium2 Bass kernel for AlignmentContrastiveLoss (8 NeuronCores, SPMD).

Reference semantics:
  im = im_set[:, 1:, :]           [256, 36, 1024]
  s  = s_seq[:, 1:-2, :]          [256, 32, 1024]
  align[i,j,n,m] = im[i,n] . s[j,m], zeroed where n >= im_len[i]-1 or m >= s_len[j]-3
  scores[i,j] = sum_m max_n align[i,j,n,m]
  loss = sum_i relu(M + max_{j!=i} scores[i,j] - scores[i,i])
       + sum_j relu(M + max_{i!=j} scores[i,j] - scores[j,j])

Sharding: data-parallel over images (32 per core), s replicated.

v2 design:
  - f32 tokens are cast once to fp8e4 (im-mask fused as activation scale),
    bitcast to 16-bit fp8-pairs and transposed via the DMA XBAR (pure bit
    movement) into a packed layout: partition p of q-chunk q holds the d
    pair (256q+2p, 256q+2p+1) interleaved per token (HW-verified mapping).
  - s feeds the PE as RAW packed bytes via MatmulPerfMode.DoubleRowSwInterleave
    (stationary side accepts interleaved pairs; output partitions come out
    token-REVERSED, verified on HW). im (small) is deinterleaved to planar
    [128, q, 2, tok] fp8 by one gpsimd 4D copy per tile, which also compacts
    away the XBAR pad columns. No PE transposes, no PSUM->SBUF staging copies.
  - The s-token reversal is compensated in the word-sum weights (w4 built
    from a reversed word mask), so scoresT and the loss tail are unchanged.
  - wt-outer loop: per s-tile, 3 region-chunk matmul groups + vector MAX
    reduce; word-sum is a tiny PE matmul against s-mask-weighted block-ones
    (applies the s word mask for free and directly yields scoresT[img,sent]).
  - im staged as 11 tiles of 3 images (112-partition windows, 4-col overlap
    into the next tile's range which is later overwritten with real data).
  - Cross-core traffic: one 520-float AllGather of per-core column-max
    partials + scattered diagonals + local cost_s sum.
"""

import numpy as np

MARGIN = 0.2
B = 256          # global batch (images == sentences)
NCORES = 8
BI = B // NCORES  # images per core = 32
NREG = 36        # regions per image after stripping
NWORD = 32       # words per sentence after stripping
D = 1024
IM_TOK = BI * NREG      # 1152 dense im tokens
S_TOK = B * NWORD       # 8192 s tokens
S_TILES = S_TOK // 128  # 64
BIG = 1.0e30
# region chunks for the main matmul: (token offset, ntok, nimg)
RCHUNKS = [(0, 432, 12), (432, 432, 12), (864, 288, 8)]
# im staging tiles: (dense token offset, window (mult of 16), first image, n images)
IM_STAGE = [(108 * t, 112, 3 * t, 3) for t in range(10)] + [(1080, 80, 30, 2)]
IM_TP_COLS = 112 * 11  # padded XBAR destination: disjoint 112-col windows
BLK = 520  # allgather block floats: 256 colmax | 256 diag-scatter | 1 cost_s | pad


def fix_multiwaits(nc, mybir):
    """This toolchain's walrus accepts 1 wait per instruction (2 for
    EventSemaphore); Tile can emit more. Offload surplus waits onto
    inserted same-engine NoOps placed immediately before the instruction."""
    n_fix = 0
    for fn in nc.m.functions:
        for blk in fn.blocks:
            insts = blk.instructions
            i = 0
            while i < len(insts):
                inst = insts[i]
                si = inst.sync_info
                waits = list(si.on_wait) if si is not None and si.on_wait else []
                cap = 2 if isinstance(inst, mybir.InstEventSemaphore) else 1
                if len(waits) > cap:
                    surplus, keep = waits[:-cap], waits[-cap:]
                    si.on_wait = keep
                    for w in surplus:
                        nop = mybir.InstNoOp(
                            name=f"{inst.name}_wsplit{n_fix}",
                            engine=inst.engine,
                            ins=[],
                            outs=[],
                            sync_info=mybir.SyncInfo(on_wait=[w], on_update=[]),
                        )
                        insts.insert(i, nop)
                        n_fix += 1
                        i += 1
                i += 1
    return n_fix


DEFAULT_OPTS = {
    "sf_bufs": 16,     # f32 staging tiles
    "pk_bufs": 5,      # packed fp8-as-bf16 staging quad tiles
    "alp_bufs": 6,     # PSUM align buffers
    "gpsimd_cast": 3,  # every Nth s cast on vector (0 = all scalar)
    "s_prefetch": 3,   # s quad-loads dispatched ahead of their pack stage
    "im_head": 4,      # im tiles staged before the first s tile
}


def build_graph(opts=None):
    import concourse.bass as bass
    import concourse.mybir as mybir
    import concourse.tile as tile
    from concourse.masks import make_identity
    from contextlib import ExitStack

    opts = {**DEFAULT_OPTS, **(opts or {})}

    f32 = mybir.dt.float32
    bf16 = mybir.dt.bfloat16
    fp8 = mybir.dt.float8e4
    i32 = mybir.dt.int32
    ALU = mybir.AluOpType
    AX = mybir.AxisListType
    ACTF = mybir.ActivationFunctionType
    SWI = mybir.MatmulPerfMode.DoubleRowSwInterleave

    nc = bass.Bass()

    im_ext = nc.declare_dram_parameter("im_set", [BI, 37, D], f32, isOutput=False)
    s_ext = nc.declare_dram_parameter("s_seq", [B, 35, D], f32, isOutput=False)
    imlen_ext = nc.declare_dram_parameter("im_len", [BI], i32, isOutput=False)
    slen_ext = nc.declare_dram_parameter("s_len", [B], i32, isOutput=False)
    dmask_ext = nc.declare_dram_parameter("diag_mask", [B, BI], f32, isOutput=False)
    dmaskT_ext = nc.declare_dram_parameter("diag_maskT", [BI, B], f32, isOutput=False)
    out_ext = nc.declare_dram_parameter("out", [1], f32, isOutput=True)
    if opts.get("debug"):
        dbg_sel = nc.declare_dram_parameter("dbg_sel", [128, 128], f32, isOutput=True)
        dbg_pm = nc.declare_dram_parameter("dbg_pm", [128, 128], f32, isOutput=True)
        dbg_rb = nc.declare_dram_parameter("dbg_rb", [128, 128], f32, isOutput=True)
        dbg_tc = nc.declare_dram_parameter("dbg_tc", [128, 2], f32, isOutput=True)
        dbg_bm = nc.declare_dram_parameter("dbg_bm", [2, 128, 128], f32, isOutput=True)
        dbg_w4 = nc.declare_dram_parameter("dbg_w4", [128, S_TILES, 4], f32, isOutput=True)
        dbg_scT = nc.declare_dram_parameter("dbg_scT", [BI, S_TILES, 4], f32, isOutput=True)
        dbg_mx = nc.declare_dram_parameter("dbg_mx", [128, S_TILES, BI], f32, isOutput=True)

    with tile.TileContext(nc) as tc, ExitStack() as top:
        # ---------------- constants ----------------
        const = top.enter_context(tc.tile_pool(name="const", bufs=1))
        ident_f32 = const.tile([128, 128], f32)
        make_identity(nc, ident_f32)
        ones32 = const.tile([32, 1], f32)
        nc.gpsimd.memset(ones32, 1.0)
        ones128 = const.tile([128, 1], f32)
        nc.gpsimd.memset(ones128, 1.0)

        # ---------------- token masks (device-side) ----------------
        mpool = top.enter_context(tc.tile_pool(name="masks", bufs=1))
        dram = top.enter_context(tc.tile_pool(name="dram", bufs=1, space="DRAM"))

        # per-image region mask [BI, NREG]: n < im_len-1
        imlen_sb = mpool.tile([BI, 1], i32)
        nc.sync.dma_start(imlen_sb, imlen_ext.rearrange("(p o) -> p o", o=1))
        il_sb = mpool.tile([BI, 1], f32)
        nc.gpsimd.tensor_scalar(il_sb, imlen_sb, 1, None, op0=ALU.subtract)
        iota_r = mpool.tile([BI, NREG], f32)
        nc.gpsimd.iota(
            iota_r, pattern=[[1, NREG]], base=0, channel_multiplier=0,
            allow_small_or_imprecise_dtypes=True,
        )
        mask_im = mpool.tile([BI, NREG], f32)
        nc.gpsimd.tensor_scalar(mask_im, iota_r, il_sb, None, op0=ALU.is_lt)
        # maskcol_im [128, 11]: per (partition, im stage tile) in REGION-major
        # stage order (p = ni*n + i); pad rows -> 0
        mask_im_dram = dram.tile([BI, NREG], f32)
        nc.sync.dma_start(mask_im_dram[:, :], mask_im)
        maskcol_im = mpool.tile([128, len(IM_STAGE)], f32)
        nc.gpsimd.memset(maskcol_im, 0.0)
        for t, (toff, win, i0, ni) in enumerate(IM_STAGE):
            nc.sync.dma_start(
                maskcol_im[0:36 * ni, t:t + 1],
                mask_im_dram[i0:i0 + ni, :].rearrange("i n -> n i"),
            )

        # s word masks -> w4 block-ones weights [128, 64, 4] bf16:
        #   w4[32*jj + m, wt, jj] = (m < s_len[4*wt+jj] - 3)
        slen_sb = mpool.tile([128, 2], i32)
        nc.sync.dma_start(slen_sb, slen_ext.rearrange("(t p) -> p t", p=128))
        sl_sb = mpool.tile([128, 2], f32)
        nc.gpsimd.tensor_scalar(sl_sb, slen_sb, 3, None, op0=ALU.subtract)
        iota_w = mpool.tile([128, NWORD], f32)
        nc.gpsimd.iota(
            iota_w, pattern=[[1, NWORD]], base=0, channel_multiplier=0,
            allow_small_or_imprecise_dtypes=True,
        )
        # Word-sum weights for WORD-major stage order (p = 4w + j) combined
        # with the SwInterleave token reversal (partition p <-> raw col 127-p):
        #   w4[p, wt, jj] = [ (127-p)%4 == jj ] * ( (127-p)//4 < sl[4*wt+jj] )
        # Built transposed (partition c = sentence-within-half, free p) then
        # PE-transposed into place.
        rb = mpool.tile([128, 128], f32)     # rb[c, p] = (127-p)//4
        nc.gpsimd.iota(rb, pattern=[[-1, 32], [0, 4]], base=31,
                       channel_multiplier=0, allow_small_or_imprecise_dtypes=True)
        # sel[c, p] = (p%4 == 3 - c%4)  <=>  ((c + p + 1) & 3 == 0)
        cp_i = mpool.tile([128, 128], i32)
        nc.gpsimd.iota(cp_i, pattern=[[1, 128]], base=1, channel_multiplier=1)
        cp_a = mpool.tile([128, 128], i32)
        nc.vector.tensor_scalar(cp_a, cp_i, 3, None, op0=ALU.bitwise_and)
        sel = mpool.tile([128, 128], f32)
        nc.vector.tensor_scalar(sel, cp_a, 0, None, op0=ALU.is_equal)
        w4 = mpool.tile([128, S_TILES, 4], bf16)
        with tc.tile_pool(name="w4ps", bufs=2, space="PSUM") as wps:
            for h in range(2):
                bh = mpool.tile([128, 128], f32, tag=f"w4bh{h}")
                nc.vector.tensor_scalar(
                    bh, rb, sl_sb[:, h:h + 1], None, op0=ALU.is_lt
                )
                bm = mpool.tile([128, 128], f32, tag=f"w4bm{h}")
                nc.vector.tensor_mul(bm, bh, sel)
                wt_ps = wps.tile([128, 128], f32, tag=f"w4t{h}")
                nc.tensor.transpose(wt_ps, bm, ident_f32)
                nc.vector.tensor_copy(
                    w4[:, 32 * h:32 * (h + 1), :].rearrange("p a b -> p (a b)"),
                    wt_ps,
                )
                if opts.get("debug"):
                    nc.sync.dma_start(dbg_bm[h, :, :], bm)
        if opts.get("debug"):
            nc.sync.dma_start(dbg_sel[:, :], sel)
            nc.sync.dma_start(dbg_rb[:, :], rb)

        # diag masks (sharding metadata inputs)
        dmask_sb = mpool.tile([128, 2, BI], f32)
        nc.sync.dma_start(dmask_sb, dmask_ext.rearrange("(t p) i -> p t i", p=128))
        dmaskT_sb = mpool.tile([BI, 2, 128], f32)
        nc.sync.dma_start(dmaskT_sb, dmaskT_ext.rearrange("p (t f) -> p t f", f=128))

        # ---------------- persistent big buffers ----------------
        big = top.enter_context(tc.tile_pool(name="big", bufs=1))
        # packed-transposed fp8 pairs, stored as bf16 bit containers.
        # One tile per staging unit so the (whole-tile) dependency tracking
        # of the XBAR DMA writes stays exactly per-tile.
        imTp_t = [big.tile([128, 4, 112], bf16, name=f"imTp{t}") for t in range(len(IM_STAGE))]
        imP = big.tile([128, 4, 2, IM_TOK], fp8)      # dense planar im
        sTp_g = [
            big.tile([128, 4, 4, 128], bf16, name=f"sTpg{g}")
            for g in range(S_TILES // 4)
        ]
        maxima = big.tile([128, S_TILES, BI], bf16)  # per (word, wtile, img) region-max
        scoresT_sb = big.tile([BI, S_TILES, 4], f32)  # [img, wt, jj] == [img, sent]
        scores_sb = big.tile([128, 2, BI], f32)       # [sent%128, sent//128, img]

        # fp8 views: im pair-split for the deinterleave, s raw for SwInterleave
        imTp8_t = [
            t.bitcast(fp8).rearrange("p q (t b) -> p q b t", b=2) for t in imTp_t
        ]
        sTraw_g = [t.bitcast(fp8) for t in sTp_g]     # each [128, 4, 4, 256]

        with ExitStack() as mid:
            stage = mid.enter_context(
                tc.tile_pool(name="stage", bufs=opts["sf_bufs"])
            )
            pk = mid.enter_context(tc.tile_pool(name="pk", bufs=opts["pk_bufs"]))
            alp = mid.enter_context(
                tc.tile_pool(name="alp", bufs=opts["alp_bufs"], space="PSUM")
            )
            scp = mid.enter_context(tc.tile_pool(name="scp", bufs=1, space="PSUM"))
            scoresT_ps = scp.tile([BI, S_TILES, 4], f32)

            def stage_im(t):
                toff, win, i0, ni = IM_STAGE[t]
                nreal = 36 * ni
                tf32 = stage.tile([128, D], f32, tag="sf32")
                nc.sync.dma_start(
                    tf32[0:nreal, :],
                    im_ext[i0:i0 + ni, 1:1 + NREG, :].rearrange("i n d -> n i d"),
                )
                tbf = pk.tile([128, D // 2], bf16, tag="spk")
                nc.scalar.activation(
                    tbf.bitcast(fp8)[0:win, :], tf32[0:win, :],
                    ACTF.Copy, scale=maskcol_im[0:win, t:t + 1],
                )
                # XBAR into the tile's private buffer
                nc.scalar.dma_start(
                    imTp_t[t][:, :, 0:win], tbf[0:win, :], transpose=True
                )
                # deinterleave + compact + un-permute (region-major stage
                # order ni*n + i -> dense 36*i + n) in one strided copy
                nc.gpsimd.tensor_copy(
                    imP[:, :, :, toff:toff + nreal].rearrange(
                        "p q b (i n) -> p q b i n", n=NREG
                    ),
                    imTp8_t[t][:, :, :, 0:nreal].rearrange(
                        "p q b (n i) -> p q b n i", i=ni
                    ).rearrange("p q b n i -> p q b i n"),
                )

            def s_load(gq):
                # four per-tile loads (the DMA AP balancer caps at 3 dims,
                # so a quad can't be one DMA), word-major per tile
                tfs = []
                for a in range(4):
                    i = 4 * gq + a
                    tf32 = stage.tile([128, D], f32, tag="sf32")
                    nc.sync.dma_start(
                        tf32,
                        s_ext[4 * i:4 * i + 4, 1:1 + NWORD, :].rearrange(
                            "j w d -> w j d"
                        ),
                    )
                    tfs.append(tf32)
                return tfs

            def s_pack(gq, tfs):
                pkq = pk.tile([128, 4, D // 2], bf16, tag="spk")
                gsp = opts["gpsimd_cast"]
                for a in range(4):
                    i = 4 * gq + a
                    if gsp and (i % gsp == gsp - 1):
                        nc.vector.tensor_copy(
                            pkq[:, a, :].bitcast(fp8), tfs[a]
                        )
                    else:
                        nc.scalar.activation(
                            pkq[:, a, :].bitcast(fp8), tfs[a], ACTF.Copy
                        )
                eng = nc.sync if (gq % 2 == 0) else nc.scalar
                eng.dma_start(
                    sTp_g[gq].rearrange("p a q t -> p (a q) t"),
                    pkq.rearrange("p a c -> p (a c)"),
                    transpose=True,
                )

            # im head first (needed by rc0); s quad-loads run ahead of their
            # cast+transpose stages so no dispatch queue ever blocks
            head = opts["im_head"]
            PRE = opts["s_prefetch"]
            for t in range(head):
                stage_im(t)
            pending = []
            for gq in range(S_TILES // 4):
                pending.append((gq, s_load(gq)))
                if gq == PRE - 1:
                    for t in range(head, len(IM_STAGE)):
                        stage_im(t)
                if len(pending) > PRE:
                    s_pack(*pending.pop(0))
            for it in pending:
                s_pack(*it)

            # ---------------- main matmul + region-max + word-sum ----------------
            def word_sum(wt):
                # scoresT[img, 4wt+jj] = sum_m maxima[(jj,m), wt, img] * wmask
                nc.tensor.matmul(
                    scoresT_ps[:, wt, :],
                    lhsT=maxima[:, wt, :],
                    rhs=w4[:, wt, :],
                    start=True, stop=True,
                )

            for wt in range(S_TILES):
                for rci, (toff, ntok, nimg) in enumerate(RCHUNKS):
                    pal = alp.tile([128, nimg, NREG], f32, tag="align")
                    for q in range(4):
                        nc.tensor.matmul(
                            pal.rearrange("p a b -> p (a b)"),
                            lhsT=sTraw_g[wt // 4][:, wt % 4, q, :],
                            rhs=imP[:, q, :, toff:toff + ntok],
                            start=(q == 0),
                            stop=(q == 3),
                            perf_mode=SWI,
                        )
                    nc.vector.tensor_reduce(
                        maxima[:, wt, toff // NREG:toff // NREG + nimg],
                        pal, axis=AX.X, op=ALU.max,
                    )
                    # emit the previous tile's word-sum between rc chunks so the
                    # PE never waits on the vector MAX of the current tile
                    if rci == 0 and wt > 0:
                        word_sum(wt - 1)
            word_sum(S_TILES - 1)

            # scoresT -> SBUF, then transpose back to [sent, img]
            nc.vector.tensor_copy(scoresT_sb, scoresT_ps)
            if opts.get("debug"):
                w4d = mpool.tile([128, S_TILES, 4], f32)
                nc.vector.tensor_copy(w4d, w4)
                nc.sync.dma_start(dbg_w4[:, :, :], w4d)
                nc.sync.dma_start(dbg_scT[:, :, :], scoresT_sb)
                mxd = mpool.tile([128, S_TILES, BI], f32)
                nc.vector.tensor_copy(mxd, maxima)
                nc.sync.dma_start(dbg_mx[:, :, :], mxd)
            sc_ps = scp.tile([128, 2, BI], f32)
            for t in range(2):
                nc.tensor.transpose(
                    sc_ps[:, t, :],
                    scoresT_sb[:, 32 * t:32 * (t + 1), :].rearrange(
                        "p a b -> p (a b)"
                    ),
                    ident_f32[:BI, :BI],
                )
                nc.vector.tensor_copy(scores_sb[:, t, :], sc_ps[:, t, :])

        # ---------------- loss tail ----------------
        with ExitStack() as tail:
            tp = tail.enter_context(tc.tile_pool(name="tailp", bufs=1, space="PSUM"))
            ts = tail.enter_context(tc.tile_pool(name="tails", bufs=1))

            # col-max over local images (diag excluded) + scattered diag
            masked = ts.tile([128, 2, BI], f32)
            nc.vector.scalar_tensor_tensor(
                masked, dmask_sb, -BIG, scores_sb, op0=ALU.mult, op1=ALU.add
            )
            colmax_p = ts.tile([128, 2], f32)
            nc.vector.tensor_reduce(colmax_p, masked, axis=AX.X, op=ALU.max)
            dtmp = ts.tile([128, 2, BI], f32)
            nc.vector.tensor_mul(dtmp, dmask_sb, scores_sb)
            dscat = ts.tile([128, 2], f32)
            nc.vector.tensor_reduce(dscat, dtmp, axis=AX.X, op=ALU.add)

            # row-max over sentences (diag excluded); scoresT_sb is [img, sent]
            scT_flat = scoresT_sb.rearrange("p a b -> p (a b)")
            dmaskT_flat = dmaskT_sb.rearrange("p a b -> p (a b)")
            maskedT = ts.tile([BI, B], f32)
            nc.vector.scalar_tensor_tensor(
                maskedT, dmaskT_flat, -BIG, scT_flat, op0=ALU.mult, op1=ALU.add
            )
            rowmax = ts.tile([BI, 1], f32)
            nc.vector.tensor_reduce(rowmax, maskedT, axis=AX.X, op=ALU.max)
            dT_tmp = ts.tile([BI, B], f32)
            nc.vector.tensor_mul(dT_tmp, dmaskT_flat, scT_flat)
            d_row = ts.tile([BI, 1], f32)
            nc.vector.tensor_reduce(d_row, dT_tmp, axis=AX.X, op=ALU.add)

            cost_s = ts.tile([BI, 1], f32)
            nc.vector.tensor_sub(cost_s, rowmax, d_row)
            nc.vector.tensor_scalar(
                cost_s, cost_s, MARGIN, 0.0, op0=ALU.add, op1=ALU.max
            )
            cs_ps = tp.tile([1, 1], f32)
            nc.tensor.matmul(cs_ps, lhsT=ones32, rhs=cost_s, start=True, stop=True)
            cs_sb = ts.tile([1, 8], f32)
            nc.gpsimd.memset(cs_sb, 0.0)
            nc.vector.tensor_copy(cs_sb[:, 0:1], cs_ps)

            # pack allgather block: [0:256) colmax | [256:512) dscat | 512 cost_s
            blk = dram.tile([BLK], f32)
            nc.sync.dma_start(
                blk[0:256].rearrange("(t p) -> p t", p=128), colmax_p
            )
            nc.sync.dma_start(
                blk[256:512].rearrange("(t p) -> p t", p=128), dscat
            )
            nc.sync.dma_start(blk[512:520], cs_sb[0, :])
            gath = dram.tile([NCORES, BLK], f32, addr_space="Shared")
            nc.gpsimd.collective_compute(
                "AllGather",
                ALU.bypass,
                ins=[blk.opt()],
                outs=[gath.opt()],
                replica_groups=[list(range(NCORES))],
            )

            # redundant final reduction on every core
            g_cm = ts.tile([128, 2, NCORES], f32)
            g_d = ts.tile([128, 2, NCORES], f32)
            for t in range(2):
                nc.sync.dma_start(
                    g_cm[:, t, :],
                    gath[:, 128 * t:128 * (t + 1)].rearrange("c p -> p c"),
                )
                nc.sync.dma_start(
                    g_d[:, t, :],
                    gath[:, 256 + 128 * t:256 + 128 * (t + 1)].rearrange("c p -> p c"),
                )
            g_cs = ts.tile([1, NCORES], f32)
            nc.sync.dma_start(g_cs, gath[:, 512:513].rearrange("a b -> b a"))

            colmax_g = ts.tile([128, 2], f32)
            nc.vector.tensor_reduce(colmax_g, g_cm, axis=AX.X, op=ALU.max)
            d_all = ts.tile([128, 2], f32)
            nc.vector.tensor_reduce(d_all, g_d, axis=AX.X, op=ALU.add)
            cim = ts.tile([128, 2], f32)
            nc.vector.tensor_sub(cim, colmax_g, d_all)
            nc.vector.tensor_scalar(cim, cim, MARGIN, 0.0, op0=ALU.add, op1=ALU.max)
            cim_r = ts.tile([128, 1], f32)
            nc.vector.tensor_reduce(cim_r, cim, axis=AX.X, op=ALU.add)
            tot_ps = tp.tile([1, 1], f32)
            nc.tensor.matmul(tot_ps, lhsT=ones128, rhs=cim_r, start=True, stop=True)
            cs_tot = ts.tile([1, 1], f32)
            nc.vector.tensor_reduce(cs_tot, g_cs, axis=AX.X, op=ALU.add)
            total = ts.tile([1, 1], f32)
            nc.vector.tensor_add(total, tot_ps, cs_tot)
            nc.sync.dma_start(out_ext[0:1], total[0, :])

    fix_multiwaits(nc, mybir)
    return nc


_CACHE = {}


def _get_nc():
    if "nc" not in _CACHE:
        _CACHE["nc"] = build_graph()
    return _CACHE["nc"]


def make_in_maps(im_set, s_seq, im_len, s_len):
    im_set = np.ascontiguousarray(im_set, dtype=np.float32)
    s_seq = np.ascontiguousarray(s_seq, dtype=np.float32)
    im_len = np.ascontiguousarray(im_len, dtype=np.int32)
    s_len = np.ascontiguousarray(s_len, dtype=np.int32)
    in_maps = []
    for c in range(NCORES):
        dm = np.zeros((B, BI), dtype=np.float32)
        for i in range(BI):
            dm[BI * c + i, i] = 1.0
        in_maps.append({
            "im_set": im_set[BI * c:BI * (c + 1)],
            "s_seq": s_seq,
            "im_len": im_len[BI * c:BI * (c + 1)],
            "s_len": s_len,
            "diag_mask": dm,
            "diag_maskT": np.ascontiguousarray(dm.T),
        })
    return in_maps


def kernel(im_set, s_seq, im_len, s_len):
    import time
    from concourse.bass_utils import run_bass_kernel_spmd

    nc = _get_nc()
    in_maps = make_in_maps(im_set, s_seq, im_len, s_len)
    last = None
    for attempt in range(3):
        try:
            res = run_bass_kernel_spmd(nc, in_maps, core_ids=list(range(NCORES)))
            return np.asarray(
                res.results[0]["out"], dtype=np.float32
            ).reshape(())[()]
        except Exception as e:  # transient device-unrecoverable happens
            last = e
            time.sleep(30 * (attempt + 1))
    raise last


# revision 24
# speedup vs baseline: 1.4302x; 1.0003x over previous
"""Trainium2 Bass kernel for AlignmentContrastiveLoss (8 NeuronCores, SPMD).

Reference semantics:
  im = im_set[:, 1:, :]           [256, 36, 1024]
  s  = s_seq[:, 1:-2, :]          [256, 32, 1024]
  align[i,j,n,m] = im[i,n] . s[j,m], zeroed where n >= im_len[i]-1 or m >= s_len[j]-3
  scores[i,j] = sum_m max_n align[i,j,n,m]
  loss = sum_i relu(M + max_{j!=i} scores[i,j] - scores[i,i])
       + sum_j relu(M + max_{i!=j} scores[i,j] - scores[j,j])

Sharding: data-parallel over images (32 per core), s replicated.

v2 design:
  - f32 tokens are cast once to fp8e4 (im-mask fused as activation scale),
    bitcast to 16-bit fp8-pairs and transposed via the DMA XBAR (pure bit
    movement) into a packed layout: partition p of q-chunk q holds the d
    pair (256q+2p, 256q+2p+1) interleaved per token (HW-verified mapping).
  - s feeds the PE as RAW packed bytes via MatmulPerfMode.DoubleRowSwInterleave
    (stationary side accepts interleaved pairs; output partitions come out
    token-REVERSED, verified on HW). im (small) is deinterleaved to planar
    [128, q, 2, tok] fp8 by one gpsimd 4D copy per tile, which also compacts
    away the XBAR pad columns. No PE transposes, no PSUM->SBUF staging copies.
  - The s-token reversal is compensated in the word-sum weights (w4 built
    from a reversed word mask), so scoresT and the loss tail are unchanged.
  - wt-outer loop: per s-tile, 3 region-chunk matmul groups + vector MAX
    reduce; word-sum is a tiny PE matmul against s-mask-weighted block-ones
    (applies the s word mask for free and directly yields scoresT[img,sent]).
  - im staged as 11 tiles of 3 images (112-partition windows, 4-col overlap
    into the next tile's range which is later overwritten with real data).
  - Cross-core traffic: one 520-float AllGather of per-core column-max
    partials + scattered diagonals + local cost_s sum.
"""

import numpy as np

MARGIN = 0.2
B = 256          # global batch (images == sentences)
NCORES = 8
BI = B // NCORES  # images per core = 32
NREG = 36        # regions per image after stripping
NWORD = 32       # words per sentence after stripping
D = 1024
IM_TOK = BI * NREG      # 1152 dense im tokens
S_TOK = B * NWORD       # 8192 s tokens
S_TILES = S_TOK // 128  # 64
BIG = 1.0e30
# region chunks for the main matmul: (token offset, ntok, nimg)
RCHUNKS = [(0, 432, 12), (432, 432, 12), (864, 288, 8)]
# im staging tiles: (dense token offset, window (mult of 16), first image, n images)
IM_STAGE = [(108 * t, 112, 3 * t, 3) for t in range(10)] + [(1080, 80, 30, 2)]
IM_TP_COLS = 112 * 11  # padded XBAR destination: disjoint 112-col windows
BLK = 520  # allgather block floats: 256 colmax | 256 diag-scatter | 1 cost_s | pad


def fix_multiwaits(nc, mybir):
    """This toolchain's walrus accepts 1 wait per instruction (2 for
    EventSemaphore); Tile can emit more. Offload surplus waits onto
    inserted same-engine NoOps placed immediately before the instruction."""
    n_fix = 0
    for fn in nc.m.functions:
        for blk in fn.blocks:
            insts = blk.instructions
            i = 0
            while i < len(insts):
                inst = insts[i]
                si = inst.sync_info
                waits = list(si.on_wait) if si is not None and si.on_wait else []
                cap = 2 if isinstance(inst, mybir.InstEventSemaphore) else 1
                if len(waits) > cap:
                    surplus, keep = waits[:-cap], waits[-cap:]
                    si.on_wait = keep
                    for w in surplus:
                        nop = mybir.InstNoOp(
                            name=f"{inst.name}_wsplit{n_fix}",
                            engine=inst.engine,
                            ins=[],
                            outs=[],
                            sync_info=mybir.SyncInfo(on_wait=[w], on_update=[]),
                        )
                        insts.insert(i, nop)
                        n_fix += 1
                        i += 1
                i += 1
    return n_fix


DEFAULT_OPTS = {
    "sf_bufs": 16,     # f32 staging tiles
    "pk_bufs": 5,      # packed fp8-as-bf16 staging quad tiles
    "alp_bufs": 6,     # PSUM align buffers
    "gpsimd_cast": 0,  # every Nth s cast on vector (0 = all scalar)
    "s_prefetch": 3,   # s quad-loads dispatched ahead of their pack stage
    "im_head": 4,      # im tiles staged before the first s tile
}


def build_graph(opts=None):
    import concourse.bass as bass
    import concourse.mybir as mybir
    import concourse.tile as tile
    from concourse.masks import make_identity
    from contextlib import ExitStack

    opts = {**DEFAULT_OPTS, **(opts or {})}

    f32 = mybir.dt.float32
    bf16 = mybir.dt.bfloat16
    fp8 = mybir.dt.float8e4
    i32 = mybir.dt.int32
    ALU = mybir.AluOpType
    AX = mybir.AxisListType
    ACTF = mybir.ActivationFunctionType
    SWI = mybir.MatmulPerfMode.DoubleRowSwInterleave

    nc = bass.Bass()

    im_ext = nc.declare_dram_parameter("im_set", [BI, 37, D], f32, isOutput=False)
    s_ext = nc.declare_dram_parameter("s_seq", [B, 35, D], f32, isOutput=False)
    imlen_ext = nc.declare_dram_parameter("im_len", [BI], i32, isOutput=False)
    slen_ext = nc.declare_dram_parameter("s_len", [B], i32, isOutput=False)
    dmask_ext = nc.declare_dram_parameter("diag_mask", [B, BI], f32, isOutput=False)
    dmaskT_ext = nc.declare_dram_parameter("diag_maskT", [BI, B], f32, isOutput=False)
    out_ext = nc.declare_dram_parameter("out", [1], f32, isOutput=True)
    if opts.get("debug"):
        dbg_sel = nc.declare_dram_parameter("dbg_sel", [128, 128], f32, isOutput=True)
        dbg_pm = nc.declare_dram_parameter("dbg_pm", [128, 128], f32, isOutput=True)
        dbg_rb = nc.declare_dram_parameter("dbg_rb", [128, 128], f32, isOutput=True)
        dbg_tc = nc.declare_dram_parameter("dbg_tc", [128, 2], f32, isOutput=True)
        dbg_bm = nc.declare_dram_parameter("dbg_bm", [2, 128, 128], f32, isOutput=True)
        dbg_w4 = nc.declare_dram_parameter("dbg_w4", [128, S_TILES, 4], f32, isOutput=True)
        dbg_scT = nc.declare_dram_parameter("dbg_scT", [BI, S_TILES, 4], f32, isOutput=True)
        dbg_mx = nc.declare_dram_parameter("dbg_mx", [128, S_TILES, BI], f32, isOutput=True)

    with tile.TileContext(nc) as tc, ExitStack() as top:
        # ---------------- constants ----------------
        const = top.enter_context(tc.tile_pool(name="const", bufs=1))
        ident_f32 = const.tile([128, 128], f32)
        make_identity(nc, ident_f32)
        ones32 = const.tile([32, 1], f32)
        nc.gpsimd.memset(ones32, 1.0)
        ones128 = const.tile([128, 1], f32)
        nc.gpsimd.memset(ones128, 1.0)

        # ---------------- token masks (device-side) ----------------
        mpool = top.enter_context(tc.tile_pool(name="masks", bufs=1))
        dram = top.enter_context(tc.tile_pool(name="dram", bufs=1, space="DRAM"))

        # per-image region mask [BI, NREG]: n < im_len-1
        imlen_sb = mpool.tile([BI, 1], i32)
        nc.sync.dma_start(imlen_sb, imlen_ext.rearrange("(p o) -> p o", o=1))
        il_sb = mpool.tile([BI, 1], f32)
        nc.gpsimd.tensor_scalar(il_sb, imlen_sb, 1, None, op0=ALU.subtract)
        iota_r = mpool.tile([BI, NREG], f32)
        nc.gpsimd.iota(
            iota_r, pattern=[[1, NREG]], base=0, channel_multiplier=0,
            allow_small_or_imprecise_dtypes=True,
        )
        mask_im = mpool.tile([BI, NREG], f32)
        nc.gpsimd.tensor_scalar(mask_im, iota_r, il_sb, None, op0=ALU.is_lt)
        # maskcol_im [128, 11]: per (partition, im stage tile) in REGION-major
        # stage order (p = ni*n + i); pad rows -> 0
        mask_im_dram = dram.tile([BI, NREG], f32)
        nc.sync.dma_start(mask_im_dram[:, :], mask_im)
        maskcol_im = mpool.tile([128, len(IM_STAGE)], f32)
        nc.gpsimd.memset(maskcol_im, 0.0)
        for t, (toff, win, i0, ni) in enumerate(IM_STAGE):
            nc.sync.dma_start(
                maskcol_im[0:36 * ni, t:t + 1],
                mask_im_dram[i0:i0 + ni, :].rearrange("i n -> n i"),
            )

        # s word masks -> w4 block-ones weights [128, 64, 4] bf16:
        #   w4[32*jj + m, wt, jj] = (m < s_len[4*wt+jj] - 3)
        slen_sb = mpool.tile([128, 2], i32)
        nc.sync.dma_start(slen_sb, slen_ext.rearrange("(t p) -> p t", p=128))
        sl_sb = mpool.tile([128, 2], f32)
        nc.gpsimd.tensor_scalar(sl_sb, slen_sb, 3, None, op0=ALU.subtract)
        iota_w = mpool.tile([128, NWORD], f32)
        nc.gpsimd.iota(
            iota_w, pattern=[[1, NWORD]], base=0, channel_multiplier=0,
            allow_small_or_imprecise_dtypes=True,
        )
        # Word-sum weights for WORD-major stage order (p = 4w + j) combined
        # with the SwInterleave token reversal (partition p <-> raw col 127-p):
        #   w4[p, wt, jj] = [ (127-p)%4 == jj ] * ( (127-p)//4 < sl[4*wt+jj] )
        # Built transposed (partition c = sentence-within-half, free p) then
        # PE-transposed into place.
        rb = mpool.tile([128, 128], f32)     # rb[c, p] = (127-p)//4
        nc.gpsimd.iota(rb, pattern=[[-1, 32], [0, 4]], base=31,
                       channel_multiplier=0, allow_small_or_imprecise_dtypes=True)
        # sel[c, p] = (p%4 == 3 - c%4)  <=>  ((c + p + 1) & 3 == 0)
        cp_i = mpool.tile([128, 128], i32)
        nc.gpsimd.iota(cp_i, pattern=[[1, 128]], base=1, channel_multiplier=1)
        cp_a = mpool.tile([128, 128], i32)
        nc.vector.tensor_scalar(cp_a, cp_i, 3, None, op0=ALU.bitwise_and)
        sel = mpool.tile([128, 128], f32)
        nc.vector.tensor_scalar(sel, cp_a, 0, None, op0=ALU.is_equal)
        w4 = mpool.tile([128, S_TILES, 4], bf16)
        with tc.tile_pool(name="w4ps", bufs=2, space="PSUM") as wps:
            for h in range(2):
                bh = mpool.tile([128, 128], f32, tag=f"w4bh{h}")
                nc.vector.tensor_scalar(
                    bh, rb, sl_sb[:, h:h + 1], None, op0=ALU.is_lt
                )
                bm = mpool.tile([128, 128], f32, tag=f"w4bm{h}")
                nc.vector.tensor_mul(bm, bh, sel)
                wt_ps = wps.tile([128, 128], f32, tag=f"w4t{h}")
                nc.tensor.transpose(wt_ps, bm, ident_f32)
                nc.vector.tensor_copy(
                    w4[:, 32 * h:32 * (h + 1), :].rearrange("p a b -> p (a b)"),
                    wt_ps,
                )
                if opts.get("debug"):
                    nc.sync.dma_start(dbg_bm[h, :, :], bm)
        if opts.get("debug"):
            nc.sync.dma_start(dbg_sel[:, :], sel)
            nc.sync.dma_start(dbg_rb[:, :], rb)

        # diag masks (sharding metadata inputs)
        dmask_sb = mpool.tile([128, 2, BI], f32)
        nc.sync.dma_start(dmask_sb, dmask_ext.rearrange("(t p) i -> p t i", p=128))
        dmaskT_sb = mpool.tile([BI, 2, 128], f32)
        nc.sync.dma_start(dmaskT_sb, dmaskT_ext.rearrange("p (t f) -> p t f", f=128))

        # ---------------- persistent big buffers ----------------
        big = top.enter_context(tc.tile_pool(name="big", bufs=1))
        # packed-transposed fp8 pairs, stored as bf16 bit containers.
        # One tile per staging unit so the (whole-tile) dependency tracking
        # of the XBAR DMA writes stays exactly per-tile.
        imTp_t = [big.tile([128, 4, 112], bf16, name=f"imTp{t}") for t in range(len(IM_STAGE))]
        imP = big.tile([128, 4, 2, IM_TOK], fp8)      # dense planar im
        sTp_g = [
            big.tile([128, 4, 4, 128], bf16, name=f"sTpg{g}")
            for g in range(S_TILES // 4)
        ]
        maxima = big.tile([128, S_TILES, BI], bf16)  # per (word, wtile, img) region-max
        scoresT_sb = big.tile([BI, S_TILES, 4], f32)  # [img, wt, jj] == [img, sent]
        scores_sb = big.tile([128, 2, BI], f32)       # [sent%128, sent//128, img]

        # fp8 views: im pair-split for the deinterleave, s raw for SwInterleave
        imTp8_t = [
            t.bitcast(fp8).rearrange("p q (t b) -> p q b t", b=2) for t in imTp_t
        ]
        sTraw_g = [t.bitcast(fp8) for t in sTp_g]     # each [128, 4, 4, 256]

        with ExitStack() as mid:
            stage = mid.enter_context(
                tc.tile_pool(name="stage", bufs=opts["sf_bufs"])
            )
            pk = mid.enter_context(tc.tile_pool(name="pk", bufs=opts["pk_bufs"]))
            alp = mid.enter_context(
                tc.tile_pool(name="alp", bufs=opts["alp_bufs"], space="PSUM")
            )
            scp = mid.enter_context(tc.tile_pool(name="scp", bufs=1, space="PSUM"))
            scoresT_ps = scp.tile([BI, S_TILES, 4], f32)

            def stage_im(t):
                toff, win, i0, ni = IM_STAGE[t]
                nreal = 36 * ni
                tf32 = stage.tile([128, D], f32, tag="sf32")
                nc.sync.dma_start(
                    tf32[0:nreal, :],
                    im_ext[i0:i0 + ni, 1:1 + NREG, :].rearrange("i n d -> n i d"),
                )
                tbf = pk.tile([128, D // 2], bf16, tag="spk")
                nc.scalar.activation(
                    tbf.bitcast(fp8)[0:win, :], tf32[0:win, :],
                    ACTF.Copy, scale=maskcol_im[0:win, t:t + 1],
                )
                # XBAR into the tile's private buffer
                nc.scalar.dma_start(
                    imTp_t[t][:, :, 0:win], tbf[0:win, :], transpose=True
                )
                # deinterleave + compact + un-permute (region-major stage
                # order ni*n + i -> dense 36*i + n) in one strided copy
                nc.gpsimd.tensor_copy(
                    imP[:, :, :, toff:toff + nreal].rearrange(
                        "p q b (i n) -> p q b i n", n=NREG
                    ),
                    imTp8_t[t][:, :, :, 0:nreal].rearrange(
                        "p q b (n i) -> p q b n i", i=ni
                    ).rearrange("p q b n i -> p q b i n"),
                )

            def s_load(gq):
                # four per-tile loads (the DMA AP balancer caps at 3 dims,
                # so a quad can't be one DMA), word-major per tile
                tfs = []
                for a in range(4):
                    i = 4 * gq + a
                    tf32 = stage.tile([128, D], f32, tag="sf32")
                    nc.sync.dma_start(
                        tf32,
                        s_ext[4 * i:4 * i + 4, 1:1 + NWORD, :].rearrange(
                            "j w d -> w j d"
                        ),
                    )
                    tfs.append(tf32)
                return tfs

            def s_pack(gq, tfs):
                pkq = pk.tile([128, 4, D // 2], bf16, tag="spk")
                gsp = opts["gpsimd_cast"]
                for a in range(4):
                    i = 4 * gq + a
                    if gsp and (i % gsp == gsp - 1):
                        nc.vector.tensor_copy(
                            pkq[:, a, :].bitcast(fp8), tfs[a]
                        )
                    else:
                        nc.scalar.activation(
                            pkq[:, a, :].bitcast(fp8), tfs[a], ACTF.Copy
                        )
                eng = nc.sync if (gq % 2 == 0) else nc.scalar
                eng.dma_start(
                    sTp_g[gq].rearrange("p a q t -> p (a q) t"),
                    pkq.rearrange("p a c -> p (a c)"),
                    transpose=True,
                )

            # im head first (needed by rc0); s quad-loads run ahead of their
            # cast+transpose stages so no dispatch queue ever blocks
            PRE = opts["s_prefetch"]
            for t in range(len(IM_STAGE)):
                stage_im(t)
            pending = []
            for gq in range(S_TILES // 4):
                pending.append((gq, s_load(gq)))
                if len(pending) > PRE:
                    s_pack(*pending.pop(0))
            for it in pending:
                s_pack(*it)

            # ---------------- main matmul + region-max + word-sum ----------------
            def word_sum(wt):
                # scoresT[img, 4wt+jj] = sum_m maxima[(jj,m), wt, img] * wmask
                nc.tensor.matmul(
                    scoresT_ps[:, wt, :],
                    lhsT=maxima[:, wt, :],
                    rhs=w4[:, wt, :],
                    start=True, stop=True,
                )

            for wt in range(S_TILES):
                for rci, (toff, ntok, nimg) in enumerate(RCHUNKS):
                    pal = alp.tile([128, nimg, NREG], f32, tag="align")
                    for q in range(4):
                        nc.tensor.matmul(
                            pal.rearrange("p a b -> p (a b)"),
                            lhsT=sTraw_g[wt // 4][:, wt % 4, q, :],
                            rhs=imP[:, q, :, toff:toff + ntok],
                            start=(q == 0),
                            stop=(q == 3),
                            perf_mode=SWI,
                        )
                    nc.vector.tensor_reduce(
                        maxima[:, wt, toff // NREG:toff // NREG + nimg],
                        pal, axis=AX.X, op=ALU.max,
                    )
                    # emit the previous tile's word-sum between rc chunks so the
                    # PE never waits on the vector MAX of the current tile
                    if rci == 0 and wt > 0:
                        word_sum(wt - 1)
            word_sum(S_TILES - 1)

            # scoresT -> SBUF, then transpose back to [sent, img]
            nc.vector.tensor_copy(scoresT_sb, scoresT_ps)
            if opts.get("debug"):
                w4d = mpool.tile([128, S_TILES, 4], f32)
                nc.vector.tensor_copy(w4d, w4)
                nc.sync.dma_start(dbg_w4[:, :, :], w4d)
                nc.sync.dma_start(dbg_scT[:, :, :], scoresT_sb)
                mxd = mpool.tile([128, S_TILES, BI], f32)
                nc.vector.tensor_copy(mxd, maxima)
                nc.sync.dma_start(dbg_mx[:, :, :], mxd)
            sc_ps = scp.tile([128, 2, BI], f32)
            for t in range(2):
                nc.tensor.transpose(
                    sc_ps[:, t, :],
                    scoresT_sb[:, 32 * t:32 * (t + 1), :].rearrange(
                        "p a b -> p (a b)"
                    ),
                    ident_f32[:BI, :BI],
                )
                nc.vector.tensor_copy(scores_sb[:, t, :], sc_ps[:, t, :])

        # ---------------- loss tail ----------------
        with ExitStack() as tail:
            tp = tail.enter_context(tc.tile_pool(name="tailp", bufs=1, space="PSUM"))
            ts = tail.enter_context(tc.tile_pool(name="tails", bufs=1))

            # col-max over local images (diag excluded) + scattered diag
            masked = ts.tile([128, 2, BI], f32)
            nc.vector.scalar_tensor_tensor(
                masked, dmask_sb, -BIG, scores_sb, op0=ALU.mult, op1=ALU.add
            )
            colmax_p = ts.tile([128, 2], f32)
            nc.vector.tensor_reduce(colmax_p, masked, axis=AX.X, op=ALU.max)
            dtmp = ts.tile([128, 2, BI], f32)
            nc.vector.tensor_mul(dtmp, dmask_sb, scores_sb)
            dscat = ts.tile([128, 2], f32)
            nc.vector.tensor_reduce(dscat, dtmp, axis=AX.X, op=ALU.add)

            # row-max over sentences (diag excluded); scoresT_sb is [img, sent]
            scT_flat = scoresT_sb.rearrange("p a b -> p (a b)")
            dmaskT_flat = dmaskT_sb.rearrange("p a b -> p (a b)")
            maskedT = ts.tile([BI, B], f32)
            nc.vector.scalar_tensor_tensor(
                maskedT, dmaskT_flat, -BIG, scT_flat, op0=ALU.mult, op1=ALU.add
            )
            rowmax = ts.tile([BI, 1], f32)
            nc.vector.tensor_reduce(rowmax, maskedT, axis=AX.X, op=ALU.max)
            dT_tmp = ts.tile([BI, B], f32)
            nc.vector.tensor_mul(dT_tmp, dmaskT_flat, scT_flat)
            d_row = ts.tile([BI, 1], f32)
            nc.vector.tensor_reduce(d_row, dT_tmp, axis=AX.X, op=ALU.add)

            cost_s = ts.tile([BI, 1], f32)
            nc.vector.tensor_sub(cost_s, rowmax, d_row)
            nc.vector.tensor_scalar(
                cost_s, cost_s, MARGIN, 0.0, op0=ALU.add, op1=ALU.max
            )
            cs_ps = tp.tile([1, 1], f32)
            nc.tensor.matmul(cs_ps, lhsT=ones32, rhs=cost_s, start=True, stop=True)
            cs_sb = ts.tile([1, 8], f32)
            nc.gpsimd.memset(cs_sb, 0.0)
            nc.vector.tensor_copy(cs_sb[:, 0:1], cs_ps)

            # pack allgather block: [0:256) colmax | [256:512) dscat | 512 cost_s
            blk = dram.tile([BLK], f32)
            nc.sync.dma_start(
                blk[0:256].rearrange("(t p) -> p t", p=128), colmax_p
            )
            nc.sync.dma_start(
                blk[256:512].rearrange("(t p) -> p t", p=128), dscat
            )
            nc.sync.dma_start(blk[512:520], cs_sb[0, :])
            gath = dram.tile([NCORES, BLK], f32, addr_space="Shared")
            nc.gpsimd.collective_compute(
                "AllGather",
                ALU.bypass,
                ins=[blk.opt()],
                outs=[gath.opt()],
                replica_groups=[list(range(NCORES))],
            )

            # redundant final reduction on every core
            g_cm = ts.tile([128, 2, NCORES], f32)
            g_d = ts.tile([128, 2, NCORES], f32)
            for t in range(2):
                nc.sync.dma_start(
                    g_cm[:, t, :],
                    gath[:, 128 * t:128 * (t + 1)].rearrange("c p -> p c"),
                )
                nc.sync.dma_start(
                    g_d[:, t, :],
                    gath[:, 256 + 128 * t:256 + 128 * (t + 1)].rearrange("c p -> p c"),
                )
            g_cs = ts.tile([1, NCORES], f32)
            nc.sync.dma_start(g_cs, gath[:, 512:513].rearrange("a b -> b a"))

            colmax_g = ts.tile([128, 2], f32)
            nc.vector.tensor_reduce(colmax_g, g_cm, axis=AX.X, op=ALU.max)
            d_all = ts.tile([128, 2], f32)
            nc.vector.tensor_reduce(d_all, g_d, axis=AX.X, op=ALU.add)
            cim = ts.tile([128, 2], f32)
            nc.vector.tensor_sub(cim, colmax_g, d_all)
            nc.vector.tensor_scalar(cim, cim, MARGIN, 0.0, op0=ALU.add, op1=ALU.max)
            cim_r = ts.tile([128, 1], f32)
            nc.vector.tensor_reduce(cim_r, cim, axis=AX.X, op=ALU.add)
            tot_ps = tp.tile([1, 1], f32)
            nc.tensor.matmul(tot_ps, lhsT=ones128, rhs=cim_r, start=True, stop=True)
            cs_tot = ts.tile([1, 1], f32)
            nc.vector.tensor_reduce(cs_tot, g_cs, axis=AX.X, op=ALU.add)
            total = ts.tile([1, 1], f32)
            nc.vector.tensor_add(total, tot_ps, cs_tot)
            nc.sync.dma_start(out_ext[0:1], total[0, :])

    fix_multiwaits(nc, mybir)
    return nc


_CACHE = {}


def _get_nc():
    if "nc" not in _CACHE:
        _CACHE["nc"] = build_graph()
    return _CACHE["nc"]


def make_in_maps(im_set, s_seq, im_len, s_len):
    im_set = np.ascontiguousarray(im_set, dtype=np.float32)
    s_seq = np.ascontiguousarray(s_seq, dtype=np.float32)
    im_len = np.ascontiguousarray(im_len, dtype=np.int32)
    s_len = np.ascontiguousarray(s_len, dtype=np.int32)
    in_maps = []
    for c in range(NCORES):
        dm = np.zeros((B, BI), dtype=np.float32)
        for i in range(BI):
            dm[BI * c + i, i] = 1.0
        in_maps.append({
            "im_set": im_set[BI * c:BI * (c + 1)],
            "s_seq": s_seq,
            "im_len": im_len[BI * c:BI * (c + 1)],
            "s_len": s_len,
            "diag_mask": dm,
            "diag_maskT": np.ascontiguousarray(dm.T),
        })
    return in_maps


def kernel(im_set, s_seq, im_len, s_len):
    import time
    from concourse.bass_utils import run_bass_kernel_spmd

    nc = _get_nc()
    in_maps = make_in_maps(im_set, s_seq, im_len, s_len)
    last = None
    for attempt in range(3):
        try:
            res = run_bass_kernel_spmd(nc, in_maps, core_ids=list(range(NCORES)))
            return np.asarray(
                res.results[0]["out"], dtype=np.float32
            ).reshape(())[()]
        except Exception as e:  # transient device-unrecoverable happens
            last = e
            time.sleep(30 * (attempt + 1))
    raise last


# revision 25
# speedup vs baseline: 1.4430x; 1.0090x over previous
"""Trainium2 Bass kernel for AlignmentContrastiveLoss (8 NeuronCores, SPMD).

Reference semantics:
  im = im_set[:, 1:, :]           [256, 36, 1024]
  s  = s_seq[:, 1:-2, :]          [256, 32, 1024]
  align[i,j,n,m] = im[i,n] . s[j,m], zeroed where n >= im_len[i]-1 or m >= s_len[j]-3
  scores[i,j] = sum_m max_n align[i,j,n,m]
  loss = sum_i relu(M + max_{j!=i} scores[i,j] - scores[i,i])
       + sum_j relu(M + max_{i!=j} scores[i,j] - scores[j,j])

Sharding: data-parallel over images (32 per core), s replicated.

v2 design:
  - f32 tokens are cast once to fp8e4 (im-mask fused as activation scale),
    bitcast to 16-bit fp8-pairs and transposed via the DMA XBAR (pure bit
    movement) into a packed layout: partition p of q-chunk q holds the d
    pair (256q+2p, 256q+2p+1) interleaved per token (HW-verified mapping).
  - s feeds the PE as RAW packed bytes via MatmulPerfMode.DoubleRowSwInterleave
    (stationary side accepts interleaved pairs; output partitions come out
    token-REVERSED, verified on HW). im (small) is deinterleaved to planar
    [128, q, 2, tok] fp8 by one gpsimd 4D copy per tile, which also compacts
    away the XBAR pad columns. No PE transposes, no PSUM->SBUF staging copies.
  - The s-token reversal is compensated in the word-sum weights (w4 built
    from a reversed word mask), so scoresT and the loss tail are unchanged.
  - wt-outer loop: per s-tile, 3 region-chunk matmul groups + vector MAX
    reduce; word-sum is a tiny PE matmul against s-mask-weighted block-ones
    (applies the s word mask for free and directly yields scoresT[img,sent]).
  - im staged as 11 tiles of 3 images (112-partition windows, 4-col overlap
    into the next tile's range which is later overwritten with real data).
  - Cross-core traffic: one 520-float AllGather of per-core column-max
    partials + scattered diagonals + local cost_s sum.
"""

import numpy as np

MARGIN = 0.2
B = 256          # global batch (images == sentences)
NCORES = 8
BI = B // NCORES  # images per core = 32
NREG = 36        # regions per image after stripping
NWORD = 32       # words per sentence after stripping
D = 1024
IM_TOK = BI * NREG      # 1152 dense im tokens
S_TOK = B * NWORD       # 8192 s tokens
S_TILES = S_TOK // 128  # 64
BIG = 1.0e30
# region chunks for the main matmul: (token offset, ntok, nimg)
RCHUNKS = [(0, 432, 12), (432, 432, 12), (864, 288, 8)]
# im staging tiles: (dense token offset, window (mult of 16), first image, n images)
IM_STAGE = [(108 * t, 112, 3 * t, 3) for t in range(10)] + [(1080, 80, 30, 2)]
IM_TP_COLS = 112 * 11  # padded XBAR destination: disjoint 112-col windows
BLK = 520  # allgather block floats: 256 colmax | 256 diag-scatter | 1 cost_s | pad


def fix_multiwaits(nc, mybir):
    """This toolchain's walrus accepts 1 wait per instruction (2 for
    EventSemaphore); Tile can emit more. Offload surplus waits onto
    inserted same-engine NoOps placed immediately before the instruction."""
    n_fix = 0
    for fn in nc.m.functions:
        for blk in fn.blocks:
            insts = blk.instructions
            i = 0
            while i < len(insts):
                inst = insts[i]
                si = inst.sync_info
                waits = list(si.on_wait) if si is not None and si.on_wait else []
                cap = 2 if isinstance(inst, mybir.InstEventSemaphore) else 1
                if len(waits) > cap:
                    surplus, keep = waits[:-cap], waits[-cap:]
                    si.on_wait = keep
                    for w in surplus:
                        nop = mybir.InstNoOp(
                            name=f"{inst.name}_wsplit{n_fix}",
                            engine=inst.engine,
                            ins=[],
                            outs=[],
                            sync_info=mybir.SyncInfo(on_wait=[w], on_update=[]),
                        )
                        insts.insert(i, nop)
                        n_fix += 1
                        i += 1
                i += 1
    return n_fix


DEFAULT_OPTS = {
    "sf_bufs": 16,     # f32 staging tiles
    "pk_bufs": 5,      # packed fp8-as-bf16 staging quad tiles
    "alp_bufs": 6,     # PSUM align buffers
    "gpsimd_cast": 0,  # every Nth s cast on vector (0 = all scalar)
    "s_prefetch": 3,   # s quad-loads dispatched ahead of their pack stage
    "im_head": 4,      # im tiles staged before the first s tile
}


def build_graph(opts=None):
    import concourse.bass as bass
    import concourse.mybir as mybir
    import concourse.tile as tile
    from concourse.masks import make_identity
    from contextlib import ExitStack

    opts = {**DEFAULT_OPTS, **(opts or {})}

    f32 = mybir.dt.float32
    bf16 = mybir.dt.bfloat16
    fp8 = mybir.dt.float8e4
    i32 = mybir.dt.int32
    ALU = mybir.AluOpType
    AX = mybir.AxisListType
    ACTF = mybir.ActivationFunctionType
    SWI = mybir.MatmulPerfMode.DoubleRowSwInterleave

    nc = bass.Bass()

    im_ext = nc.declare_dram_parameter("im_set", [BI, 37, D], f32, isOutput=False)
    s_ext = nc.declare_dram_parameter("s_seq", [B, 35, D], f32, isOutput=False)
    imlen_ext = nc.declare_dram_parameter("im_len", [BI], i32, isOutput=False)
    slen_ext = nc.declare_dram_parameter("s_len", [B], i32, isOutput=False)
    dmask_ext = nc.declare_dram_parameter("diag_mask", [B, BI], f32, isOutput=False)
    dmaskT_ext = nc.declare_dram_parameter("diag_maskT", [BI, B], f32, isOutput=False)
    out_ext = nc.declare_dram_parameter("out", [1], f32, isOutput=True)
    if opts.get("debug"):
        dbg_sel = nc.declare_dram_parameter("dbg_sel", [128, 128], f32, isOutput=True)
        dbg_pm = nc.declare_dram_parameter("dbg_pm", [128, 128], f32, isOutput=True)
        dbg_rb = nc.declare_dram_parameter("dbg_rb", [128, 128], f32, isOutput=True)
        dbg_tc = nc.declare_dram_parameter("dbg_tc", [128, 2], f32, isOutput=True)
        dbg_bm = nc.declare_dram_parameter("dbg_bm", [2, 128, 128], f32, isOutput=True)
        dbg_w4 = nc.declare_dram_parameter("dbg_w4", [128, S_TILES, 4], f32, isOutput=True)
        dbg_scT = nc.declare_dram_parameter("dbg_scT", [BI, S_TILES, 4], f32, isOutput=True)
        dbg_mx = nc.declare_dram_parameter("dbg_mx", [128, S_TILES, BI], f32, isOutput=True)

    with tile.TileContext(nc) as tc, ExitStack() as top:
        # ---------------- constants ----------------
        const = top.enter_context(tc.tile_pool(name="const", bufs=1))
        ident_f32 = const.tile([128, 128], f32)
        make_identity(nc, ident_f32)
        ones32 = const.tile([32, 1], f32)
        nc.gpsimd.memset(ones32, 1.0)
        ones128 = const.tile([128, 1], f32)
        nc.gpsimd.memset(ones128, 1.0)

        # ---------------- token masks (device-side) ----------------
        mpool = top.enter_context(tc.tile_pool(name="masks", bufs=1))
        dram = top.enter_context(tc.tile_pool(name="dram", bufs=1, space="DRAM"))

        # per-image region mask [BI, NREG]: n < im_len-1
        imlen_sb = mpool.tile([BI, 1], i32)
        nc.sync.dma_start(imlen_sb, imlen_ext.rearrange("(p o) -> p o", o=1))
        il_sb = mpool.tile([BI, 1], f32)
        nc.gpsimd.tensor_scalar(il_sb, imlen_sb, 1, None, op0=ALU.subtract)
        iota_r = mpool.tile([BI, NREG], f32)
        nc.gpsimd.iota(
            iota_r, pattern=[[1, NREG]], base=0, channel_multiplier=0,
            allow_small_or_imprecise_dtypes=True,
        )
        mask_im = mpool.tile([BI, NREG], f32)
        nc.gpsimd.tensor_scalar(mask_im, iota_r, il_sb, None, op0=ALU.is_lt)
        # maskcol_im [128, 11]: per (partition, im stage tile) in REGION-major
        # stage order (p = ni*n + i); pad rows -> 0
        mask_im_dram = dram.tile([BI, NREG], f32)
        nc.sync.dma_start(mask_im_dram[:, :], mask_im)
        maskcol_im = mpool.tile([128, len(IM_STAGE)], f32)
        nc.gpsimd.memset(maskcol_im, 0.0)
        for t, (toff, win, i0, ni) in enumerate(IM_STAGE):
            nc.sync.dma_start(
                maskcol_im[0:36 * ni, t:t + 1],
                mask_im_dram[i0:i0 + ni, :].rearrange("i n -> n i"),
            )

        # s word masks -> w4 block-ones weights [128, 64, 4] bf16:
        #   w4[32*jj + m, wt, jj] = (m < s_len[4*wt+jj] - 3)
        slen_sb = mpool.tile([128, 2], i32)
        nc.sync.dma_start(slen_sb, slen_ext.rearrange("(t p) -> p t", p=128))
        sl_sb = mpool.tile([128, 2], f32)
        nc.gpsimd.tensor_scalar(sl_sb, slen_sb, 3, None, op0=ALU.subtract)
        iota_w = mpool.tile([128, NWORD], f32)
        nc.gpsimd.iota(
            iota_w, pattern=[[1, NWORD]], base=0, channel_multiplier=0,
            allow_small_or_imprecise_dtypes=True,
        )
        # Word-sum weights for WORD-major stage order (p = 4w + j) combined
        # with the SwInterleave token reversal (partition p <-> raw col 127-p):
        #   w4[p, wt, jj] = [ (127-p)%4 == jj ] * ( (127-p)//4 < sl[4*wt+jj] )
        # Built transposed (partition c = sentence-within-half, free p) then
        # PE-transposed into place.
        rb = mpool.tile([128, 128], f32)     # rb[c, p] = (127-p)//4
        nc.gpsimd.iota(rb, pattern=[[-1, 32], [0, 4]], base=31,
                       channel_multiplier=0, allow_small_or_imprecise_dtypes=True)
        # sel[c, p] = (p%4 == 3 - c%4)  <=>  ((c + p + 1) & 3 == 0)
        cp_i = mpool.tile([128, 128], i32)
        nc.gpsimd.iota(cp_i, pattern=[[1, 128]], base=1, channel_multiplier=1)
        cp_a = mpool.tile([128, 128], i32)
        nc.vector.tensor_scalar(cp_a, cp_i, 3, None, op0=ALU.bitwise_and)
        sel = mpool.tile([128, 128], f32)
        nc.vector.tensor_scalar(sel, cp_a, 0, None, op0=ALU.is_equal)
        w4 = mpool.tile([128, S_TILES, 4], bf16)
        with tc.tile_pool(name="w4ps", bufs=2, space="PSUM") as wps:
            for h in range(2):
                bh = mpool.tile([128, 128], f32, tag=f"w4bh{h}")
                nc.vector.tensor_scalar(
                    bh, rb, sl_sb[:, h:h + 1], None, op0=ALU.is_lt
                )
                bm = mpool.tile([128, 128], f32, tag=f"w4bm{h}")
                nc.vector.tensor_mul(bm, bh, sel)
                wt_ps = wps.tile([128, 128], f32, tag=f"w4t{h}")
                nc.tensor.transpose(wt_ps, bm, ident_f32)
                nc.vector.tensor_copy(
                    w4[:, 32 * h:32 * (h + 1), :].rearrange("p a b -> p (a b)"),
                    wt_ps,
                )
                if opts.get("debug"):
                    nc.sync.dma_start(dbg_bm[h, :, :], bm)
        if opts.get("debug"):
            nc.sync.dma_start(dbg_sel[:, :], sel)
            nc.sync.dma_start(dbg_rb[:, :], rb)

        # diag masks (sharding metadata inputs)
        dmask_sb = mpool.tile([128, 2, BI], f32)
        nc.sync.dma_start(dmask_sb, dmask_ext.rearrange("(t p) i -> p t i", p=128))
        dmaskT_sb = mpool.tile([BI, 2, 128], f32)
        nc.sync.dma_start(dmaskT_sb, dmaskT_ext.rearrange("p (t f) -> p t f", f=128))

        # ---------------- persistent big buffers ----------------
        big = top.enter_context(tc.tile_pool(name="big", bufs=1))
        # packed-transposed fp8 pairs, stored as bf16 bit containers.
        # One tile per staging unit so the (whole-tile) dependency tracking
        # of the XBAR DMA writes stays exactly per-tile.
        imTp_t = [big.tile([128, 4, 112], bf16, name=f"imTp{t}") for t in range(len(IM_STAGE))]
        imP = big.tile([128, 4, 2, IM_TOK], fp8)      # dense planar im
        sTp_g = [
            big.tile([128, 4, 4, 128], bf16, name=f"sTpg{g}")
            for g in range(S_TILES // 4)
        ]
        maxima = big.tile([128, S_TILES, BI], bf16)  # per (word, wtile, img) region-max
        scoresT_sb = big.tile([BI, S_TILES, 4], f32)  # [img, wt, jj] == [img, sent]
        scores_sb = big.tile([128, 2, BI], f32)       # [sent%128, sent//128, img]

        # fp8 views: im pair-split for the deinterleave, s raw for SwInterleave
        imTp8_t = [
            t.bitcast(fp8).rearrange("p q (t b) -> p q b t", b=2) for t in imTp_t
        ]
        sTraw_g = [t.bitcast(fp8) for t in sTp_g]     # each [128, 4, 4, 256]

        with ExitStack() as mid:
            stage = mid.enter_context(
                tc.tile_pool(name="stage", bufs=opts["sf_bufs"])
            )
            pk = mid.enter_context(tc.tile_pool(name="pk", bufs=opts["pk_bufs"]))
            alp = mid.enter_context(
                tc.tile_pool(name="alp", bufs=opts["alp_bufs"], space="PSUM")
            )
            scp = mid.enter_context(tc.tile_pool(name="scp", bufs=1, space="PSUM"))
            scoresT_ps = scp.tile([BI, S_TILES, 4], f32)

            def stage_im(t):
                toff, win, i0, ni = IM_STAGE[t]
                nreal = 36 * ni
                tf32 = stage.tile([128, D], f32, tag="sf32")
                nc.sync.dma_start(
                    tf32[0:nreal, :],
                    im_ext[i0:i0 + ni, 1:1 + NREG, :].rearrange("i n d -> n i d"),
                )
                tbf = pk.tile([128, D // 2], bf16, tag="spk")
                # masked cast on the (ramp-idle) vector engine
                nc.vector.tensor_scalar(
                    tbf.bitcast(fp8)[0:win, :], tf32[0:win, :],
                    maskcol_im[0:win, t:t + 1], None, op0=ALU.mult,
                )
                # XBAR into the tile's private buffer
                eng = nc.sync if (t % 2 == 0) else nc.scalar
                eng.dma_start(
                    imTp_t[t][:, :, 0:win], tbf[0:win, :], transpose=True
                )
                # deinterleave + compact + un-permute (region-major stage
                # order ni*n + i -> dense 36*i + n) in one strided copy
                nc.vector.tensor_copy(
                    imP[:, :, :, toff:toff + nreal].rearrange(
                        "p q b (i n) -> p q b i n", n=NREG
                    ),
                    imTp8_t[t][:, :, :, 0:nreal].rearrange(
                        "p q b (n i) -> p q b n i", i=ni
                    ).rearrange("p q b n i -> p q b i n"),
                )

            def s_load(gq):
                # four per-tile loads (the DMA AP balancer caps at 3 dims,
                # so a quad can't be one DMA), word-major per tile
                tfs = []
                for a in range(4):
                    i = 4 * gq + a
                    tf32 = stage.tile([128, D], f32, tag="sf32")
                    nc.sync.dma_start(
                        tf32,
                        s_ext[4 * i:4 * i + 4, 1:1 + NWORD, :].rearrange(
                            "j w d -> w j d"
                        ),
                    )
                    tfs.append(tf32)
                return tfs

            def s_pack(gq, tfs):
                pkq = pk.tile([128, 4, D // 2], bf16, tag="spk")
                gsp = opts["gpsimd_cast"]
                for a in range(4):
                    i = 4 * gq + a
                    if gsp and (i % gsp == gsp - 1):
                        nc.vector.tensor_copy(
                            pkq[:, a, :].bitcast(fp8), tfs[a]
                        )
                    else:
                        nc.scalar.activation(
                            pkq[:, a, :].bitcast(fp8), tfs[a], ACTF.Copy
                        )
                eng = nc.sync if (gq % 2 == 0) else nc.scalar
                eng.dma_start(
                    sTp_g[gq].rearrange("p a q t -> p (a q) t"),
                    pkq.rearrange("p a c -> p (a c)"),
                    transpose=True,
                )

            # im head first (needed by rc0); s quad-loads run ahead of their
            # cast+transpose stages so no dispatch queue ever blocks
            PRE = opts["s_prefetch"]
            for t in range(len(IM_STAGE)):
                stage_im(t)
            pending = []
            for gq in range(S_TILES // 4):
                pending.append((gq, s_load(gq)))
                if len(pending) > PRE:
                    s_pack(*pending.pop(0))
            for it in pending:
                s_pack(*it)

            # ---------------- main matmul + region-max + word-sum ----------------
            def word_sum(wt):
                # scoresT[img, 4wt+jj] = sum_m maxima[(jj,m), wt, img] * wmask
                nc.tensor.matmul(
                    scoresT_ps[:, wt, :],
                    lhsT=maxima[:, wt, :],
                    rhs=w4[:, wt, :],
                    start=True, stop=True,
                )

            for wt in range(S_TILES):
                for rci, (toff, ntok, nimg) in enumerate(RCHUNKS):
                    pal = alp.tile([128, nimg, NREG], f32, tag="align")
                    for q in range(4):
                        nc.tensor.matmul(
                            pal.rearrange("p a b -> p (a b)"),
                            lhsT=sTraw_g[wt // 4][:, wt % 4, q, :],
                            rhs=imP[:, q, :, toff:toff + ntok],
                            start=(q == 0),
                            stop=(q == 3),
                            perf_mode=SWI,
                        )
                    nc.vector.tensor_reduce(
                        maxima[:, wt, toff // NREG:toff // NREG + nimg],
                        pal, axis=AX.X, op=ALU.max,
                    )
                    # emit the previous tile's word-sum between rc chunks so the
                    # PE never waits on the vector MAX of the current tile
                    if rci == 0 and wt > 0:
                        word_sum(wt - 1)
            word_sum(S_TILES - 1)

            # scoresT -> SBUF, then transpose back to [sent, img]
            nc.vector.tensor_copy(scoresT_sb, scoresT_ps)
            if opts.get("debug"):
                w4d = mpool.tile([128, S_TILES, 4], f32)
                nc.vector.tensor_copy(w4d, w4)
                nc.sync.dma_start(dbg_w4[:, :, :], w4d)
                nc.sync.dma_start(dbg_scT[:, :, :], scoresT_sb)
                mxd = mpool.tile([128, S_TILES, BI], f32)
                nc.vector.tensor_copy(mxd, maxima)
                nc.sync.dma_start(dbg_mx[:, :, :], mxd)
            sc_ps = scp.tile([128, 2, BI], f32)
            for t in range(2):
                nc.tensor.transpose(
                    sc_ps[:, t, :],
                    scoresT_sb[:, 32 * t:32 * (t + 1), :].rearrange(
                        "p a b -> p (a b)"
                    ),
                    ident_f32[:BI, :BI],
                )
                nc.vector.tensor_copy(scores_sb[:, t, :], sc_ps[:, t, :])

        # ---------------- loss tail ----------------
        with ExitStack() as tail:
            tp = tail.enter_context(tc.tile_pool(name="tailp", bufs=1, space="PSUM"))
            ts = tail.enter_context(tc.tile_pool(name="tails", bufs=1))

            # col-max over local images (diag excluded) + scattered diag
            masked = ts.tile([128, 2, BI], f32)
            nc.vector.scalar_tensor_tensor(
                masked, dmask_sb, -BIG, scores_sb, op0=ALU.mult, op1=ALU.add
            )
            colmax_p = ts.tile([128, 2], f32)
            nc.vector.tensor_reduce(colmax_p, masked, axis=AX.X, op=ALU.max)
            dtmp = ts.tile([128, 2, BI], f32)
            nc.vector.tensor_mul(dtmp, dmask_sb, scores_sb)
            dscat = ts.tile([128, 2], f32)
            nc.vector.tensor_reduce(dscat, dtmp, axis=AX.X, op=ALU.add)

            # row-max over sentences (diag excluded); scoresT_sb is [img, sent]
            scT_flat = scoresT_sb.rearrange("p a b -> p (a b)")
            dmaskT_flat = dmaskT_sb.rearrange("p a b -> p (a b)")
            maskedT = ts.tile([BI, B], f32)
            nc.vector.scalar_tensor_tensor(
                maskedT, dmaskT_flat, -BIG, scT_flat, op0=ALU.mult, op1=ALU.add
            )
            rowmax = ts.tile([BI, 1], f32)
            nc.vector.tensor_reduce(rowmax, maskedT, axis=AX.X, op=ALU.max)
            dT_tmp = ts.tile([BI, B], f32)
            nc.vector.tensor_mul(dT_tmp, dmaskT_flat, scT_flat)
            d_row = ts.tile([BI, 1], f32)
            nc.vector.tensor_reduce(d_row, dT_tmp, axis=AX.X, op=ALU.add)

            cost_s = ts.tile([BI, 1], f32)
            nc.vector.tensor_sub(cost_s, rowmax, d_row)
            nc.vector.tensor_scalar(
                cost_s, cost_s, MARGIN, 0.0, op0=ALU.add, op1=ALU.max
            )
            cs_ps = tp.tile([1, 1], f32)
            nc.tensor.matmul(cs_ps, lhsT=ones32, rhs=cost_s, start=True, stop=True)
            cs_sb = ts.tile([1, 8], f32)
            nc.gpsimd.memset(cs_sb, 0.0)
            nc.vector.tensor_copy(cs_sb[:, 0:1], cs_ps)

            # pack allgather block: [0:256) colmax | [256:512) dscat | 512 cost_s
            blk = dram.tile([BLK], f32)
            nc.sync.dma_start(
                blk[0:256].rearrange("(t p) -> p t", p=128), colmax_p
            )
            nc.sync.dma_start(
                blk[256:512].rearrange("(t p) -> p t", p=128), dscat
            )
            nc.sync.dma_start(blk[512:520], cs_sb[0, :])
            gath = dram.tile([NCORES, BLK], f32, addr_space="Shared")
            nc.gpsimd.collective_compute(
                "AllGather",
                ALU.bypass,
                ins=[blk.opt()],
                outs=[gath.opt()],
                replica_groups=[list(range(NCORES))],
            )

            # redundant final reduction on every core
            g_cm = ts.tile([128, 2, NCORES], f32)
            g_d = ts.tile([128, 2, NCORES], f32)
            for t in range(2):
                nc.sync.dma_start(
                    g_cm[:, t, :],
                    gath[:, 128 * t:128 * (t + 1)].rearrange("c p -> p c"),
                )
                nc.sync.dma_start(
                    g_d[:, t, :],
                    gath[:, 256 + 128 * t:256 + 128 * (t + 1)].rearrange("c p -> p c"),
                )
            g_cs = ts.tile([1, NCORES], f32)
            nc.sync.dma_start(g_cs, gath[:, 512:513].rearrange("a b -> b a"))

            colmax_g = ts.tile([128, 2], f32)
            nc.vector.tensor_reduce(colmax_g, g_cm, axis=AX.X, op=ALU.max)
            d_all = ts.tile([128, 2], f32)
            nc.vector.tensor_reduce(d_all, g_d, axis=AX.X, op=ALU.add)
            cim = ts.tile([128, 2], f32)
            nc.vector.tensor_sub(cim, colmax_g, d_all)
            nc.vector.tensor_scalar(cim, cim, MARGIN, 0.0, op0=ALU.add, op1=ALU.max)
            cim_r = ts.tile([128, 1], f32)
            nc.vector.tensor_reduce(cim_r, cim, axis=AX.X, op=ALU.add)
            tot_ps = tp.tile([1, 1], f32)
            nc.tensor.matmul(tot_ps, lhsT=ones128, rhs=cim_r, start=True, stop=True)
            cs_tot = ts.tile([1, 1], f32)
            nc.vector.tensor_reduce(cs_tot, g_cs, axis=AX.X, op=ALU.add)
            total = ts.tile([1, 1], f32)
            nc.vector.tensor_add(total, tot_ps, cs_tot)
            nc.sync.dma_start(out_ext[0:1], total[0, :])

    fix_multiwaits(nc, mybir)
    return nc


_CACHE = {}


def _get_nc():
    if "nc" not in _CACHE:
        _CACHE["nc"] = build_graph()
    return _CACHE["nc"]


def make_in_maps(im_set, s_seq, im_len, s_len):
    im_set = np.ascontiguousarray(im_set, dtype=np.float32)
    s_seq = np.ascontiguousarray(s_seq, dtype=np.float32)
    im_len = np.ascontiguousarray(im_len, dtype=np.int32)
    s_len = np.ascontiguousarray(s_len, dtype=np.int32)
    in_maps = []
    for c in range(NCORES):
        dm = np.zeros((B, BI), dtype=np.float32)
        for i in range(BI):
            dm[BI * c + i, i] = 1.0
        in_maps.append({
            "im_set": im_set[BI * c:BI * (c + 1)],
            "s_seq": s_seq,
            "im_len": im_len[BI * c:BI * (c + 1)],
            "s_len": s_len,
            "diag_mask": dm,
            "diag_maskT": np.ascontiguousarray(dm.T),
        })
    return in_maps


def kernel(im_set, s_seq, im_len, s_len):
    import time
    from concourse.bass_utils import run_bass_kernel_spmd

    nc = _get_nc()
    in_maps = make_in_maps(im_set, s_seq, im_len, s_len)
    last = None
    for attempt in range(3):
        try:
            res = run_bass_kernel_spmd(nc, in_maps, core_ids=list(range(NCORES)))
            return np.asarray(
                res.results[0]["out"], dtype=np.float32
            ).reshape(())[()]
        except Exception as e:  # transient device-unrecoverable happens
            last = e
            time.sleep(30 * (attempt + 1))
    raise last


# revision 26
# speedup vs baseline: 1.4866x; 1.0302x over previous
"""Trainium2 Bass kernel for AlignmentContrastiveLoss (8 NeuronCores, SPMD).

Reference semantics:
  im = im_set[:, 1:, :]           [256, 36, 1024]
  s  = s_seq[:, 1:-2, :]          [256, 32, 1024]
  align[i,j,n,m] = im[i,n] . s[j,m], zeroed where n >= im_len[i]-1 or m >= s_len[j]-3
  scores[i,j] = sum_m max_n align[i,j,n,m]
  loss = sum_i relu(M + max_{j!=i} scores[i,j] - scores[i,i])
       + sum_j relu(M + max_{i!=j} scores[i,j] - scores[j,j])

Sharding: data-parallel over images (32 per core), s replicated.

v2 design:
  - f32 tokens are cast once to fp8e4 (im-mask fused as activation scale),
    bitcast to 16-bit fp8-pairs and transposed via the DMA XBAR (pure bit
    movement) into a packed layout: partition p of q-chunk q holds the d
    pair (256q+2p, 256q+2p+1) interleaved per token (HW-verified mapping).
  - s feeds the PE as RAW packed bytes via MatmulPerfMode.DoubleRowSwInterleave
    (stationary side accepts interleaved pairs; output partitions come out
    token-REVERSED, verified on HW). im (small) is deinterleaved to planar
    [128, q, 2, tok] fp8 by one gpsimd 4D copy per tile, which also compacts
    away the XBAR pad columns. No PE transposes, no PSUM->SBUF staging copies.
  - The s-token reversal is compensated in the word-sum weights (w4 built
    from a reversed word mask), so scoresT and the loss tail are unchanged.
  - wt-outer loop: per s-tile, 3 region-chunk matmul groups + vector MAX
    reduce; word-sum is a tiny PE matmul against s-mask-weighted block-ones
    (applies the s word mask for free and directly yields scoresT[img,sent]).
  - im staged as 11 tiles of 3 images (112-partition windows, 4-col overlap
    into the next tile's range which is later overwritten with real data).
  - Cross-core traffic: one 520-float AllGather of per-core column-max
    partials + scattered diagonals + local cost_s sum.
"""

import numpy as np

MARGIN = 0.2
B = 256          # global batch (images == sentences)
NCORES = 8
BI = B // NCORES  # images per core = 32
NREG = 36        # regions per image after stripping
NWORD = 32       # words per sentence after stripping
D = 1024
IM_TOK = BI * NREG      # 1152 dense im tokens
S_TOK = B * NWORD       # 8192 s tokens
S_TILES = S_TOK // 128  # 64
BIG = 1.0e30
# region chunks for the main matmul: (token offset, ntok, nimg)
RCHUNKS = [(0, 432, 12), (432, 432, 12), (864, 288, 8)]
# im staging tiles: (dense token offset, window (mult of 16), first image, n images)
IM_STAGE = [(108 * t, 112, 3 * t, 3) for t in range(10)] + [(1080, 80, 30, 2)]
IM_TP_COLS = 112 * 11  # padded XBAR destination: disjoint 112-col windows
BLK = 520  # allgather block floats: 256 colmax | 256 diag-scatter | 1 cost_s | pad


def fix_multiwaits(nc, mybir):
    """This toolchain's walrus accepts 1 wait per instruction (2 for
    EventSemaphore); Tile can emit more. Offload surplus waits onto
    inserted same-engine NoOps placed immediately before the instruction."""
    n_fix = 0
    for fn in nc.m.functions:
        for blk in fn.blocks:
            insts = blk.instructions
            i = 0
            while i < len(insts):
                inst = insts[i]
                si = inst.sync_info
                waits = list(si.on_wait) if si is not None and si.on_wait else []
                cap = 2 if isinstance(inst, mybir.InstEventSemaphore) else 1
                if len(waits) > cap:
                    surplus, keep = waits[:-cap], waits[-cap:]
                    si.on_wait = keep
                    for w in surplus:
                        nop = mybir.InstNoOp(
                            name=f"{inst.name}_wsplit{n_fix}",
                            engine=inst.engine,
                            ins=[],
                            outs=[],
                            sync_info=mybir.SyncInfo(on_wait=[w], on_update=[]),
                        )
                        insts.insert(i, nop)
                        n_fix += 1
                        i += 1
                i += 1
    return n_fix


DEFAULT_OPTS = {
    "sf_bufs": 16,     # f32 staging tiles
    "pk_bufs": 5,      # packed fp8-as-bf16 staging quad tiles
    "alp_bufs": 6,     # PSUM align buffers
    "gpsimd_cast": 0,  # every Nth s cast on vector (0 = all scalar)
    "s_prefetch": 3,   # s quad-loads dispatched ahead of their pack stage
    "im_head": 4,      # im tiles staged before the first s tile
}


def build_graph(opts=None):
    import concourse.bass as bass
    import concourse.mybir as mybir
    import concourse.tile as tile
    from concourse.masks import make_identity
    from contextlib import ExitStack

    opts = {**DEFAULT_OPTS, **(opts or {})}

    f32 = mybir.dt.float32
    bf16 = mybir.dt.bfloat16
    fp8 = mybir.dt.float8e4
    i32 = mybir.dt.int32
    ALU = mybir.AluOpType
    AX = mybir.AxisListType
    ACTF = mybir.ActivationFunctionType
    SWI = mybir.MatmulPerfMode.DoubleRowSwInterleave

    nc = bass.Bass()

    im_ext = nc.declare_dram_parameter("im_set", [BI, 37, D], f32, isOutput=False)
    s_ext = nc.declare_dram_parameter("s_seq", [B, 35, D], f32, isOutput=False)
    imlen_ext = nc.declare_dram_parameter("im_len", [BI], i32, isOutput=False)
    slen_ext = nc.declare_dram_parameter("s_len", [B], i32, isOutput=False)
    dmask_ext = nc.declare_dram_parameter("diag_mask", [B, BI], f32, isOutput=False)
    dmaskT_ext = nc.declare_dram_parameter("diag_maskT", [BI, B], f32, isOutput=False)
    out_ext = nc.declare_dram_parameter("out", [1], f32, isOutput=True)
    if opts.get("debug"):
        dbg_sel = nc.declare_dram_parameter("dbg_sel", [128, 128], f32, isOutput=True)
        dbg_pm = nc.declare_dram_parameter("dbg_pm", [128, 128], f32, isOutput=True)
        dbg_rb = nc.declare_dram_parameter("dbg_rb", [128, 128], f32, isOutput=True)
        dbg_tc = nc.declare_dram_parameter("dbg_tc", [128, 2], f32, isOutput=True)
        dbg_bm = nc.declare_dram_parameter("dbg_bm", [2, 128, 128], f32, isOutput=True)
        dbg_w4 = nc.declare_dram_parameter("dbg_w4", [128, S_TILES, 4], f32, isOutput=True)
        dbg_scT = nc.declare_dram_parameter("dbg_scT", [BI, S_TILES, 4], f32, isOutput=True)
        dbg_mx = nc.declare_dram_parameter("dbg_mx", [128, S_TILES, BI], f32, isOutput=True)

    with tile.TileContext(nc) as tc, ExitStack() as top:
        # ---------------- constants ----------------
        const = top.enter_context(tc.tile_pool(name="const", bufs=1))
        ident_f32 = const.tile([128, 128], f32)
        make_identity(nc, ident_f32)
        ones32 = const.tile([32, 1], f32)
        nc.gpsimd.memset(ones32, 1.0)
        ones128 = const.tile([128, 1], f32)
        nc.gpsimd.memset(ones128, 1.0)

        # ---------------- token masks (device-side) ----------------
        mpool = top.enter_context(tc.tile_pool(name="masks", bufs=1))
        dram = top.enter_context(tc.tile_pool(name="dram", bufs=1, space="DRAM"))

        # per-image region mask [BI, NREG]: n < im_len-1
        imlen_sb = mpool.tile([BI, 1], i32)
        nc.sync.dma_start(imlen_sb, imlen_ext.rearrange("(p o) -> p o", o=1))
        il_sb = mpool.tile([BI, 1], f32)
        nc.gpsimd.tensor_scalar(il_sb, imlen_sb, 1, None, op0=ALU.subtract)
        iota_r = mpool.tile([BI, NREG], f32)
        nc.gpsimd.iota(
            iota_r, pattern=[[1, NREG]], base=0, channel_multiplier=0,
            allow_small_or_imprecise_dtypes=True,
        )
        mask_im = mpool.tile([BI, NREG], f32)
        nc.gpsimd.tensor_scalar(mask_im, iota_r, il_sb, None, op0=ALU.is_lt)
        # maskcol_im [128, 11]: per (partition, im stage tile) in REGION-major
        # stage order (p = ni*n + i); pad rows -> 0
        mask_im_dram = dram.tile([BI, NREG], f32)
        nc.sync.dma_start(mask_im_dram[:, :], mask_im)
        maskcol_im = mpool.tile([128, len(IM_STAGE)], f32)
        nc.gpsimd.memset(maskcol_im, 0.0)
        for t, (toff, win, i0, ni) in enumerate(IM_STAGE):
            nc.sync.dma_start(
                maskcol_im[0:36 * ni, t:t + 1],
                mask_im_dram[i0:i0 + ni, :].rearrange("i n -> n i"),
            )

        # s word masks -> w4 block-ones weights [128, 64, 4] bf16:
        #   w4[32*jj + m, wt, jj] = (m < s_len[4*wt+jj] - 3)
        slen_sb = mpool.tile([128, 2], i32)
        nc.sync.dma_start(slen_sb, slen_ext.rearrange("(t p) -> p t", p=128))
        sl_sb = mpool.tile([128, 2], f32)
        nc.gpsimd.tensor_scalar(sl_sb, slen_sb, 3, None, op0=ALU.subtract)
        iota_w = mpool.tile([128, NWORD], f32)
        nc.gpsimd.iota(
            iota_w, pattern=[[1, NWORD]], base=0, channel_multiplier=0,
            allow_small_or_imprecise_dtypes=True,
        )
        # Word-sum weights for WORD-major stage order (p = 4w + j) combined
        # with the SwInterleave token reversal (partition p <-> raw col 127-p):
        #   w4[p, wt, jj] = [ (127-p)%4 == jj ] * ( (127-p)//4 < sl[4*wt+jj] )
        # Built transposed (partition c = sentence-within-half, free p) then
        # PE-transposed into place.
        rb = mpool.tile([128, 128], f32)     # rb[c, p] = (127-p)//4
        nc.gpsimd.iota(rb, pattern=[[-1, 32], [0, 4]], base=31,
                       channel_multiplier=0, allow_small_or_imprecise_dtypes=True)
        # sel[c, p] = (p%4 == 3 - c%4)  <=>  ((c + p + 1) & 3 == 0)
        cp_i = mpool.tile([128, 128], i32)
        nc.gpsimd.iota(cp_i, pattern=[[1, 128]], base=1, channel_multiplier=1)
        cp_a = mpool.tile([128, 128], i32)
        nc.vector.tensor_scalar(cp_a, cp_i, 3, None, op0=ALU.bitwise_and)
        sel = mpool.tile([128, 128], f32)
        nc.vector.tensor_scalar(sel, cp_a, 0, None, op0=ALU.is_equal)
        w4 = mpool.tile([128, S_TILES, 4], bf16)
        with tc.tile_pool(name="w4ps", bufs=2, space="PSUM") as wps:
            for h in range(2):
                bh = mpool.tile([128, 128], f32, tag=f"w4bh{h}")
                nc.vector.tensor_scalar(
                    bh, rb, sl_sb[:, h:h + 1], None, op0=ALU.is_lt
                )
                bm = mpool.tile([128, 128], f32, tag=f"w4bm{h}")
                nc.vector.tensor_mul(bm, bh, sel)
                wt_ps = wps.tile([128, 128], f32, tag=f"w4t{h}")
                nc.tensor.transpose(wt_ps, bm, ident_f32)
                nc.vector.tensor_copy(
                    w4[:, 32 * h:32 * (h + 1), :].rearrange("p a b -> p (a b)"),
                    wt_ps,
                )
                if opts.get("debug"):
                    nc.sync.dma_start(dbg_bm[h, :, :], bm)
        if opts.get("debug"):
            nc.sync.dma_start(dbg_sel[:, :], sel)
            nc.sync.dma_start(dbg_rb[:, :], rb)

        # diag masks (sharding metadata inputs)
        dmask_sb = mpool.tile([128, 2, BI], f32)
        nc.sync.dma_start(dmask_sb, dmask_ext.rearrange("(t p) i -> p t i", p=128))
        dmaskT_sb = mpool.tile([BI, 2, 128], f32)
        nc.sync.dma_start(dmaskT_sb, dmaskT_ext.rearrange("p (t f) -> p t f", f=128))

        # ---------------- persistent big buffers ----------------
        big = top.enter_context(tc.tile_pool(name="big", bufs=1))
        # packed-transposed fp8 pairs, stored as bf16 bit containers.
        # One tile per staging unit so the (whole-tile) dependency tracking
        # of the XBAR DMA writes stays exactly per-tile.
        imTp_t = [big.tile([128, 4, 112], bf16, name=f"imTp{t}") for t in range(len(IM_STAGE))]
        imP = big.tile([128, 4, 2, IM_TOK], fp8)      # dense planar im
        sTp_g = [
            big.tile([128, 4, 4, 128], bf16, name=f"sTpg{g}")
            for g in range(S_TILES // 4)
        ]
        maxima = big.tile([128, S_TILES, BI], bf16)  # per (word, wtile, img) region-max
        scoresT_sb = big.tile([BI, S_TILES, 4], f32)  # [img, wt, jj] == [img, sent]
        scores_sb = big.tile([128, 2, BI], f32)       # [sent%128, sent//128, img]

        # fp8 views: im pair-split for the deinterleave, s raw for SwInterleave
        imTp8_t = [
            t.bitcast(fp8).rearrange("p q (t b) -> p q b t", b=2) for t in imTp_t
        ]
        sTraw_g = [t.bitcast(fp8) for t in sTp_g]     # each [128, 4, 4, 256]

        with ExitStack() as mid:
            stage = mid.enter_context(
                tc.tile_pool(name="stage", bufs=opts["sf_bufs"])
            )
            pk = mid.enter_context(tc.tile_pool(name="pk", bufs=opts["pk_bufs"]))
            alp = mid.enter_context(
                tc.tile_pool(name="alp", bufs=opts["alp_bufs"], space="PSUM")
            )
            scp = mid.enter_context(tc.tile_pool(name="scp", bufs=1, space="PSUM"))
            scoresT_ps = scp.tile([BI, S_TILES, 4], f32)

            def stage_im(t):
                toff, win, i0, ni = IM_STAGE[t]
                nreal = 36 * ni
                tf32 = stage.tile([128, D], f32, tag="sf32")
                nc.sync.dma_start(
                    tf32[0:nreal, :],
                    im_ext[i0:i0 + ni, 1:1 + NREG, :].rearrange("i n d -> n i d"),
                )
                tbf = pk.tile([128, D // 2], bf16, tag="spk")
                # masked cast on the (ramp-idle) vector engine
                nc.vector.tensor_scalar(
                    tbf.bitcast(fp8)[0:win, :], tf32[0:win, :],
                    maskcol_im[0:win, t:t + 1], None, op0=ALU.mult,
                )
                # XBAR into the tile's private buffer (scalar queue: keeps
                # the sync queue free to prefetch s loads without blocking)
                nc.scalar.dma_start(
                    imTp_t[t][:, :, 0:win], tbf[0:win, :], transpose=True
                )
                # deinterleave + compact + un-permute (region-major stage
                # order ni*n + i -> dense 36*i + n) in one strided copy
                nc.vector.tensor_copy(
                    imP[:, :, :, toff:toff + nreal].rearrange(
                        "p q b (i n) -> p q b i n", n=NREG
                    ),
                    imTp8_t[t][:, :, :, 0:nreal].rearrange(
                        "p q b (n i) -> p q b n i", i=ni
                    ).rearrange("p q b n i -> p q b i n"),
                )

            def s_load(gq):
                # four per-tile loads (the DMA AP balancer caps at 3 dims,
                # so a quad can't be one DMA), word-major per tile
                tfs = []
                for a in range(4):
                    i = 4 * gq + a
                    tf32 = stage.tile([128, D], f32, tag="sf32")
                    nc.sync.dma_start(
                        tf32,
                        s_ext[4 * i:4 * i + 4, 1:1 + NWORD, :].rearrange(
                            "j w d -> w j d"
                        ),
                    )
                    tfs.append(tf32)
                return tfs

            def s_pack(gq, tfs):
                pkq = pk.tile([128, 4, D // 2], bf16, tag="spk")
                gsp = opts["gpsimd_cast"]
                for a in range(4):
                    i = 4 * gq + a
                    if gsp and (i % gsp == gsp - 1):
                        nc.vector.tensor_copy(
                            pkq[:, a, :].bitcast(fp8), tfs[a]
                        )
                    else:
                        nc.scalar.activation(
                            pkq[:, a, :].bitcast(fp8), tfs[a], ACTF.Copy
                        )
                nc.sync.dma_start(
                    sTp_g[gq].rearrange("p a q t -> p (a q) t"),
                    pkq.rearrange("p a c -> p (a c)"),
                    transpose=True,
                )

            # im head first (needed by rc0); s quad-loads run ahead of their
            # cast+transpose stages so no dispatch queue ever blocks
            PRE = opts["s_prefetch"]
            for t in range(len(IM_STAGE)):
                stage_im(t)
            pending = []
            for gq in range(S_TILES // 4):
                pending.append((gq, s_load(gq)))
                if len(pending) > PRE:
                    s_pack(*pending.pop(0))
            for it in pending:
                s_pack(*it)

            # ---------------- main matmul + region-max + word-sum ----------------
            def word_sum(wt):
                # scoresT[img, 4wt+jj] = sum_m maxima[(jj,m), wt, img] * wmask
                nc.tensor.matmul(
                    scoresT_ps[:, wt, :],
                    lhsT=maxima[:, wt, :],
                    rhs=w4[:, wt, :],
                    start=True, stop=True,
                )

            for wt in range(S_TILES):
                for rci, (toff, ntok, nimg) in enumerate(RCHUNKS):
                    pal = alp.tile([128, nimg, NREG], f32, tag="align")
                    for q in range(4):
                        nc.tensor.matmul(
                            pal.rearrange("p a b -> p (a b)"),
                            lhsT=sTraw_g[wt // 4][:, wt % 4, q, :],
                            rhs=imP[:, q, :, toff:toff + ntok],
                            start=(q == 0),
                            stop=(q == 3),
                            perf_mode=SWI,
                        )
                    nc.vector.tensor_reduce(
                        maxima[:, wt, toff // NREG:toff // NREG + nimg],
                        pal, axis=AX.X, op=ALU.max,
                    )
                    # emit the previous tile's word-sum between rc chunks so the
                    # PE never waits on the vector MAX of the current tile
                    if rci == 0 and wt > 0:
                        word_sum(wt - 1)
            word_sum(S_TILES - 1)

            # scoresT -> SBUF, then transpose back to [sent, img]
            nc.vector.tensor_copy(scoresT_sb, scoresT_ps)
            if opts.get("debug"):
                w4d = mpool.tile([128, S_TILES, 4], f32)
                nc.vector.tensor_copy(w4d, w4)
                nc.sync.dma_start(dbg_w4[:, :, :], w4d)
                nc.sync.dma_start(dbg_scT[:, :, :], scoresT_sb)
                mxd = mpool.tile([128, S_TILES, BI], f32)
                nc.vector.tensor_copy(mxd, maxima)
                nc.sync.dma_start(dbg_mx[:, :, :], mxd)
            sc_ps = scp.tile([128, 2, BI], f32)
            for t in range(2):
                nc.tensor.transpose(
                    sc_ps[:, t, :],
                    scoresT_sb[:, 32 * t:32 * (t + 1), :].rearrange(
                        "p a b -> p (a b)"
                    ),
                    ident_f32[:BI, :BI],
                )
                nc.vector.tensor_copy(scores_sb[:, t, :], sc_ps[:, t, :])

        # ---------------- loss tail ----------------
        with ExitStack() as tail:
            tp = tail.enter_context(tc.tile_pool(name="tailp", bufs=1, space="PSUM"))
            ts = tail.enter_context(tc.tile_pool(name="tails", bufs=1))

            # col-max over local images (diag excluded) + scattered diag
            masked = ts.tile([128, 2, BI], f32)
            nc.vector.scalar_tensor_tensor(
                masked, dmask_sb, -BIG, scores_sb, op0=ALU.mult, op1=ALU.add
            )
            colmax_p = ts.tile([128, 2], f32)
            nc.vector.tensor_reduce(colmax_p, masked, axis=AX.X, op=ALU.max)
            dtmp = ts.tile([128, 2, BI], f32)
            nc.vector.tensor_mul(dtmp, dmask_sb, scores_sb)
            dscat = ts.tile([128, 2], f32)
            nc.vector.tensor_reduce(dscat, dtmp, axis=AX.X, op=ALU.add)

            # row-max over sentences (diag excluded); scoresT_sb is [img, sent]
            scT_flat = scoresT_sb.rearrange("p a b -> p (a b)")
            dmaskT_flat = dmaskT_sb.rearrange("p a b -> p (a b)")
            maskedT = ts.tile([BI, B], f32)
            nc.vector.scalar_tensor_tensor(
                maskedT, dmaskT_flat, -BIG, scT_flat, op0=ALU.mult, op1=ALU.add
            )
            rowmax = ts.tile([BI, 1], f32)
            nc.vector.tensor_reduce(rowmax, maskedT, axis=AX.X, op=ALU.max)
            dT_tmp = ts.tile([BI, B], f32)
            nc.vector.tensor_mul(dT_tmp, dmaskT_flat, scT_flat)
            d_row = ts.tile([BI, 1], f32)
            nc.vector.tensor_reduce(d_row, dT_tmp, axis=AX.X, op=ALU.add)

            cost_s = ts.tile([BI, 1], f32)
            nc.vector.tensor_sub(cost_s, rowmax, d_row)
            nc.vector.tensor_scalar(
                cost_s, cost_s, MARGIN, 0.0, op0=ALU.add, op1=ALU.max
            )
            cs_ps = tp.tile([1, 1], f32)
            nc.tensor.matmul(cs_ps, lhsT=ones32, rhs=cost_s, start=True, stop=True)
            cs_sb = ts.tile([1, 8], f32)
            nc.gpsimd.memset(cs_sb, 0.0)
            nc.vector.tensor_copy(cs_sb[:, 0:1], cs_ps)

            # pack allgather block: [0:256) colmax | [256:512) dscat | 512 cost_s
            blk = dram.tile([BLK], f32)
            nc.sync.dma_start(
                blk[0:256].rearrange("(t p) -> p t", p=128), colmax_p
            )
            nc.sync.dma_start(
                blk[256:512].rearrange("(t p) -> p t", p=128), dscat
            )
            nc.sync.dma_start(blk[512:520], cs_sb[0, :])
            gath = dram.tile([NCORES, BLK], f32, addr_space="Shared")
            nc.gpsimd.collective_compute(
                "AllGather",
                ALU.bypass,
                ins=[blk.opt()],
                outs=[gath.opt()],
                replica_groups=[list(range(NCORES))],
            )

            # redundant final reduction on every core
            g_cm = ts.tile([128, 2, NCORES], f32)
            g_d = ts.tile([128, 2, NCORES], f32)
            for t in range(2):
                nc.sync.dma_start(
                    g_cm[:, t, :],
                    gath[:, 128 * t:128 * (t + 1)].rearrange("c p -> p c"),
                )
                nc.sync.dma_start(
                    g_d[:, t, :],
                    gath[:, 256 + 128 * t:256 + 128 * (t + 1)].rearrange("c p -> p c"),
                )
            g_cs = ts.tile([1, NCORES], f32)
            nc.sync.dma_start(g_cs, gath[:, 512:513].rearrange("a b -> b a"))

            colmax_g = ts.tile([128, 2], f32)
            nc.vector.tensor_reduce(colmax_g, g_cm, axis=AX.X, op=ALU.max)
            d_all = ts.tile([128, 2], f32)
            nc.vector.tensor_reduce(d_all, g_d, axis=AX.X, op=ALU.add)
            cim = ts.tile([128, 2], f32)
            nc.vector.tensor_sub(cim, colmax_g, d_all)
            nc.vector.tensor_scalar(cim, cim, MARGIN, 0.0, op0=ALU.add, op1=ALU.max)
            cim_r = ts.tile([128, 1], f32)
            nc.vector.tensor_reduce(cim_r, cim, axis=AX.X, op=ALU.add)
            tot_ps = tp.tile([1, 1], f32)
            nc.tensor.matmul(tot_ps, lhsT=ones128, rhs=cim_r, start=True, stop=True)
            cs_tot = ts.tile([1, 1], f32)
            nc.vector.tensor_reduce(cs_tot, g_cs, axis=AX.X, op=ALU.add)
            total = ts.tile([1, 1], f32)
            nc.vector.tensor_add(total, tot_ps, cs_tot)
            nc.sync.dma_start(out_ext[0:1], total[0, :])

    fix_multiwaits(nc, mybir)
    return nc


_CACHE = {}


def _get_nc():
    if "nc" not in _CACHE:
        _CACHE["nc"] = build_graph()
    return _CACHE["nc"]


def make_in_maps(im_set, s_seq, im_len, s_len):
    im_set = np.ascontiguousarray(im_set, dtype=np.float32)
    s_seq = np.ascontiguousarray(s_seq, dtype=np.float32)
    im_len = np.ascontiguousarray(im_len, dtype=np.int32)
    s_len = np.ascontiguousarray(s_len, dtype=np.int32)
    in_maps = []
    for c in range(NCORES):
        dm = np.zeros((B, BI), dtype=np.float32)
        for i in range(BI):
            dm[BI * c + i, i] = 1.0
        in_maps.append({
            "im_set": im_set[BI * c:BI * (c + 1)],
            "s_seq": s_seq,
            "im_len": im_len[BI * c:BI * (c + 1)],
            "s_len": s_len,
            "diag_mask": dm,
            "diag_maskT": np.ascontiguousarray(dm.T),
        })
    return in_maps


def kernel(im_set, s_seq, im_len, s_len):
    import time
    from concourse.bass_utils import run_bass_kernel_spmd

    nc = _get_nc()
    in_maps = make_in_maps(im_set, s_seq, im_len, s_len)
    last = None
    for attempt in range(3):
        try:
            res = run_bass_kernel_spmd(nc, in_maps, core_ids=list(range(NCORES)))
            return np.asarray(
                res.results[0]["out"], dtype=np.float32
            ).reshape(())[()]
        except Exception as e:  # transient device-unrecoverable happens
            last = e
            time.sleep(30 * (attempt + 1))
    raise last


# revision 27
# speedup vs baseline: 1.5878x; 1.0681x over previous
"""Trainium2 Bass kernel for AlignmentContrastiveLoss (8 NeuronCores, SPMD).

Reference semantics:
  im = im_set[:, 1:, :]           [256, 36, 1024]
  s  = s_seq[:, 1:-2, :]          [256, 32, 1024]
  align[i,j,n,m] = im[i,n] . s[j,m], zeroed where n >= im_len[i]-1 or m >= s_len[j]-3
  scores[i,j] = sum_m max_n align[i,j,n,m]
  loss = sum_i relu(M + max_{j!=i} scores[i,j] - scores[i,i])
       + sum_j relu(M + max_{i!=j} scores[i,j] - scores[j,j])

Sharding: data-parallel over images (32 per core), s replicated.

v2 design:
  - f32 tokens are cast once to fp8e4 (im-mask fused as activation scale),
    bitcast to 16-bit fp8-pairs and transposed via the DMA XBAR (pure bit
    movement) into a packed layout: partition p of q-chunk q holds the d
    pair (256q+2p, 256q+2p+1) interleaved per token (HW-verified mapping).
  - s feeds the PE as RAW packed bytes via MatmulPerfMode.DoubleRowSwInterleave
    (stationary side accepts interleaved pairs; output partitions come out
    token-REVERSED, verified on HW). im (small) is deinterleaved to planar
    [128, q, 2, tok] fp8 by one gpsimd 4D copy per tile, which also compacts
    away the XBAR pad columns. No PE transposes, no PSUM->SBUF staging copies.
  - The s-token reversal is compensated in the word-sum weights (w4 built
    from a reversed word mask), so scoresT and the loss tail are unchanged.
  - wt-outer loop: per s-tile, 3 region-chunk matmul groups + vector MAX
    reduce; word-sum is a tiny PE matmul against s-mask-weighted block-ones
    (applies the s word mask for free and directly yields scoresT[img,sent]).
  - im staged as 11 tiles of 3 images (112-partition windows, 4-col overlap
    into the next tile's range which is later overwritten with real data).
  - Cross-core traffic: one 520-float AllGather of per-core column-max
    partials + scattered diagonals + local cost_s sum.
"""

import numpy as np

MARGIN = 0.2
B = 256          # global batch (images == sentences)
NCORES = 8
BI = B // NCORES  # images per core = 32
NREG = 36        # regions per image after stripping
NWORD = 32       # words per sentence after stripping
D = 1024
IM_TOK = BI * NREG      # 1152 dense im tokens
S_TOK = B * NWORD       # 8192 s tokens
S_TILES = S_TOK // 128  # 64
BIG = 1.0e30
# region chunks for the main matmul: (token offset, ntok, nimg)
RCHUNKS = [(0, 432, 12), (432, 432, 12), (864, 288, 8)]
# im staging tiles: (dense token offset, window (mult of 16), first image, n images)
IM_STAGE = [(108 * t, 112, 3 * t, 3) for t in range(10)] + [(1080, 80, 30, 2)]
IM_TP_COLS = 112 * 11  # padded XBAR destination: disjoint 112-col windows
BLK = 520  # allgather block floats: 256 colmax | 256 diag-scatter | 1 cost_s | pad


def fix_multiwaits(nc, mybir):
    """This toolchain's walrus accepts 1 wait per instruction (2 for
    EventSemaphore); Tile can emit more. Offload surplus waits onto
    inserted same-engine NoOps placed immediately before the instruction."""
    n_fix = 0
    for fn in nc.m.functions:
        for blk in fn.blocks:
            insts = blk.instructions
            i = 0
            while i < len(insts):
                inst = insts[i]
                si = inst.sync_info
                waits = list(si.on_wait) if si is not None and si.on_wait else []
                cap = 2 if isinstance(inst, mybir.InstEventSemaphore) else 1
                if len(waits) > cap:
                    surplus, keep = waits[:-cap], waits[-cap:]
                    si.on_wait = keep
                    for w in surplus:
                        nop = mybir.InstNoOp(
                            name=f"{inst.name}_wsplit{n_fix}",
                            engine=inst.engine,
                            ins=[],
                            outs=[],
                            sync_info=mybir.SyncInfo(on_wait=[w], on_update=[]),
                        )
                        insts.insert(i, nop)
                        n_fix += 1
                        i += 1
                i += 1
    return n_fix


DEFAULT_OPTS = {
    "sf_bufs": 16,     # f32 staging tiles
    "pk_bufs": 5,      # packed fp8-as-bf16 staging quad tiles
    "alp_bufs": 6,     # PSUM align buffers
    "gpsimd_cast": 0,  # every Nth s cast on vector (0 = all scalar)
    "s_prefetch": 3,   # s quad-loads dispatched ahead of their pack stage
    "im_head": 4,      # im tiles staged before the first s tile
}


def build_graph(opts=None):
    import concourse.bass as bass
    import concourse.mybir as mybir
    import concourse.tile as tile
    from concourse.masks import make_identity
    from contextlib import ExitStack

    opts = {**DEFAULT_OPTS, **(opts or {})}

    f32 = mybir.dt.float32
    bf16 = mybir.dt.bfloat16
    fp8 = mybir.dt.float8e4
    i32 = mybir.dt.int32
    ALU = mybir.AluOpType
    AX = mybir.AxisListType
    ACTF = mybir.ActivationFunctionType
    SWI = mybir.MatmulPerfMode.DoubleRowSwInterleave

    nc = bass.Bass()

    im_ext = nc.declare_dram_parameter("im_set", [BI, 37, D], f32, isOutput=False)
    s_ext = nc.declare_dram_parameter("s_seq", [B, 35, D], f32, isOutput=False)
    imlen_ext = nc.declare_dram_parameter("im_len", [BI], i32, isOutput=False)
    slen_ext = nc.declare_dram_parameter("s_len", [B], i32, isOutput=False)
    dmask_ext = nc.declare_dram_parameter("diag_mask", [B, BI], f32, isOutput=False)
    dmaskT_ext = nc.declare_dram_parameter("diag_maskT", [BI, B], f32, isOutput=False)
    out_ext = nc.declare_dram_parameter("out", [1], f32, isOutput=True)
    if opts.get("debug"):
        dbg_sel = nc.declare_dram_parameter("dbg_sel", [128, 128], f32, isOutput=True)
        dbg_pm = nc.declare_dram_parameter("dbg_pm", [128, 128], f32, isOutput=True)
        dbg_rb = nc.declare_dram_parameter("dbg_rb", [128, 128], f32, isOutput=True)
        dbg_tc = nc.declare_dram_parameter("dbg_tc", [128, 2], f32, isOutput=True)
        dbg_bm = nc.declare_dram_parameter("dbg_bm", [2, 128, 128], f32, isOutput=True)
        dbg_w4 = nc.declare_dram_parameter("dbg_w4", [128, S_TILES, 4], f32, isOutput=True)
        dbg_scT = nc.declare_dram_parameter("dbg_scT", [BI, S_TILES, 4], f32, isOutput=True)
        dbg_mx = nc.declare_dram_parameter("dbg_mx", [128, S_TILES, BI], f32, isOutput=True)

    with tile.TileContext(nc) as tc, ExitStack() as top:
        # ---------------- constants ----------------
        const = top.enter_context(tc.tile_pool(name="const", bufs=1))
        ident_f32 = const.tile([128, 128], f32)
        make_identity(nc, ident_f32)
        ones32 = const.tile([32, 1], f32)
        nc.gpsimd.memset(ones32, 1.0)
        ones128 = const.tile([128, 1], f32)
        nc.gpsimd.memset(ones128, 1.0)

        # ---------------- token masks (device-side) ----------------
        mpool = top.enter_context(tc.tile_pool(name="masks", bufs=1))
        dram = top.enter_context(tc.tile_pool(name="dram", bufs=1, space="DRAM"))

        # per-image region mask [BI, NREG]: n < im_len-1
        imlen_sb = mpool.tile([BI, 1], i32)
        nc.sync.dma_start(imlen_sb, imlen_ext.rearrange("(p o) -> p o", o=1))
        il_sb = mpool.tile([BI, 1], f32)
        nc.gpsimd.tensor_scalar(il_sb, imlen_sb, 1, None, op0=ALU.subtract)
        iota_r = mpool.tile([BI, NREG], f32)
        nc.gpsimd.iota(
            iota_r, pattern=[[1, NREG]], base=0, channel_multiplier=0,
            allow_small_or_imprecise_dtypes=True,
        )
        mask_im = mpool.tile([BI, NREG], f32)
        nc.gpsimd.tensor_scalar(mask_im, iota_r, il_sb, None, op0=ALU.is_lt)
        # maskcol_im [128, 11]: per (partition, im stage tile) in REGION-major
        # stage order (p = ni*n + i); pad rows -> 0
        mask_im_dram = dram.tile([BI, NREG], f32)
        nc.sync.dma_start(mask_im_dram[:, :], mask_im)
        maskcol_im = mpool.tile([128, len(IM_STAGE)], f32)
        nc.gpsimd.memset(maskcol_im, 0.0)
        for t, (toff, win, i0, ni) in enumerate(IM_STAGE):
            nc.sync.dma_start(
                maskcol_im[0:36 * ni, t:t + 1],
                mask_im_dram[i0:i0 + ni, :].rearrange("i n -> n i"),
            )

        # s word masks -> w4 block-ones weights [128, 64, 4] bf16:
        #   w4[32*jj + m, wt, jj] = (m < s_len[4*wt+jj] - 3)
        slen_sb = mpool.tile([128, 2], i32)
        nc.sync.dma_start(slen_sb, slen_ext.rearrange("(t p) -> p t", p=128))
        sl_sb = mpool.tile([128, 2], f32)
        nc.gpsimd.tensor_scalar(sl_sb, slen_sb, 3, None, op0=ALU.subtract)
        iota_w = mpool.tile([128, NWORD], f32)
        nc.gpsimd.iota(
            iota_w, pattern=[[1, NWORD]], base=0, channel_multiplier=0,
            allow_small_or_imprecise_dtypes=True,
        )
        # Word-sum weights for WORD-major stage order (p = 4w + j) combined
        # with the SwInterleave token reversal (partition p <-> raw col 127-p):
        #   w4[p, wt, jj] = [ (127-p)%4 == jj ] * ( (127-p)//4 < sl[4*wt+jj] )
        # Built transposed (partition c = sentence-within-half, free p) then
        # PE-transposed into place.
        rb = mpool.tile([128, 128], f32)     # rb[c, p] = (127-p)//4
        nc.gpsimd.iota(rb, pattern=[[-1, 32], [0, 4]], base=31,
                       channel_multiplier=0, allow_small_or_imprecise_dtypes=True)
        # sel[c, p] = (p%4 == 3 - c%4)  <=>  ((c + p + 1) & 3 == 0)
        cp_i = mpool.tile([128, 128], i32)
        nc.gpsimd.iota(cp_i, pattern=[[1, 128]], base=1, channel_multiplier=1)
        cp_a = mpool.tile([128, 128], i32)
        nc.vector.tensor_scalar(cp_a, cp_i, 3, None, op0=ALU.bitwise_and)
        sel = mpool.tile([128, 128], f32)
        nc.vector.tensor_scalar(sel, cp_a, 0, None, op0=ALU.is_equal)
        w4 = mpool.tile([128, S_TILES, 4], bf16)
        with tc.tile_pool(name="w4ps", bufs=2, space="PSUM") as wps:
            for h in range(2):
                bh = mpool.tile([128, 128], f32, tag=f"w4bh{h}")
                nc.vector.tensor_scalar(
                    bh, rb, sl_sb[:, h:h + 1], None, op0=ALU.is_lt
                )
                bm = mpool.tile([128, 128], f32, tag=f"w4bm{h}")
                nc.vector.tensor_mul(bm, bh, sel)
                wt_ps = wps.tile([128, 128], f32, tag=f"w4t{h}")
                nc.tensor.transpose(wt_ps, bm, ident_f32)
                nc.vector.tensor_copy(
                    w4[:, 32 * h:32 * (h + 1), :].rearrange("p a b -> p (a b)"),
                    wt_ps,
                )
                if opts.get("debug"):
                    nc.sync.dma_start(dbg_bm[h, :, :], bm)
        if opts.get("debug"):
            nc.sync.dma_start(dbg_sel[:, :], sel)
            nc.sync.dma_start(dbg_rb[:, :], rb)

        # diag masks (sharding metadata inputs)
        dmask_sb = mpool.tile([128, 2, BI], f32)
        nc.sync.dma_start(dmask_sb, dmask_ext.rearrange("(t p) i -> p t i", p=128))
        dmaskT_sb = mpool.tile([BI, 2, 128], f32)
        nc.sync.dma_start(dmaskT_sb, dmaskT_ext.rearrange("p (t f) -> p t f", f=128))

        # ---------------- persistent big buffers ----------------
        big = top.enter_context(tc.tile_pool(name="big", bufs=1))
        # packed-transposed fp8 pairs, stored as bf16 bit containers.
        # One tile per staging unit so the (whole-tile) dependency tracking
        # of the XBAR DMA writes stays exactly per-tile.
        IMQ = [[0, 1, 2, 3], [4, 5, 6, 7], [8, 9], [10]]
        imTq_g = [
            big.tile([128, 4 * len(ts), 112], bf16, name=f"imTq{g}")
            for g, ts in enumerate(IMQ)
        ]
        imP = big.tile([128, 4, 2, IM_TOK], fp8)      # dense planar im
        sTp_g = [
            big.tile([128, 4, 4, 128], bf16, name=f"sTpg{g}")
            for g in range(S_TILES // 4)
        ]
        maxima = big.tile([128, S_TILES, BI], bf16)  # per (word, wtile, img) region-max
        scoresT_sb = big.tile([BI, S_TILES, 4], f32)  # [img, wt, jj] == [img, sent]
        scores_sb = big.tile([128, 2, BI], f32)       # [sent%128, sent//128, img]

        # fp8 views: im pair-split for the deinterleave, s raw for SwInterleave
        imTq8_g = [
            t.bitcast(fp8).rearrange("p m (t b) -> p m b t", b=2) for t in imTq_g
        ]
        sTraw_g = [t.bitcast(fp8) for t in sTp_g]     # each [128, 4, 4, 256]

        with ExitStack() as mid:
            stage = mid.enter_context(
                tc.tile_pool(name="stage", bufs=opts["sf_bufs"])
            )
            pk = mid.enter_context(tc.tile_pool(name="pk", bufs=opts["pk_bufs"]))
            alp = mid.enter_context(
                tc.tile_pool(name="alp", bufs=opts["alp_bufs"], space="PSUM")
            )
            scp = mid.enter_context(tc.tile_pool(name="scp", bufs=1, space="PSUM"))
            scoresT_ps = scp.tile([BI, S_TILES, 4], f32)

            def stage_im_quad(g):
                ts = IMQ[g]
                win_g = IM_STAGE[ts[-1]][1]
                tf32s = []
                for t in ts:
                    toff, win, i0, ni = IM_STAGE[t]
                    nreal = 36 * ni
                    tf32 = stage.tile([128, D], f32, tag="sf32")
                    nc.sync.dma_start(
                        tf32[0:nreal, :],
                        im_ext[i0:i0 + ni, 1:1 + NREG, :].rearrange(
                            "i n d -> n i d"
                        ),
                    )
                    tf32s.append(tf32)
                ipkq = pk.tile([128, len(ts), D // 2], bf16, tag="spk")
                for k, t in enumerate(ts):
                    toff, win, i0, ni = IM_STAGE[t]
                    # masked cast on the (ramp-idle) vector engine
                    nc.vector.tensor_scalar(
                        ipkq[:, k, :].bitcast(fp8)[0:win, :], tf32s[k][0:win, :],
                        maskcol_im[0:win, t:t + 1], None, op0=ALU.mult,
                    )
                # one XBAR DMA for the whole quad (scalar queue: keeps the
                # sync queue free to prefetch s loads without blocking)
                nc.scalar.dma_start(
                    imTq_g[g][:, :, 0:win_g],
                    ipkq[0:win_g, :, :].rearrange("p a c -> p (a c)"),
                    transpose=True,
                )
                # deinterleave + compact + un-permute (region-major stage
                # order ni*n + i -> dense 36*i + n), one strided copy per tile
                for k, t in enumerate(ts):
                    toff, win, i0, ni = IM_STAGE[t]
                    nreal = 36 * ni
                    nc.vector.tensor_copy(
                        imP[:, :, :, toff:toff + nreal].rearrange(
                            "p q b (i n) -> p q b i n", n=NREG
                        ),
                        imTq8_g[g][:, 4 * k:4 * k + 4, :, 0:nreal].rearrange(
                            "p q b (n i) -> p q b n i", i=ni
                        ).rearrange("p q b n i -> p q b i n"),
                    )

            def s_load(gq):
                # four per-tile loads (the DMA AP balancer caps at 3 dims,
                # so a quad can't be one DMA), word-major per tile
                tfs = []
                for a in range(4):
                    i = 4 * gq + a
                    tf32 = stage.tile([128, D], f32, tag="sf32")
                    nc.sync.dma_start(
                        tf32,
                        s_ext[4 * i:4 * i + 4, 1:1 + NWORD, :].rearrange(
                            "j w d -> w j d"
                        ),
                    )
                    tfs.append(tf32)
                return tfs

            def s_pack(gq, tfs):
                pkq = pk.tile([128, 4, D // 2], bf16, tag="spk")
                gsp = opts["gpsimd_cast"]
                for a in range(4):
                    i = 4 * gq + a
                    if gsp and (i % gsp == gsp - 1):
                        nc.vector.tensor_copy(
                            pkq[:, a, :].bitcast(fp8), tfs[a]
                        )
                    else:
                        nc.scalar.activation(
                            pkq[:, a, :].bitcast(fp8), tfs[a], ACTF.Copy
                        )
                nc.sync.dma_start(
                    sTp_g[gq].rearrange("p a q t -> p (a q) t"),
                    pkq.rearrange("p a c -> p (a c)"),
                    transpose=True,
                )

            # im head first (needed by rc0); s quad-loads run ahead of their
            # cast+transpose stages so no dispatch queue ever blocks
            PRE = opts["s_prefetch"]
            for g in range(len(IMQ)):
                stage_im_quad(g)
            pending = []
            for gq in range(S_TILES // 4):
                pending.append((gq, s_load(gq)))
                if len(pending) > PRE:
                    s_pack(*pending.pop(0))
            for it in pending:
                s_pack(*it)

            # ---------------- main matmul + region-max + word-sum ----------------
            def word_sum(wt):
                # scoresT[img, 4wt+jj] = sum_m maxima[(jj,m), wt, img] * wmask
                nc.tensor.matmul(
                    scoresT_ps[:, wt, :],
                    lhsT=maxima[:, wt, :],
                    rhs=w4[:, wt, :],
                    start=True, stop=True,
                )

            for wt in range(S_TILES):
                for rci, (toff, ntok, nimg) in enumerate(RCHUNKS):
                    pal = alp.tile([128, nimg, NREG], f32, tag="align")
                    for q in range(4):
                        nc.tensor.matmul(
                            pal.rearrange("p a b -> p (a b)"),
                            lhsT=sTraw_g[wt // 4][:, wt % 4, q, :],
                            rhs=imP[:, q, :, toff:toff + ntok],
                            start=(q == 0),
                            stop=(q == 3),
                            perf_mode=SWI,
                        )
                    nc.vector.tensor_reduce(
                        maxima[:, wt, toff // NREG:toff // NREG + nimg],
                        pal, axis=AX.X, op=ALU.max,
                    )
                    # emit the previous tile's word-sum between rc chunks so the
                    # PE never waits on the vector MAX of the current tile
                    if rci == 0 and wt > 0:
                        word_sum(wt - 1)
            word_sum(S_TILES - 1)

            # scoresT -> SBUF, then transpose back to [sent, img]
            nc.vector.tensor_copy(scoresT_sb, scoresT_ps)
            if opts.get("debug"):
                w4d = mpool.tile([128, S_TILES, 4], f32)
                nc.vector.tensor_copy(w4d, w4)
                nc.sync.dma_start(dbg_w4[:, :, :], w4d)
                nc.sync.dma_start(dbg_scT[:, :, :], scoresT_sb)
                mxd = mpool.tile([128, S_TILES, BI], f32)
                nc.vector.tensor_copy(mxd, maxima)
                nc.sync.dma_start(dbg_mx[:, :, :], mxd)
            sc_ps = scp.tile([128, 2, BI], f32)
            for t in range(2):
                nc.tensor.transpose(
                    sc_ps[:, t, :],
                    scoresT_sb[:, 32 * t:32 * (t + 1), :].rearrange(
                        "p a b -> p (a b)"
                    ),
                    ident_f32[:BI, :BI],
                )
                nc.vector.tensor_copy(scores_sb[:, t, :], sc_ps[:, t, :])

        # ---------------- loss tail ----------------
        with ExitStack() as tail:
            tp = tail.enter_context(tc.tile_pool(name="tailp", bufs=1, space="PSUM"))
            ts = tail.enter_context(tc.tile_pool(name="tails", bufs=1))

            # col-max over local images (diag excluded) + scattered diag
            masked = ts.tile([128, 2, BI], f32)
            nc.vector.scalar_tensor_tensor(
                masked, dmask_sb, -BIG, scores_sb, op0=ALU.mult, op1=ALU.add
            )
            colmax_p = ts.tile([128, 2], f32)
            nc.vector.tensor_reduce(colmax_p, masked, axis=AX.X, op=ALU.max)
            dtmp = ts.tile([128, 2, BI], f32)
            nc.vector.tensor_mul(dtmp, dmask_sb, scores_sb)
            dscat = ts.tile([128, 2], f32)
            nc.vector.tensor_reduce(dscat, dtmp, axis=AX.X, op=ALU.add)

            # row-max over sentences (diag excluded); scoresT_sb is [img, sent]
            scT_flat = scoresT_sb.rearrange("p a b -> p (a b)")
            dmaskT_flat = dmaskT_sb.rearrange("p a b -> p (a b)")
            maskedT = ts.tile([BI, B], f32)
            nc.vector.scalar_tensor_tensor(
                maskedT, dmaskT_flat, -BIG, scT_flat, op0=ALU.mult, op1=ALU.add
            )
            rowmax = ts.tile([BI, 1], f32)
            nc.vector.tensor_reduce(rowmax, maskedT, axis=AX.X, op=ALU.max)
            dT_tmp = ts.tile([BI, B], f32)
            nc.vector.tensor_mul(dT_tmp, dmaskT_flat, scT_flat)
            d_row = ts.tile([BI, 1], f32)
            nc.vector.tensor_reduce(d_row, dT_tmp, axis=AX.X, op=ALU.add)

            cost_s = ts.tile([BI, 1], f32)
            nc.vector.tensor_sub(cost_s, rowmax, d_row)
            nc.vector.tensor_scalar(
                cost_s, cost_s, MARGIN, 0.0, op0=ALU.add, op1=ALU.max
            )
            cs_ps = tp.tile([1, 1], f32)
            nc.tensor.matmul(cs_ps, lhsT=ones32, rhs=cost_s, start=True, stop=True)
            cs_sb = ts.tile([1, 8], f32)
            nc.gpsimd.memset(cs_sb, 0.0)
            nc.vector.tensor_copy(cs_sb[:, 0:1], cs_ps)

            # pack allgather block: [0:256) colmax | [256:512) dscat | 512 cost_s
            blk = dram.tile([BLK], f32)
            nc.sync.dma_start(
                blk[0:256].rearrange("(t p) -> p t", p=128), colmax_p
            )
            nc.sync.dma_start(
                blk[256:512].rearrange("(t p) -> p t", p=128), dscat
            )
            nc.sync.dma_start(blk[512:520], cs_sb[0, :])
            gath = dram.tile([NCORES, BLK], f32, addr_space="Shared")
            nc.gpsimd.collective_compute(
                "AllGather",
                ALU.bypass,
                ins=[blk.opt()],
                outs=[gath.opt()],
                replica_groups=[list(range(NCORES))],
            )

            # redundant final reduction on every core
            g_cm = ts.tile([128, 2, NCORES], f32)
            g_d = ts.tile([128, 2, NCORES], f32)
            for t in range(2):
                nc.sync.dma_start(
                    g_cm[:, t, :],
                    gath[:, 128 * t:128 * (t + 1)].rearrange("c p -> p c"),
                )
                nc.sync.dma_start(
                    g_d[:, t, :],
                    gath[:, 256 + 128 * t:256 + 128 * (t + 1)].rearrange("c p -> p c"),
                )
            g_cs = ts.tile([1, NCORES], f32)
            nc.sync.dma_start(g_cs, gath[:, 512:513].rearrange("a b -> b a"))

            colmax_g = ts.tile([128, 2], f32)
            nc.vector.tensor_reduce(colmax_g, g_cm, axis=AX.X, op=ALU.max)
            d_all = ts.tile([128, 2], f32)
            nc.vector.tensor_reduce(d_all, g_d, axis=AX.X, op=ALU.add)
            cim = ts.tile([128, 2], f32)
            nc.vector.tensor_sub(cim, colmax_g, d_all)
            nc.vector.tensor_scalar(cim, cim, MARGIN, 0.0, op0=ALU.add, op1=ALU.max)
            cim_r = ts.tile([128, 1], f32)
            nc.vector.tensor_reduce(cim_r, cim, axis=AX.X, op=ALU.add)
            tot_ps = tp.tile([1, 1], f32)
            nc.tensor.matmul(tot_ps, lhsT=ones128, rhs=cim_r, start=True, stop=True)
            cs_tot = ts.tile([1, 1], f32)
            nc.vector.tensor_reduce(cs_tot, g_cs, axis=AX.X, op=ALU.add)
            total = ts.tile([1, 1], f32)
            nc.vector.tensor_add(total, tot_ps, cs_tot)
            nc.sync.dma_start(out_ext[0:1], total[0, :])

    fix_multiwaits(nc, mybir)
    return nc


_CACHE = {}


def _get_nc():
    if "nc" not in _CACHE:
        _CACHE["nc"] = build_graph()
    return _CACHE["nc"]


def make_in_maps(im_set, s_seq, im_len, s_len):
    im_set = np.ascontiguousarray(im_set, dtype=np.float32)
    s_seq = np.ascontiguousarray(s_seq, dtype=np.float32)
    im_len = np.ascontiguousarray(im_len, dtype=np.int32)
    s_len = np.ascontiguousarray(s_len, dtype=np.int32)
    in_maps = []
    for c in range(NCORES):
        dm = np.zeros((B, BI), dtype=np.float32)
        for i in range(BI):
            dm[BI * c + i, i] = 1.0
        in_maps.append({
            "im_set": im_set[BI * c:BI * (c + 1)],
            "s_seq": s_seq,
            "im_len": im_len[BI * c:BI * (c + 1)],
            "s_len": s_len,
            "diag_mask": dm,
            "diag_maskT": np.ascontiguousarray(dm.T),
        })
    return in_maps


def kernel(im_set, s_seq, im_len, s_len):
    import time
    from concourse.bass_utils import run_bass_kernel_spmd

    nc = _get_nc()
    in_maps = make_in_maps(im_set, s_seq, im_len, s_len)
    last = None
    for attempt in range(3):
        try:
            res = run_bass_kernel_spmd(nc, in_maps, core_ids=list(range(NCORES)))
            return np.asarray(
                res.results[0]["out"], dtype=np.float32
            ).reshape(())[()]
        except Exception as e:  # transient device-unrecoverable happens
            last = e
            time.sleep(30 * (attempt + 1))
    raise last


# revision 28
# speedup vs baseline: 1.6166x; 1.0182x over previous
"""Trainium2 Bass kernel for AlignmentContrastiveLoss (8 NeuronCores, SPMD).

Reference semantics:
  im = im_set[:, 1:, :]           [256, 36, 1024]
  s  = s_seq[:, 1:-2, :]          [256, 32, 1024]
  align[i,j,n,m] = im[i,n] . s[j,m], zeroed where n >= im_len[i]-1 or m >= s_len[j]-3
  scores[i,j] = sum_m max_n align[i,j,n,m]
  loss = sum_i relu(M + max_{j!=i} scores[i,j] - scores[i,i])
       + sum_j relu(M + max_{i!=j} scores[i,j] - scores[j,j])

Sharding: data-parallel over images (32 per core), s replicated.

v2 design:
  - f32 tokens are cast once to fp8e4 (im-mask fused as activation scale),
    bitcast to 16-bit fp8-pairs and transposed via the DMA XBAR (pure bit
    movement) into a packed layout: partition p of q-chunk q holds the d
    pair (256q+2p, 256q+2p+1) interleaved per token (HW-verified mapping).
  - s feeds the PE as RAW packed bytes via MatmulPerfMode.DoubleRowSwInterleave
    (stationary side accepts interleaved pairs; output partitions come out
    token-REVERSED, verified on HW). im (small) is deinterleaved to planar
    [128, q, 2, tok] fp8 by one gpsimd 4D copy per tile, which also compacts
    away the XBAR pad columns. No PE transposes, no PSUM->SBUF staging copies.
  - The s-token reversal is compensated in the word-sum weights (w4 built
    from a reversed word mask), so scoresT and the loss tail are unchanged.
  - wt-outer loop: per s-tile, 3 region-chunk matmul groups + vector MAX
    reduce; word-sum is a tiny PE matmul against s-mask-weighted block-ones
    (applies the s word mask for free and directly yields scoresT[img,sent]).
  - im staged as 11 tiles of 3 images (112-partition windows, 4-col overlap
    into the next tile's range which is later overwritten with real data).
  - Cross-core traffic: one 520-float AllGather of per-core column-max
    partials + scattered diagonals + local cost_s sum.
"""

import numpy as np

MARGIN = 0.2
B = 256          # global batch (images == sentences)
NCORES = 8
BI = B // NCORES  # images per core = 32
NREG = 36        # regions per image after stripping
NWORD = 32       # words per sentence after stripping
D = 1024
IM_TOK = BI * NREG      # 1152 dense im tokens
S_TOK = B * NWORD       # 8192 s tokens
S_TILES = S_TOK // 128  # 64
BIG = 1.0e30
# region chunks for the main matmul: (token offset, ntok, nimg)
RCHUNKS = [(0, 432, 12), (432, 432, 12), (864, 288, 8)]
# im staging tiles: (dense token offset, window (mult of 16), first image, n images)
IM_STAGE = [(108 * t, 112, 3 * t, 3) for t in range(10)] + [(1080, 80, 30, 2)]
IM_TP_COLS = 112 * 11  # padded XBAR destination: disjoint 112-col windows
BLK = 520  # allgather block floats: 256 colmax | 256 diag-scatter | 1 cost_s | pad


def fix_multiwaits(nc, mybir):
    """This toolchain's walrus accepts 1 wait per instruction (2 for
    EventSemaphore); Tile can emit more. Offload surplus waits onto
    inserted same-engine NoOps placed immediately before the instruction."""
    n_fix = 0
    for fn in nc.m.functions:
        for blk in fn.blocks:
            insts = blk.instructions
            i = 0
            while i < len(insts):
                inst = insts[i]
                si = inst.sync_info
                waits = list(si.on_wait) if si is not None and si.on_wait else []
                cap = 2 if isinstance(inst, mybir.InstEventSemaphore) else 1
                if len(waits) > cap:
                    surplus, keep = waits[:-cap], waits[-cap:]
                    si.on_wait = keep
                    for w in surplus:
                        nop = mybir.InstNoOp(
                            name=f"{inst.name}_wsplit{n_fix}",
                            engine=inst.engine,
                            ins=[],
                            outs=[],
                            sync_info=mybir.SyncInfo(on_wait=[w], on_update=[]),
                        )
                        insts.insert(i, nop)
                        n_fix += 1
                        i += 1
                i += 1
    return n_fix


DEFAULT_OPTS = {
    "sf_bufs": 16,     # f32 staging tiles
    "pk_bufs": 5,      # packed fp8-as-bf16 staging quad tiles
    "alp_bufs": 6,     # PSUM align buffers
    "gpsimd_cast": 0,  # every Nth s cast on vector (0 = all scalar)
    "s_prefetch": 3,   # s quad-loads dispatched ahead of their pack stage
    "im_head": 4,      # im tiles staged before the first s tile
}


def build_graph(opts=None):
    import concourse.bass as bass
    import concourse.mybir as mybir
    import concourse.tile as tile
    from concourse.masks import make_identity
    from contextlib import ExitStack

    opts = {**DEFAULT_OPTS, **(opts or {})}

    f32 = mybir.dt.float32
    bf16 = mybir.dt.bfloat16
    fp8 = mybir.dt.float8e4
    i32 = mybir.dt.int32
    ALU = mybir.AluOpType
    AX = mybir.AxisListType
    ACTF = mybir.ActivationFunctionType
    SWI = mybir.MatmulPerfMode.DoubleRowSwInterleave

    nc = bass.Bass()

    im_ext = nc.declare_dram_parameter("im_set", [BI, 37, D], f32, isOutput=False)
    s_ext = nc.declare_dram_parameter("s_seq", [B, 35, D], f32, isOutput=False)
    imlen_ext = nc.declare_dram_parameter("im_len", [BI], i32, isOutput=False)
    slen_ext = nc.declare_dram_parameter("s_len", [B], i32, isOutput=False)
    dmask_ext = nc.declare_dram_parameter("diag_mask", [B, BI], f32, isOutput=False)
    dmaskT_ext = nc.declare_dram_parameter("diag_maskT", [BI, B], f32, isOutput=False)
    out_ext = nc.declare_dram_parameter("out", [1], f32, isOutput=True)
    if opts.get("debug"):
        dbg_sel = nc.declare_dram_parameter("dbg_sel", [128, 128], f32, isOutput=True)
        dbg_pm = nc.declare_dram_parameter("dbg_pm", [128, 128], f32, isOutput=True)
        dbg_rb = nc.declare_dram_parameter("dbg_rb", [128, 128], f32, isOutput=True)
        dbg_tc = nc.declare_dram_parameter("dbg_tc", [128, 2], f32, isOutput=True)
        dbg_bm = nc.declare_dram_parameter("dbg_bm", [2, 128, 128], f32, isOutput=True)
        dbg_w4 = nc.declare_dram_parameter("dbg_w4", [128, S_TILES, 4], f32, isOutput=True)
        dbg_scT = nc.declare_dram_parameter("dbg_scT", [BI, S_TILES, 4], f32, isOutput=True)
        dbg_mx = nc.declare_dram_parameter("dbg_mx", [128, S_TILES, BI], f32, isOutput=True)

    with tile.TileContext(nc) as tc, ExitStack() as top:
        # ---------------- constants ----------------
        const = top.enter_context(tc.tile_pool(name="const", bufs=1))
        ident_f32 = const.tile([128, 128], f32)
        make_identity(nc, ident_f32)
        ones32 = const.tile([32, 1], f32)
        nc.gpsimd.memset(ones32, 1.0)
        ones128 = const.tile([128, 1], f32)
        nc.gpsimd.memset(ones128, 1.0)

        # ---------------- token masks (device-side) ----------------
        mpool = top.enter_context(tc.tile_pool(name="masks", bufs=1))
        dram = top.enter_context(tc.tile_pool(name="dram", bufs=1, space="DRAM"))

        # per-image region mask [BI, NREG]: n < im_len-1
        imlen_sb = mpool.tile([BI, 1], i32)
        nc.sync.dma_start(imlen_sb, imlen_ext.rearrange("(p o) -> p o", o=1))
        il_sb = mpool.tile([BI, 1], f32)
        nc.gpsimd.tensor_scalar(il_sb, imlen_sb, 1, None, op0=ALU.subtract)
        iota_r = mpool.tile([BI, NREG], f32)
        nc.gpsimd.iota(
            iota_r, pattern=[[1, NREG]], base=0, channel_multiplier=0,
            allow_small_or_imprecise_dtypes=True,
        )
        mask_im = mpool.tile([BI, NREG], f32)
        nc.gpsimd.tensor_scalar(mask_im, iota_r, il_sb, None, op0=ALU.is_lt)
        # maskcol_im [128, 11]: per (partition, im stage tile) in REGION-major
        # stage order (p = ni*n + i); pad rows -> 0
        mask_im_dram = dram.tile([BI, NREG], f32)
        nc.sync.dma_start(mask_im_dram[:, :], mask_im)
        maskcol_im = mpool.tile([128, len(IM_STAGE)], f32)
        nc.gpsimd.memset(maskcol_im, 0.0)
        for t, (toff, win, i0, ni) in enumerate(IM_STAGE):
            nc.sync.dma_start(
                maskcol_im[0:36 * ni, t:t + 1],
                mask_im_dram[i0:i0 + ni, :].rearrange("i n -> n i"),
            )

        # s word masks -> w4 block-ones weights [128, 64, 4] bf16:
        #   w4[32*jj + m, wt, jj] = (m < s_len[4*wt+jj] - 3)
        slen_sb = mpool.tile([128, 2], i32)
        nc.sync.dma_start(slen_sb, slen_ext.rearrange("(t p) -> p t", p=128))
        sl_sb = mpool.tile([128, 2], f32)
        nc.gpsimd.tensor_scalar(sl_sb, slen_sb, 3, None, op0=ALU.subtract)
        iota_w = mpool.tile([128, NWORD], f32)
        nc.gpsimd.iota(
            iota_w, pattern=[[1, NWORD]], base=0, channel_multiplier=0,
            allow_small_or_imprecise_dtypes=True,
        )
        # Word-sum weights for WORD-major stage order (p = 4w + j) combined
        # with the SwInterleave token reversal (partition p <-> raw col 127-p):
        #   w4[p, wt, jj] = [ (127-p)%4 == jj ] * ( (127-p)//4 < sl[4*wt+jj] )
        # Built transposed (partition c = sentence-within-half, free p) then
        # PE-transposed into place.
        rb = mpool.tile([128, 128], f32)     # rb[c, p] = (127-p)//4
        nc.gpsimd.iota(rb, pattern=[[-1, 32], [0, 4]], base=31,
                       channel_multiplier=0, allow_small_or_imprecise_dtypes=True)
        # sel[c, p] = (p%4 == 3 - c%4)  <=>  ((c + p + 1) & 3 == 0)
        cp_i = mpool.tile([128, 128], i32)
        nc.gpsimd.iota(cp_i, pattern=[[1, 128]], base=1, channel_multiplier=1)
        cp_a = mpool.tile([128, 128], i32)
        nc.vector.tensor_scalar(cp_a, cp_i, 3, None, op0=ALU.bitwise_and)
        sel = mpool.tile([128, 128], f32)
        nc.vector.tensor_scalar(sel, cp_a, 0, None, op0=ALU.is_equal)
        w4 = mpool.tile([128, S_TILES, 4], bf16)
        with tc.tile_pool(name="w4ps", bufs=2, space="PSUM") as wps:
            for h in range(2):
                bh = mpool.tile([128, 128], f32, tag=f"w4bh{h}")
                nc.vector.tensor_scalar(
                    bh, rb, sl_sb[:, h:h + 1], None, op0=ALU.is_lt
                )
                bm = mpool.tile([128, 128], f32, tag=f"w4bm{h}")
                nc.vector.tensor_mul(bm, bh, sel)
                wt_ps = wps.tile([128, 128], f32, tag=f"w4t{h}")
                nc.tensor.transpose(wt_ps, bm, ident_f32)
                nc.vector.tensor_copy(
                    w4[:, 32 * h:32 * (h + 1), :].rearrange("p a b -> p (a b)"),
                    wt_ps,
                )
                if opts.get("debug"):
                    nc.sync.dma_start(dbg_bm[h, :, :], bm)
        if opts.get("debug"):
            nc.sync.dma_start(dbg_sel[:, :], sel)
            nc.sync.dma_start(dbg_rb[:, :], rb)

        # diag masks (sharding metadata inputs)
        dmask_sb = mpool.tile([128, 2, BI], f32)
        nc.sync.dma_start(dmask_sb, dmask_ext.rearrange("(t p) i -> p t i", p=128))
        dmaskT_sb = mpool.tile([BI, 2, 128], f32)
        nc.sync.dma_start(dmaskT_sb, dmaskT_ext.rearrange("p (t f) -> p t f", f=128))

        # ---------------- persistent big buffers ----------------
        big = top.enter_context(tc.tile_pool(name="big", bufs=1))
        # packed-transposed fp8 pairs, stored as bf16 bit containers.
        # One tile per staging unit so the (whole-tile) dependency tracking
        # of the XBAR DMA writes stays exactly per-tile.
        IMQ = [[0, 1, 2, 3], [4, 5, 6, 7], [8, 9], [10]]
        imTq_g = [
            big.tile([128, 4 * len(ts), 112], bf16, name=f"imTq{g}")
            for g, ts in enumerate(IMQ)
        ]
        imP = big.tile([128, 4, 2, IM_TOK], fp8)      # dense planar im
        sTp_g = [
            big.tile([128, 4, 4, 128], bf16, name=f"sTpg{g}")
            for g in range(S_TILES // 4)
        ]
        maxima = big.tile([128, S_TILES, BI], bf16)  # per (word, wtile, img) region-max
        scoresT_sb = big.tile([BI, S_TILES, 4], f32)  # [img, wt, jj] == [img, sent]
        scores_sb = big.tile([128, 2, BI], f32)       # [sent%128, sent//128, img]

        # fp8 views: im pair-split for the deinterleave, s raw for SwInterleave
        imTq8_g = [
            t.bitcast(fp8).rearrange("p m (t b) -> p m b t", b=2) for t in imTq_g
        ]
        sTraw_g = [t.bitcast(fp8) for t in sTp_g]     # each [128, 4, 4, 256]

        with ExitStack() as mid:
            stage = mid.enter_context(
                tc.tile_pool(name="stage", bufs=opts["sf_bufs"])
            )
            pk = mid.enter_context(tc.tile_pool(name="pk", bufs=opts["pk_bufs"]))
            alp = mid.enter_context(
                tc.tile_pool(name="alp", bufs=opts["alp_bufs"], space="PSUM")
            )
            scp = mid.enter_context(tc.tile_pool(name="scp", bufs=1, space="PSUM"))
            scoresT_ps = scp.tile([BI, S_TILES, 4], f32)

            def stage_im_quad(g):
                ts = IMQ[g]
                win_g = IM_STAGE[ts[-1]][1]
                tf32s = []
                for t in ts:
                    toff, win, i0, ni = IM_STAGE[t]
                    nreal = 36 * ni
                    tf32 = stage.tile([128, D], f32, tag="sf32")
                    nc.sync.dma_start(
                        tf32[0:nreal, :],
                        im_ext[i0:i0 + ni, 1:1 + NREG, :].rearrange(
                            "i n d -> n i d"
                        ),
                    )
                    tf32s.append(tf32)
                ipkq = pk.tile([128, len(ts), D // 2], bf16, tag="spk")
                for k, t in enumerate(ts):
                    toff, win, i0, ni = IM_STAGE[t]
                    # masked cast on the (ramp-idle) vector engine
                    nc.vector.tensor_scalar(
                        ipkq[:, k, :].bitcast(fp8)[0:win, :], tf32s[k][0:win, :],
                        maskcol_im[0:win, t:t + 1], None, op0=ALU.mult,
                    )
                # one XBAR DMA for the whole quad (scalar queue: keeps the
                # sync queue free to prefetch s loads without blocking)
                nc.scalar.dma_start(
                    imTq_g[g][:, :, 0:win_g],
                    ipkq[0:win_g, :, :].rearrange("p a c -> p (a c)"),
                    transpose=True,
                )
                # deinterleave + compact + un-permute (region-major stage
                # order ni*n + i -> dense 36*i + n), one strided copy per tile
                for k, t in enumerate(ts):
                    toff, win, i0, ni = IM_STAGE[t]
                    nreal = 36 * ni
                    nc.gpsimd.tensor_copy(
                        imP[:, :, :, toff:toff + nreal].rearrange(
                            "p q b (i n) -> p q b i n", n=NREG
                        ),
                        imTq8_g[g][:, 4 * k:4 * k + 4, :, 0:nreal].rearrange(
                            "p q b (n i) -> p q b n i", i=ni
                        ).rearrange("p q b n i -> p q b i n"),
                    )

            def s_load(gq):
                # four per-tile loads (the DMA AP balancer caps at 3 dims,
                # so a quad can't be one DMA), word-major per tile
                tfs = []
                for a in range(4):
                    i = 4 * gq + a
                    tf32 = stage.tile([128, D], f32, tag="sf32")
                    nc.sync.dma_start(
                        tf32,
                        s_ext[4 * i:4 * i + 4, 1:1 + NWORD, :].rearrange(
                            "j w d -> w j d"
                        ),
                    )
                    tfs.append(tf32)
                return tfs

            def s_pack(gq, tfs):
                pkq = pk.tile([128, 4, D // 2], bf16, tag="spk")
                gsp = opts["gpsimd_cast"]
                for a in range(4):
                    i = 4 * gq + a
                    if gsp and (i % gsp == gsp - 1):
                        nc.vector.tensor_copy(
                            pkq[:, a, :].bitcast(fp8), tfs[a]
                        )
                    else:
                        nc.scalar.activation(
                            pkq[:, a, :].bitcast(fp8), tfs[a], ACTF.Copy
                        )
                nc.sync.dma_start(
                    sTp_g[gq].rearrange("p a q t -> p (a q) t"),
                    pkq.rearrange("p a c -> p (a c)"),
                    transpose=True,
                )

            # im head first (needed by rc0); s quad-loads run ahead of their
            # cast+transpose stages so no dispatch queue ever blocks
            PRE = opts["s_prefetch"]
            for g in range(len(IMQ)):
                stage_im_quad(g)
            pending = []
            for gq in range(S_TILES // 4):
                pending.append((gq, s_load(gq)))
                if len(pending) > PRE:
                    s_pack(*pending.pop(0))
            for it in pending:
                s_pack(*it)

            # ---------------- main matmul + region-max + word-sum ----------------
            def word_sum(wt):
                # scoresT[img, 4wt+jj] = sum_m maxima[(jj,m), wt, img] * wmask
                nc.tensor.matmul(
                    scoresT_ps[:, wt, :],
                    lhsT=maxima[:, wt, :],
                    rhs=w4[:, wt, :],
                    start=True, stop=True,
                )

            for wt in range(S_TILES):
                for rci, (toff, ntok, nimg) in enumerate(RCHUNKS):
                    pal = alp.tile([128, nimg, NREG], f32, tag="align")
                    for q in range(4):
                        nc.tensor.matmul(
                            pal.rearrange("p a b -> p (a b)"),
                            lhsT=sTraw_g[wt // 4][:, wt % 4, q, :],
                            rhs=imP[:, q, :, toff:toff + ntok],
                            start=(q == 0),
                            stop=(q == 3),
                            perf_mode=SWI,
                        )
                    nc.vector.tensor_reduce(
                        maxima[:, wt, toff // NREG:toff // NREG + nimg],
                        pal, axis=AX.X, op=ALU.max,
                    )
                    # emit the previous tile's word-sum between rc chunks so the
                    # PE never waits on the vector MAX of the current tile
                    if rci == 0 and wt > 0:
                        word_sum(wt - 1)
            word_sum(S_TILES - 1)

            # scoresT -> SBUF, then transpose back to [sent, img]
            nc.vector.tensor_copy(scoresT_sb, scoresT_ps)
            if opts.get("debug"):
                w4d = mpool.tile([128, S_TILES, 4], f32)
                nc.vector.tensor_copy(w4d, w4)
                nc.sync.dma_start(dbg_w4[:, :, :], w4d)
                nc.sync.dma_start(dbg_scT[:, :, :], scoresT_sb)
                mxd = mpool.tile([128, S_TILES, BI], f32)
                nc.vector.tensor_copy(mxd, maxima)
                nc.sync.dma_start(dbg_mx[:, :, :], mxd)
            sc_ps = scp.tile([128, 2, BI], f32)
            for t in range(2):
                nc.tensor.transpose(
                    sc_ps[:, t, :],
                    scoresT_sb[:, 32 * t:32 * (t + 1), :].rearrange(
                        "p a b -> p (a b)"
                    ),
                    ident_f32[:BI, :BI],
                )
                nc.vector.tensor_copy(scores_sb[:, t, :], sc_ps[:, t, :])

        # ---------------- loss tail ----------------
        with ExitStack() as tail:
            tp = tail.enter_context(tc.tile_pool(name="tailp", bufs=1, space="PSUM"))
            ts = tail.enter_context(tc.tile_pool(name="tails", bufs=1))

            # col-max over local images (diag excluded) + scattered diag
            masked = ts.tile([128, 2, BI], f32)
            nc.vector.scalar_tensor_tensor(
                masked, dmask_sb, -BIG, scores_sb, op0=ALU.mult, op1=ALU.add
            )
            colmax_p = ts.tile([128, 2], f32)
            nc.vector.tensor_reduce(colmax_p, masked, axis=AX.X, op=ALU.max)
            dtmp = ts.tile([128, 2, BI], f32)
            nc.vector.tensor_mul(dtmp, dmask_sb, scores_sb)
            dscat = ts.tile([128, 2], f32)
            nc.vector.tensor_reduce(dscat, dtmp, axis=AX.X, op=ALU.add)

            # row-max over sentences (diag excluded); scoresT_sb is [img, sent]
            scT_flat = scoresT_sb.rearrange("p a b -> p (a b)")
            dmaskT_flat = dmaskT_sb.rearrange("p a b -> p (a b)")
            maskedT = ts.tile([BI, B], f32)
            nc.vector.scalar_tensor_tensor(
                maskedT, dmaskT_flat, -BIG, scT_flat, op0=ALU.mult, op1=ALU.add
            )
            rowmax = ts.tile([BI, 1], f32)
            nc.vector.tensor_reduce(rowmax, maskedT, axis=AX.X, op=ALU.max)
            dT_tmp = ts.tile([BI, B], f32)
            nc.vector.tensor_mul(dT_tmp, dmaskT_flat, scT_flat)
            d_row = ts.tile([BI, 1], f32)
            nc.vector.tensor_reduce(d_row, dT_tmp, axis=AX.X, op=ALU.add)

            cost_s = ts.tile([BI, 1], f32)
            nc.vector.tensor_sub(cost_s, rowmax, d_row)
            nc.vector.tensor_scalar(
                cost_s, cost_s, MARGIN, 0.0, op0=ALU.add, op1=ALU.max
            )
            cs_ps = tp.tile([1, 1], f32)
            nc.tensor.matmul(cs_ps, lhsT=ones32, rhs=cost_s, start=True, stop=True)
            cs_sb = ts.tile([1, 8], f32)
            nc.gpsimd.memset(cs_sb, 0.0)
            nc.vector.tensor_copy(cs_sb[:, 0:1], cs_ps)

            # pack allgather block: [0:256) colmax | [256:512) dscat | 512 cost_s
            blk = dram.tile([BLK], f32)
            nc.sync.dma_start(
                blk[0:256].rearrange("(t p) -> p t", p=128), colmax_p
            )
            nc.sync.dma_start(
                blk[256:512].rearrange("(t p) -> p t", p=128), dscat
            )
            nc.sync.dma_start(blk[512:520], cs_sb[0, :])
            gath = dram.tile([NCORES, BLK], f32, addr_space="Shared")
            nc.gpsimd.collective_compute(
                "AllGather",
                ALU.bypass,
                ins=[blk.opt()],
                outs=[gath.opt()],
                replica_groups=[list(range(NCORES))],
            )

            # redundant final reduction on every core
            g_cm = ts.tile([128, 2, NCORES], f32)
            g_d = ts.tile([128, 2, NCORES], f32)
            for t in range(2):
                nc.sync.dma_start(
                    g_cm[:, t, :],
                    gath[:, 128 * t:128 * (t + 1)].rearrange("c p -> p c"),
                )
                nc.sync.dma_start(
                    g_d[:, t, :],
                    gath[:, 256 + 128 * t:256 + 128 * (t + 1)].rearrange("c p -> p c"),
                )
            g_cs = ts.tile([1, NCORES], f32)
            nc.sync.dma_start(g_cs, gath[:, 512:513].rearrange("a b -> b a"))

            colmax_g = ts.tile([128, 2], f32)
            nc.vector.tensor_reduce(colmax_g, g_cm, axis=AX.X, op=ALU.max)
            d_all = ts.tile([128, 2], f32)
            nc.vector.tensor_reduce(d_all, g_d, axis=AX.X, op=ALU.add)
            cim = ts.tile([128, 2], f32)
            nc.vector.tensor_sub(cim, colmax_g, d_all)
            nc.vector.tensor_scalar(cim, cim, MARGIN, 0.0, op0=ALU.add, op1=ALU.max)
            cim_r = ts.tile([128, 1], f32)
            nc.vector.tensor_reduce(cim_r, cim, axis=AX.X, op=ALU.add)
            tot_ps = tp.tile([1, 1], f32)
            nc.tensor.matmul(tot_ps, lhsT=ones128, rhs=cim_r, start=True, stop=True)
            cs_tot = ts.tile([1, 1], f32)
            nc.vector.tensor_reduce(cs_tot, g_cs, axis=AX.X, op=ALU.add)
            total = ts.tile([1, 1], f32)
            nc.vector.tensor_add(total, tot_ps, cs_tot)
            nc.sync.dma_start(out_ext[0:1], total[0, :])

    fix_multiwaits(nc, mybir)
    return nc


_CACHE = {}


def _get_nc():
    if "nc" not in _CACHE:
        _CACHE["nc"] = build_graph()
    return _CACHE["nc"]


def make_in_maps(im_set, s_seq, im_len, s_len):
    im_set = np.ascontiguousarray(im_set, dtype=np.float32)
    s_seq = np.ascontiguousarray(s_seq, dtype=np.float32)
    im_len = np.ascontiguousarray(im_len, dtype=np.int32)
    s_len = np.ascontiguousarray(s_len, dtype=np.int32)
    in_maps = []
    for c in range(NCORES):
        dm = np.zeros((B, BI), dtype=np.float32)
        for i in range(BI):
            dm[BI * c + i, i] = 1.0
        in_maps.append({
            "im_set": im_set[BI * c:BI * (c + 1)],
            "s_seq": s_seq,
            "im_len": im_len[BI * c:BI * (c + 1)],
            "s_len": s_len,
            "diag_mask": dm,
            "diag_maskT": np.ascontiguousarray(dm.T),
        })
    return in_maps


def kernel(im_set, s_seq, im_len, s_len):
    import time
    from concourse.bass_utils import run_bass_kernel_spmd

    nc = _get_nc()
    in_maps = make_in_maps(im_set, s_seq, im_len, s_len)
    last = None
    for attempt in range(3):
        try:
            res = run_bass_kernel_spmd(nc, in_maps, core_ids=list(range(NCORES)))
            return np.asarray(
                res.results[0]["out"], dtype=np.float32
            ).reshape(())[()]
        except Exception as e:  # transient device-unrecoverable happens
            last = e
            time.sleep(30 * (attempt + 1))
    raise last


# revision 35
# speedup vs baseline: 1.6364x; 1.0122x over previous
"""Trainium2 Bass kernel for AlignmentContrastiveLoss (8 NeuronCores, SPMD).

Reference semantics:
  im = im_set[:, 1:, :]           [256, 36, 1024]
  s  = s_seq[:, 1:-2, :]          [256, 32, 1024]
  align[i,j,n,m] = im[i,n] . s[j,m], zeroed where n >= im_len[i]-1 or m >= s_len[j]-3
  scores[i,j] = sum_m max_n align[i,j,n,m]
  loss = sum_i relu(M + max_{j!=i} scores[i,j] - scores[i,i])
       + sum_j relu(M + max_{i!=j} scores[i,j] - scores[j,j])

Sharding: data-parallel over images (32 per core), s replicated.

v2 design:
  - f32 tokens are cast once to fp8e4 (im-mask fused as activation scale),
    bitcast to 16-bit fp8-pairs and transposed via the DMA XBAR (pure bit
    movement) into a packed layout: partition p of q-chunk q holds the d
    pair (256q+2p, 256q+2p+1) interleaved per token (HW-verified mapping).
  - s feeds the PE as RAW packed bytes via MatmulPerfMode.DoubleRowSwInterleave
    (stationary side accepts interleaved pairs; output partitions come out
    token-REVERSED, verified on HW). im (small) is deinterleaved to planar
    [128, q, 2, tok] fp8 by one gpsimd 4D copy per tile, which also compacts
    away the XBAR pad columns. No PE transposes, no PSUM->SBUF staging copies.
  - The s-token reversal is compensated in the word-sum weights (w4 built
    from a reversed word mask), so scoresT and the loss tail are unchanged.
  - wt-outer loop: per s-tile, 3 region-chunk matmul groups + vector MAX
    reduce; word-sum is a tiny PE matmul against s-mask-weighted block-ones
    (applies the s word mask for free and directly yields scoresT[img,sent]).
  - im staged as 11 tiles of 3 images (112-partition windows, 4-col overlap
    into the next tile's range which is later overwritten with real data).
  - Cross-core traffic: one 520-float AllGather of per-core column-max
    partials + scattered diagonals + local cost_s sum.
"""

import numpy as np

MARGIN = 0.2
B = 256          # global batch (images == sentences)
NCORES = 8
BI = B // NCORES  # images per core = 32
NREG = 36        # regions per image after stripping
NWORD = 32       # words per sentence after stripping
D = 1024
IM_TOK = BI * NREG      # 1152 dense im tokens
S_TOK = B * NWORD       # 8192 s tokens
S_TILES = S_TOK // 128  # 64
BIG = 1.0e30
# region chunks for the main matmul: (token offset, ntok, nimg)
RCHUNKS = [(0, 432, 12), (432, 432, 12), (864, 288, 8)]
# im staging tiles: (dense token offset, window (mult of 16), first image, n images)
IM_STAGE = [(108 * t, 112, 3 * t, 3) for t in range(10)] + [(1080, 80, 30, 2)]
IM_TP_COLS = 112 * 11  # padded XBAR destination: disjoint 112-col windows
BLK = 640  # allgather block floats: [p-major x5] colmax x2 | dscat x2 | cost_s


def fix_multiwaits(nc, mybir):
    """This toolchain's walrus accepts 1 wait per instruction (2 for
    EventSemaphore); Tile can emit more. Offload surplus waits onto
    inserted same-engine NoOps placed immediately before the instruction."""
    n_fix = 0
    for fn in nc.m.functions:
        for blk in fn.blocks:
            insts = blk.instructions
            i = 0
            while i < len(insts):
                inst = insts[i]
                si = inst.sync_info
                waits = list(si.on_wait) if si is not None and si.on_wait else []
                cap = 2 if isinstance(inst, mybir.InstEventSemaphore) else 1
                if len(waits) > cap:
                    surplus, keep = waits[:-cap], waits[-cap:]
                    si.on_wait = keep
                    for w in surplus:
                        nop = mybir.InstNoOp(
                            name=f"{inst.name}_wsplit{n_fix}",
                            engine=inst.engine,
                            ins=[],
                            outs=[],
                            sync_info=mybir.SyncInfo(on_wait=[w], on_update=[]),
                        )
                        insts.insert(i, nop)
                        n_fix += 1
                        i += 1
                i += 1
    return n_fix


DEFAULT_OPTS = {
    "sf_bufs": 16,     # f32 staging tiles
    "pk_bufs": 5,      # packed fp8-as-bf16 staging quad tiles
    "alp_bufs": 6,     # PSUM align buffers
    "gpsimd_cast": 0,  # every Nth s cast on vector (0 = all scalar)
    "s_prefetch": 3,   # s quad-loads dispatched ahead of their pack stage
    "im_head": 4,      # im tiles staged before the first s tile
}


def build_graph(opts=None):
    import concourse.bass as bass
    import concourse.mybir as mybir
    import concourse.tile as tile
    from concourse.masks import make_identity
    from contextlib import ExitStack

    opts = {**DEFAULT_OPTS, **(opts or {})}

    f32 = mybir.dt.float32
    bf16 = mybir.dt.bfloat16
    fp8 = mybir.dt.float8e4
    i32 = mybir.dt.int32
    ALU = mybir.AluOpType
    AX = mybir.AxisListType
    ACTF = mybir.ActivationFunctionType
    SWI = mybir.MatmulPerfMode.DoubleRowSwInterleave

    nc = bass.Bass()

    im_ext = nc.declare_dram_parameter("im_set", [BI, 37, D], f32, isOutput=False)
    s_ext = nc.declare_dram_parameter("s_seq", [B, 35, D], f32, isOutput=False)
    imlen_ext = nc.declare_dram_parameter("im_len", [BI], i32, isOutput=False)
    slen_ext = nc.declare_dram_parameter("s_len", [B], i32, isOutput=False)
    dmask_ext = nc.declare_dram_parameter("diag_mask", [B, BI], f32, isOutput=False)
    dmaskT_ext = nc.declare_dram_parameter("diag_maskT", [BI, B], f32, isOutput=False)
    out_ext = nc.declare_dram_parameter("out", [1], f32, isOutput=True)
    if opts.get("debug"):
        dbg_sel = nc.declare_dram_parameter("dbg_sel", [128, 128], f32, isOutput=True)
        dbg_pm = nc.declare_dram_parameter("dbg_pm", [128, 128], f32, isOutput=True)
        dbg_rb = nc.declare_dram_parameter("dbg_rb", [128, 128], f32, isOutput=True)
        dbg_tc = nc.declare_dram_parameter("dbg_tc", [128, 2], f32, isOutput=True)
        dbg_bm = nc.declare_dram_parameter("dbg_bm", [2, 128, 128], f32, isOutput=True)
        dbg_w4 = nc.declare_dram_parameter("dbg_w4", [128, S_TILES, 4], f32, isOutput=True)
        dbg_scT = nc.declare_dram_parameter("dbg_scT", [BI, S_TILES, 4], f32, isOutput=True)
        dbg_mx = nc.declare_dram_parameter("dbg_mx", [128, S_TILES, BI], f32, isOutput=True)

    with tile.TileContext(nc) as tc, ExitStack() as top:
        # ---------------- constants ----------------
        const = top.enter_context(tc.tile_pool(name="const", bufs=1))
        ident_f32 = const.tile([128, 128], f32)
        make_identity(nc, ident_f32)
        ones32 = const.tile([32, 1], f32)
        nc.gpsimd.memset(ones32, 1.0)
        ones128 = const.tile([128, 1], f32)
        nc.gpsimd.memset(ones128, 1.0)

        # ---------------- token masks (device-side) ----------------
        mpool = top.enter_context(tc.tile_pool(name="masks", bufs=1))
        dram = top.enter_context(tc.tile_pool(name="dram", bufs=1, space="DRAM"))

        # per-image region mask [BI, NREG]: n < im_len-1
        imlen_sb = mpool.tile([BI, 1], i32)
        nc.gpsimd.dma_start(imlen_sb, imlen_ext.rearrange("(p o) -> p o", o=1))
        il_sb = mpool.tile([BI, 1], f32)
        nc.gpsimd.tensor_scalar(il_sb, imlen_sb, 1, None, op0=ALU.subtract)
        iota_r = mpool.tile([BI, NREG], f32)
        nc.gpsimd.iota(
            iota_r, pattern=[[1, NREG]], base=0, channel_multiplier=0,
            allow_small_or_imprecise_dtypes=True,
        )
        mask_im = mpool.tile([BI, NREG], f32)
        nc.gpsimd.tensor_scalar(mask_im, iota_r, il_sb, None, op0=ALU.is_lt)
        # maskcol_im [128, 11]: per (partition, im stage tile) in REGION-major
        # stage order (p = ni*n + i); pad rows -> 0
        mask_im_dram = dram.tile([BI * NREG], f32)
        nc.gpsimd.dma_start(
            mask_im_dram.rearrange("(i n) -> i n", n=NREG), mask_im
        )
        maskcol_im = mpool.tile([128, len(IM_STAGE)], f32)
        nc.gpsimd.memset(maskcol_im, 0.0)
        for t, (toff, win, i0, ni) in enumerate(IM_STAGE):
            nc.gpsimd.dma_start(
                maskcol_im[0:36 * ni, t:t + 1],
                mask_im_dram[36 * i0:36 * (i0 + ni)].rearrange(
                    "(i n) -> n i", n=NREG
                ),
            )

        # s word masks -> w4 block-ones weights [128, 64, 4] bf16:
        #   w4[32*jj + m, wt, jj] = (m < s_len[4*wt+jj] - 3)
        slen_sb = mpool.tile([128, 2], i32)
        nc.gpsimd.dma_start(slen_sb, slen_ext.rearrange("(t p) -> p t", p=128))
        sl_sb = mpool.tile([128, 2], f32)
        nc.gpsimd.tensor_scalar(sl_sb, slen_sb, 3, None, op0=ALU.subtract)
        iota_w = mpool.tile([128, NWORD], f32)
        nc.gpsimd.iota(
            iota_w, pattern=[[1, NWORD]], base=0, channel_multiplier=0,
            allow_small_or_imprecise_dtypes=True,
        )
        # Word-sum weights for WORD-major stage order (p = 4w + j) combined
        # with the SwInterleave token reversal (partition p <-> raw col 127-p):
        #   w4[p, wt, jj] = [ (127-p)%4 == jj ] * ( (127-p)//4 < sl[4*wt+jj] )
        # Built transposed (partition c = sentence-within-half, free p) then
        # PE-transposed into place.
        rb = mpool.tile([128, 128], f32)     # rb[c, p] = (127-p)//4
        nc.gpsimd.iota(rb, pattern=[[-1, 32], [0, 4]], base=31,
                       channel_multiplier=0, allow_small_or_imprecise_dtypes=True)
        # sel[c, p] = (p%4 == 3 - c%4)  <=>  ((c + p + 1) & 3 == 0)
        cp_i = mpool.tile([128, 128], i32)
        nc.gpsimd.iota(cp_i, pattern=[[1, 128]], base=1, channel_multiplier=1)
        cp_a = mpool.tile([128, 128], i32)
        nc.vector.tensor_scalar(cp_a, cp_i, 3, None, op0=ALU.bitwise_and)
        sel = mpool.tile([128, 128], f32)
        nc.vector.tensor_scalar(sel, cp_a, 0, None, op0=ALU.is_equal)
        w4 = mpool.tile([128, S_TILES, 4], bf16)
        with tc.tile_pool(name="w4ps", bufs=2, space="PSUM") as wps:
            for h in range(2):
                bh = mpool.tile([128, 128], f32, tag=f"w4bh{h}")
                nc.vector.tensor_scalar(
                    bh, rb, sl_sb[:, h:h + 1], None, op0=ALU.is_lt
                )
                bm = mpool.tile([128, 128], f32, tag=f"w4bm{h}")
                nc.vector.tensor_mul(bm, bh, sel)
                wt_ps = wps.tile([128, 128], f32, tag=f"w4t{h}")
                nc.tensor.transpose(wt_ps, bm, ident_f32)
                nc.vector.tensor_copy(
                    w4[:, 32 * h:32 * (h + 1), :].rearrange("p a b -> p (a b)"),
                    wt_ps,
                )
                if opts.get("debug"):
                    nc.sync.dma_start(dbg_bm[h, :, :], bm)
        if opts.get("debug"):
            nc.sync.dma_start(dbg_sel[:, :], sel)
            nc.sync.dma_start(dbg_rb[:, :], rb)

        # diag masks (sharding metadata inputs)
        dmask_sb = mpool.tile([128, 2, BI], f32)
        nc.gpsimd.dma_start(dmask_sb, dmask_ext.rearrange("(t p) i -> p t i", p=128))
        dmaskT_sb = mpool.tile([BI, 2, 128], f32)
        nc.gpsimd.dma_start(
            dmaskT_sb, dmaskT_ext.rearrange("p (t f) -> p t f", f=128)
        )

        # ---------------- persistent big buffers ----------------
        big = top.enter_context(tc.tile_pool(name="big", bufs=1))
        # packed-transposed fp8 pairs, stored as bf16 bit containers.
        # One tile per staging unit so the (whole-tile) dependency tracking
        # of the XBAR DMA writes stays exactly per-tile.
        IMQ = [[0, 1, 2, 3], [4, 5, 6, 7], [8, 9], [10]]
        imTq_g = [
            big.tile([128, 4 * len(ts), 112], bf16, name=f"imTq{g}")
            for g, ts in enumerate(IMQ)
        ]
        imP = big.tile([128, 4, 2, IM_TOK], fp8)      # dense planar im
        sTp_g = [
            big.tile([128, 4, 4, 128], bf16, name=f"sTpg{g}")
            for g in range(S_TILES // 4)
        ]
        maxima = big.tile([128, S_TILES, BI], bf16)  # per (word, wtile, img) region-max
        scoresT_sb = big.tile([BI, S_TILES, 4], f32)  # [img, wt, jj] == [img, sent]
        scores_sb = big.tile([128, 2, BI], f32)       # [sent%128, sent//128, img]

        # fp8 views: im pair-split for the deinterleave, s raw for SwInterleave
        imTq8_g = [
            t.bitcast(fp8).rearrange("p m (t b) -> p m b t", b=2) for t in imTq_g
        ]
        sTraw_g = [t.bitcast(fp8) for t in sTp_g]     # each [128, 4, 4, 256]

        with ExitStack() as mid:
            stage = mid.enter_context(
                tc.tile_pool(name="stage", bufs=opts["sf_bufs"])
            )
            pk = mid.enter_context(tc.tile_pool(name="pk", bufs=opts["pk_bufs"]))
            alp = mid.enter_context(
                tc.tile_pool(name="alp", bufs=opts["alp_bufs"], space="PSUM")
            )
            scp = mid.enter_context(tc.tile_pool(name="scp", bufs=1, space="PSUM"))
            scoresT_ps = scp.tile([BI, S_TILES, 4], f32)

            def stage_im_quad(g):
                ts = IMQ[g]
                win_g = IM_STAGE[ts[-1]][1]
                tf32s = []
                for t in ts:
                    toff, win, i0, ni = IM_STAGE[t]
                    nreal = 36 * ni
                    tf32 = stage.tile([128, D], f32, tag="sf32")
                    nc.sync.dma_start(
                        tf32[0:nreal, :],
                        im_ext[i0:i0 + ni, 1:1 + NREG, :].rearrange(
                            "i n d -> n i d"
                        ),
                    )
                    tf32s.append(tf32)
                ipkq = pk.tile([128, len(ts), D // 2], bf16, tag="spk")
                for k, t in enumerate(ts):
                    toff, win, i0, ni = IM_STAGE[t]
                    # masked cast on the (ramp-idle) vector engine
                    nc.vector.tensor_scalar(
                        ipkq[:, k, :].bitcast(fp8)[0:win, :], tf32s[k][0:win, :],
                        maskcol_im[0:win, t:t + 1], None, op0=ALU.mult,
                    )
                # one XBAR DMA for the whole quad (scalar queue: keeps the
                # sync queue free to prefetch s loads without blocking)
                nc.scalar.dma_start(
                    imTq_g[g][:, :, 0:win_g],
                    ipkq[0:win_g, :, :].rearrange("p a c -> p (a c)"),
                    transpose=True,
                )
                # deinterleave + compact + un-permute (region-major stage
                # order ni*n + i -> dense 36*i + n), one strided copy per tile
                for k, t in enumerate(ts):
                    toff, win, i0, ni = IM_STAGE[t]
                    nreal = 36 * ni
                    nc.gpsimd.tensor_copy(
                        imP[:, :, :, toff:toff + nreal].rearrange(
                            "p q b (i n) -> p q b i n", n=NREG
                        ),
                        imTq8_g[g][:, 4 * k:4 * k + 4, :, 0:nreal].rearrange(
                            "p q b (n i) -> p q b n i", i=ni
                        ).rearrange("p q b n i -> p q b i n"),
                    )

            def s_load(gq):
                # four per-tile loads (the DMA AP balancer caps at 3 dims,
                # so a quad can't be one DMA), word-major per tile
                tfs = []
                for a in range(4):
                    i = 4 * gq + a
                    tf32 = stage.tile([128, D], f32, tag="sf32")
                    nc.sync.dma_start(
                        tf32,
                        s_ext[4 * i:4 * i + 4, 1:1 + NWORD, :].rearrange(
                            "j w d -> w j d"
                        ),
                    )
                    tfs.append(tf32)
                return tfs

            def s_pack(gq, tfs):
                pkq = pk.tile([128, 4, D // 2], bf16, tag="spk")
                gsp = opts["gpsimd_cast"]
                for a in range(4):
                    i = 4 * gq + a
                    if gsp and (i % gsp == gsp - 1):
                        nc.vector.tensor_copy(
                            pkq[:, a, :].bitcast(fp8), tfs[a]
                        )
                    else:
                        nc.scalar.activation(
                            pkq[:, a, :].bitcast(fp8), tfs[a], ACTF.Copy
                        )
                nc.sync.dma_start(
                    sTp_g[gq].rearrange("p a q t -> p (a q) t"),
                    pkq.rearrange("p a c -> p (a c)"),
                    transpose=True,
                )

            # im head first (needed by rc0); s quad-loads run ahead of their
            # cast+transpose stages so no dispatch queue ever blocks
            PRE = opts["s_prefetch"]
            for g in range(len(IMQ)):
                stage_im_quad(g)
            pending = []
            for gq in range(S_TILES // 4):
                pending.append((gq, s_load(gq)))
                if len(pending) > PRE:
                    s_pack(*pending.pop(0))
            for it in pending:
                s_pack(*it)

            # ---------------- main matmul + region-max + word-sum ----------------
            def word_sum(wt):
                # scoresT[img, 4wt+jj] = sum_m maxima[(jj,m), wt, img] * wmask
                nc.tensor.matmul(
                    scoresT_ps[:, wt, :],
                    lhsT=maxima[:, wt, :],
                    rhs=w4[:, wt, :],
                    start=True, stop=True,
                )

            for wt in range(S_TILES):
                for rci, (toff, ntok, nimg) in enumerate(RCHUNKS):
                    pal = alp.tile([128, nimg, NREG], f32, tag="align")
                    for q in range(4):
                        nc.tensor.matmul(
                            pal.rearrange("p a b -> p (a b)"),
                            lhsT=sTraw_g[wt // 4][:, wt % 4, q, :],
                            rhs=imP[:, q, :, toff:toff + ntok],
                            start=(q == 0),
                            stop=(q == 3),
                            perf_mode=SWI,
                        )
                    nc.vector.tensor_reduce(
                        maxima[:, wt, toff // NREG:toff // NREG + nimg],
                        pal, axis=AX.X, op=ALU.max,
                    )
                    # emit the previous tile's word-sum between rc chunks so the
                    # PE never waits on the vector MAX of the current tile
                    if rci == 0 and wt > 0:
                        word_sum(wt - 1)
            word_sum(S_TILES - 1)

            # scoresT -> SBUF, then transpose back to [sent, img]
            nc.vector.tensor_copy(scoresT_sb, scoresT_ps)
            if opts.get("debug"):
                w4d = mpool.tile([128, S_TILES, 4], f32)
                nc.vector.tensor_copy(w4d, w4)
                nc.sync.dma_start(dbg_w4[:, :, :], w4d)
                nc.sync.dma_start(dbg_scT[:, :, :], scoresT_sb)
                mxd = mpool.tile([128, S_TILES, BI], f32)
                nc.vector.tensor_copy(mxd, maxima)
                nc.sync.dma_start(dbg_mx[:, :, :], mxd)
            sc_ps = scp.tile([128, 2, BI], f32)
            for t in range(2):
                nc.tensor.transpose(
                    sc_ps[:, t, :],
                    scoresT_sb[:, 32 * t:32 * (t + 1), :].rearrange(
                        "p a b -> p (a b)"
                    ),
                    ident_f32[:BI, :BI],
                )
                nc.vector.tensor_copy(scores_sb[:, t, :], sc_ps[:, t, :])

        # ---------------- loss tail ----------------
        with ExitStack() as tail:
            tp = tail.enter_context(tc.tile_pool(name="tailp", bufs=1, space="PSUM"))
            ts = tail.enter_context(tc.tile_pool(name="tails", bufs=1))

            # col-max over local images (diag excluded) + scattered diag
            masked = ts.tile([128, 2, BI], f32)
            nc.vector.scalar_tensor_tensor(
                masked, dmask_sb, -BIG, scores_sb, op0=ALU.mult, op1=ALU.add
            )
            colmax_p = ts.tile([128, 2], f32)
            nc.vector.tensor_reduce(colmax_p, masked, axis=AX.X, op=ALU.max)
            dtmp = ts.tile([128, 2, BI], f32)
            nc.vector.tensor_mul(dtmp, dmask_sb, scores_sb)
            dscat = ts.tile([128, 2], f32)
            nc.vector.tensor_reduce(dscat, dtmp, axis=AX.X, op=ALU.add)

            # row-max over sentences (diag excluded); scoresT_sb is [img, sent]
            scT_flat = scoresT_sb.rearrange("p a b -> p (a b)")
            dmaskT_flat = dmaskT_sb.rearrange("p a b -> p (a b)")
            maskedT = ts.tile([BI, B], f32)
            nc.vector.scalar_tensor_tensor(
                maskedT, dmaskT_flat, -BIG, scT_flat, op0=ALU.mult, op1=ALU.add
            )
            rowmax = ts.tile([BI, 1], f32)
            nc.vector.tensor_reduce(rowmax, maskedT, axis=AX.X, op=ALU.max)
            dT_tmp = ts.tile([BI, B], f32)
            nc.vector.tensor_mul(dT_tmp, dmaskT_flat, scT_flat)
            d_row = ts.tile([BI, 1], f32)
            nc.vector.tensor_reduce(d_row, dT_tmp, axis=AX.X, op=ALU.add)

            cost_s = ts.tile([BI, 1], f32)
            nc.vector.tensor_sub(cost_s, rowmax, d_row)
            nc.vector.tensor_scalar(
                cost_s, cost_s, MARGIN, 0.0, op0=ALU.add, op1=ALU.max
            )
            cs_ps = tp.tile([1, 1], f32)
            nc.tensor.matmul(cs_ps, lhsT=ones32, rhs=cost_s, start=True, stop=True)

            # one packed tile: [:,0:2]=colmax | [:,2:4]=dscat | [0,4]=cost_s
            pkt = ts.tile([128, 5], f32)
            nc.vector.tensor_copy(pkt[:, 0:2], colmax_p)
            nc.vector.tensor_copy(pkt[:, 2:4], dscat)
            nc.vector.tensor_copy(pkt[0:1, 4:5], cs_ps)
            blk = dram.tile([BLK], f32)
            nc.sync.dma_start(
                blk[0:640].rearrange("(a p) -> p a", p=128), pkt
            )
            gath = dram.tile([NCORES, BLK], f32, addr_space="Shared")
            nc.gpsimd.collective_compute(
                "AllGather",
                ALU.bypass,
                ins=[blk.opt()],
                outs=[gath.opt()],
                replica_groups=[list(range(NCORES))],
            )

            # redundant final reduction on every core; single unpack DMA
            # per-piece transposing unpacks (DMA APs cap at 3 dims with a
            # contiguous final dim), spread across both dispatch queues
            g5 = ts.tile([128, 5, NCORES], f32)
            for a in range(5):
                eng = nc.sync if a % 2 == 0 else nc.scalar
                eng.dma_start(
                    g5[:, a, :],
                    gath[:, 128 * a:128 * (a + 1)].rearrange("c p -> p c"),
                )
            colmax_g = ts.tile([128, 2], f32)
            nc.vector.tensor_reduce(colmax_g, g5[:, 0:2, :], axis=AX.X, op=ALU.max)
            d_all = ts.tile([128, 2], f32)
            nc.vector.tensor_reduce(d_all, g5[:, 2:4, :], axis=AX.X, op=ALU.add)
            cim = ts.tile([128, 2], f32)
            nc.vector.tensor_sub(cim, colmax_g, d_all)
            nc.vector.tensor_scalar(cim, cim, MARGIN, 0.0, op0=ALU.add, op1=ALU.max)
            cim_r = ts.tile([128, 1], f32)
            nc.vector.tensor_reduce(cim_r, cim, axis=AX.X, op=ALU.add)
            cs_tot = ts.tile([1, 1], f32)
            nc.vector.tensor_reduce(cs_tot, g5[0:1, 4, :], axis=AX.X, op=ALU.add)
            nc.vector.tensor_add(cim_r[0:1, :], cim_r[0:1, :], cs_tot)
            tot_ps = tp.tile([1, 1], f32)
            nc.tensor.matmul(tot_ps, lhsT=ones128, rhs=cim_r, start=True, stop=True)
            total = ts.tile([1, 1], f32)
            nc.vector.tensor_copy(total, tot_ps)
            nc.sync.dma_start(out_ext[0:1], total[0, :])

    fix_multiwaits(nc, mybir)
    return nc


_CACHE = {}


def _get_nc():
    if "nc" not in _CACHE:
        _CACHE["nc"] = build_graph()
    return _CACHE["nc"]


def make_in_maps(im_set, s_seq, im_len, s_len):
    im_set = np.ascontiguousarray(im_set, dtype=np.float32)
    s_seq = np.ascontiguousarray(s_seq, dtype=np.float32)
    im_len = np.ascontiguousarray(im_len, dtype=np.int32)
    s_len = np.ascontiguousarray(s_len, dtype=np.int32)
    in_maps = []
    for c in range(NCORES):
        dm = np.zeros((B, BI), dtype=np.float32)
        for i in range(BI):
            dm[BI * c + i, i] = 1.0
        in_maps.append({
            "im_set": im_set[BI * c:BI * (c + 1)],
            "s_seq": s_seq,
            "im_len": im_len[BI * c:BI * (c + 1)],
            "s_len": s_len,
            "diag_mask": dm,
            "diag_maskT": np.ascontiguousarray(dm.T),
        })
    return in_maps


def kernel(im_set, s_seq, im_len, s_len):
    import time
    from concourse.bass_utils import run_bass_kernel_spmd

    nc = _get_nc()
    in_maps = make_in_maps(im_set, s_seq, im_len, s_len)
    last = None
    for attempt in range(3):
        try:
            res = run_bass_kernel_spmd(nc, in_maps, core_ids=list(range(NCORES)))
            return np.asarray(
                res.results[0]["out"], dtype=np.float32
            ).reshape(())[()]
        except Exception as e:  # transient device-unrecoverable happens
            last = e
            time.sleep(30 * (attempt + 1))
    raise last


# revision 36
# speedup vs baseline: 1.6557x; 1.0118x over previous
"""Trainium2 Bass kernel for AlignmentContrastiveLoss (8 NeuronCores, SPMD).

Reference semantics:
  im = im_set[:, 1:, :]           [256, 36, 1024]
  s  = s_seq[:, 1:-2, :]          [256, 32, 1024]
  align[i,j,n,m] = im[i,n] . s[j,m], zeroed where n >= im_len[i]-1 or m >= s_len[j]-3
  scores[i,j] = sum_m max_n align[i,j,n,m]
  loss = sum_i relu(M + max_{j!=i} scores[i,j] - scores[i,i])
       + sum_j relu(M + max_{i!=j} scores[i,j] - scores[j,j])

Sharding: data-parallel over images (32 per core), s replicated.

v2 design:
  - f32 tokens are cast once to fp8e4 (im-mask fused as activation scale),
    bitcast to 16-bit fp8-pairs and transposed via the DMA XBAR (pure bit
    movement) into a packed layout: partition p of q-chunk q holds the d
    pair (256q+2p, 256q+2p+1) interleaved per token (HW-verified mapping).
  - s feeds the PE as RAW packed bytes via MatmulPerfMode.DoubleRowSwInterleave
    (stationary side accepts interleaved pairs; output partitions come out
    token-REVERSED, verified on HW). im (small) is deinterleaved to planar
    [128, q, 2, tok] fp8 by one gpsimd 4D copy per tile, which also compacts
    away the XBAR pad columns. No PE transposes, no PSUM->SBUF staging copies.
  - The s-token reversal is compensated in the word-sum weights (w4 built
    from a reversed word mask), so scoresT and the loss tail are unchanged.
  - wt-outer loop: per s-tile, 3 region-chunk matmul groups + vector MAX
    reduce; word-sum is a tiny PE matmul against s-mask-weighted block-ones
    (applies the s word mask for free and directly yields scoresT[img,sent]).
  - im staged as 11 tiles of 3 images (112-partition windows, 4-col overlap
    into the next tile's range which is later overwritten with real data).
  - Cross-core traffic: one 520-float AllGather of per-core column-max
    partials + scattered diagonals + local cost_s sum.
"""

import numpy as np

MARGIN = 0.2
B = 256          # global batch (images == sentences)
NCORES = 8
BI = B // NCORES  # images per core = 32
NREG = 36        # regions per image after stripping
NWORD = 32       # words per sentence after stripping
D = 1024
IM_TOK = BI * NREG      # 1152 dense im tokens
S_TOK = B * NWORD       # 8192 s tokens
S_TILES = S_TOK // 128  # 64
BIG = 1.0e30
# region chunks for the main matmul: (token offset, ntok, nimg)
RCHUNKS = [(0, 432, 12), (432, 432, 12), (864, 288, 8)]
# im staging tiles: (dense token offset, window (mult of 16), first image, n images)
IM_STAGE = [(108 * t, 112, 3 * t, 3) for t in range(10)] + [(1080, 112, 30, 2)]
IM_TP_COLS = 112 * 11  # padded XBAR destination: disjoint 112-col windows
BLK = 640  # allgather block floats: [p-major x5] colmax x2 | dscat x2 | cost_s


def fix_multiwaits(nc, mybir):
    """This toolchain's walrus accepts 1 wait per instruction (2 for
    EventSemaphore); Tile can emit more. Offload surplus waits onto
    inserted same-engine NoOps placed immediately before the instruction."""
    n_fix = 0
    for fn in nc.m.functions:
        for blk in fn.blocks:
            insts = blk.instructions
            i = 0
            while i < len(insts):
                inst = insts[i]
                si = inst.sync_info
                waits = list(si.on_wait) if si is not None and si.on_wait else []
                cap = 2 if isinstance(inst, mybir.InstEventSemaphore) else 1
                if len(waits) > cap:
                    surplus, keep = waits[:-cap], waits[-cap:]
                    si.on_wait = keep
                    for w in surplus:
                        nop = mybir.InstNoOp(
                            name=f"{inst.name}_wsplit{n_fix}",
                            engine=inst.engine,
                            ins=[],
                            outs=[],
                            sync_info=mybir.SyncInfo(on_wait=[w], on_update=[]),
                        )
                        insts.insert(i, nop)
                        n_fix += 1
                        i += 1
                i += 1
    return n_fix


DEFAULT_OPTS = {
    "sf_bufs": 16,     # f32 staging tiles
    "pk_bufs": 5,      # packed fp8-as-bf16 staging quad tiles
    "alp_bufs": 6,     # PSUM align buffers
    "gpsimd_cast": 0,  # every Nth s cast on vector (0 = all scalar)
    "s_prefetch": 3,   # s quad-loads dispatched ahead of their pack stage
    "im_head": 4,      # im tiles staged before the first s tile
}


def build_graph(opts=None):
    import concourse.bass as bass
    import concourse.mybir as mybir
    import concourse.tile as tile
    from concourse.masks import make_identity
    from contextlib import ExitStack

    opts = {**DEFAULT_OPTS, **(opts or {})}

    f32 = mybir.dt.float32
    bf16 = mybir.dt.bfloat16
    fp8 = mybir.dt.float8e4
    i32 = mybir.dt.int32
    ALU = mybir.AluOpType
    AX = mybir.AxisListType
    ACTF = mybir.ActivationFunctionType
    SWI = mybir.MatmulPerfMode.DoubleRowSwInterleave

    nc = bass.Bass()

    im_ext = nc.declare_dram_parameter("im_set", [BI, 37, D], f32, isOutput=False)
    s_ext = nc.declare_dram_parameter("s_seq", [B, 35, D], f32, isOutput=False)
    imlen_ext = nc.declare_dram_parameter("im_len", [BI], i32, isOutput=False)
    slen_ext = nc.declare_dram_parameter("s_len", [B], i32, isOutput=False)
    dmask_ext = nc.declare_dram_parameter("diag_mask", [B, BI], f32, isOutput=False)
    dmaskT_ext = nc.declare_dram_parameter("diag_maskT", [BI, B], f32, isOutput=False)
    out_ext = nc.declare_dram_parameter("out", [1], f32, isOutput=True)
    if opts.get("debug"):
        dbg_sel = nc.declare_dram_parameter("dbg_sel", [128, 128], f32, isOutput=True)
        dbg_pm = nc.declare_dram_parameter("dbg_pm", [128, 128], f32, isOutput=True)
        dbg_rb = nc.declare_dram_parameter("dbg_rb", [128, 128], f32, isOutput=True)
        dbg_tc = nc.declare_dram_parameter("dbg_tc", [128, 2], f32, isOutput=True)
        dbg_bm = nc.declare_dram_parameter("dbg_bm", [2, 128, 128], f32, isOutput=True)
        dbg_w4 = nc.declare_dram_parameter("dbg_w4", [128, S_TILES, 4], f32, isOutput=True)
        dbg_scT = nc.declare_dram_parameter("dbg_scT", [BI, S_TILES, 4], f32, isOutput=True)
        dbg_mx = nc.declare_dram_parameter("dbg_mx", [128, S_TILES, BI], f32, isOutput=True)

    with tile.TileContext(nc) as tc, ExitStack() as top:
        # ---------------- constants ----------------
        const = top.enter_context(tc.tile_pool(name="const", bufs=1))
        ident_f32 = const.tile([128, 128], f32)
        make_identity(nc, ident_f32)
        ones32 = const.tile([32, 1], f32)
        nc.gpsimd.memset(ones32, 1.0)
        ones128 = const.tile([128, 1], f32)
        nc.gpsimd.memset(ones128, 1.0)

        # ---------------- token masks (device-side) ----------------
        mpool = top.enter_context(tc.tile_pool(name="masks", bufs=1))
        dram = top.enter_context(tc.tile_pool(name="dram", bufs=1, space="DRAM"))

        # per-image region mask [BI, NREG]: n < im_len-1
        imlen_sb = mpool.tile([BI, 1], i32)
        nc.gpsimd.dma_start(imlen_sb, imlen_ext.rearrange("(p o) -> p o", o=1))
        il_sb = mpool.tile([BI, 1], f32)
        nc.gpsimd.tensor_scalar(il_sb, imlen_sb, 1, None, op0=ALU.subtract)
        iota_r = mpool.tile([BI, NREG], f32)
        nc.gpsimd.iota(
            iota_r, pattern=[[1, NREG]], base=0, channel_multiplier=0,
            allow_small_or_imprecise_dtypes=True,
        )
        mask_im = mpool.tile([BI, NREG], f32)
        nc.gpsimd.tensor_scalar(mask_im, iota_r, il_sb, None, op0=ALU.is_lt)
        # maskcol_im [128, 11]: per (partition, im stage tile) in REGION-major
        # stage order (p = ni*n + i); pad rows -> 0
        mask_im_dram = dram.tile([BI * NREG], f32)
        nc.gpsimd.dma_start(
            mask_im_dram.rearrange("(i n) -> i n", n=NREG), mask_im
        )
        maskcol_im = mpool.tile([128, len(IM_STAGE)], f32)
        nc.gpsimd.memset(maskcol_im, 0.0)
        for t, (toff, win, i0, ni) in enumerate(IM_STAGE):
            nc.gpsimd.dma_start(
                maskcol_im[0:36 * ni, t:t + 1],
                mask_im_dram[36 * i0:36 * (i0 + ni)].rearrange(
                    "(i n) -> n i", n=NREG
                ),
            )

        # s word masks -> w4 block-ones weights [128, 64, 4] bf16:
        #   w4[32*jj + m, wt, jj] = (m < s_len[4*wt+jj] - 3)
        slen_sb = mpool.tile([128, 2], i32)
        nc.gpsimd.dma_start(slen_sb, slen_ext.rearrange("(t p) -> p t", p=128))
        sl_sb = mpool.tile([128, 2], f32)
        nc.gpsimd.tensor_scalar(sl_sb, slen_sb, 3, None, op0=ALU.subtract)
        iota_w = mpool.tile([128, NWORD], f32)
        nc.gpsimd.iota(
            iota_w, pattern=[[1, NWORD]], base=0, channel_multiplier=0,
            allow_small_or_imprecise_dtypes=True,
        )
        # Word-sum weights for WORD-major stage order (p = 4w + j) combined
        # with the SwInterleave token reversal (partition p <-> raw col 127-p):
        #   w4[p, wt, jj] = [ (127-p)%4 == jj ] * ( (127-p)//4 < sl[4*wt+jj] )
        # Built transposed (partition c = sentence-within-half, free p) then
        # PE-transposed into place.
        rb = mpool.tile([128, 128], f32)     # rb[c, p] = (127-p)//4
        nc.gpsimd.iota(rb, pattern=[[-1, 32], [0, 4]], base=31,
                       channel_multiplier=0, allow_small_or_imprecise_dtypes=True)
        # sel[c, p] = (p%4 == 3 - c%4)  <=>  ((c + p + 1) & 3 == 0)
        cp_i = mpool.tile([128, 128], i32)
        nc.gpsimd.iota(cp_i, pattern=[[1, 128]], base=1, channel_multiplier=1)
        cp_a = mpool.tile([128, 128], i32)
        nc.vector.tensor_scalar(cp_a, cp_i, 3, None, op0=ALU.bitwise_and)
        sel = mpool.tile([128, 128], f32)
        nc.vector.tensor_scalar(sel, cp_a, 0, None, op0=ALU.is_equal)
        w4 = mpool.tile([128, S_TILES, 4], bf16)
        with tc.tile_pool(name="w4ps", bufs=2, space="PSUM") as wps:
            for h in range(2):
                bh = mpool.tile([128, 128], f32, tag=f"w4bh{h}")
                nc.vector.tensor_scalar(
                    bh, rb, sl_sb[:, h:h + 1], None, op0=ALU.is_lt
                )
                bm = mpool.tile([128, 128], f32, tag=f"w4bm{h}")
                nc.vector.tensor_mul(bm, bh, sel)
                wt_ps = wps.tile([128, 128], f32, tag=f"w4t{h}")
                nc.tensor.transpose(wt_ps, bm, ident_f32)
                nc.vector.tensor_copy(
                    w4[:, 32 * h:32 * (h + 1), :].rearrange("p a b -> p (a b)"),
                    wt_ps,
                )
                if opts.get("debug"):
                    nc.sync.dma_start(dbg_bm[h, :, :], bm)
        if opts.get("debug"):
            nc.sync.dma_start(dbg_sel[:, :], sel)
            nc.sync.dma_start(dbg_rb[:, :], rb)

        # diag masks (sharding metadata inputs)
        dmask_sb = mpool.tile([128, 2, BI], f32)
        nc.gpsimd.dma_start(dmask_sb, dmask_ext.rearrange("(t p) i -> p t i", p=128))
        dmaskT_sb = mpool.tile([BI, 2, 128], f32)
        nc.gpsimd.dma_start(
            dmaskT_sb, dmaskT_ext.rearrange("p (t f) -> p t f", f=128)
        )

        # ---------------- persistent big buffers ----------------
        big = top.enter_context(tc.tile_pool(name="big", bufs=1))
        # packed-transposed fp8 pairs, stored as bf16 bit containers.
        # One tile per staging unit so the (whole-tile) dependency tracking
        # of the XBAR DMA writes stays exactly per-tile.
        IMQ = [[0, 1, 2, 3], [4, 5, 6, 7], [8, 9], [10]]
        imTq_g = [
            big.tile([128, 4 * len(ts), 112], bf16, name=f"imTq{g}")
            for g, ts in enumerate(IMQ)
        ]
        imP = big.tile([128, 4, 2, IM_TOK], fp8)      # dense planar im
        sTp_g = [
            big.tile([128, 4, 4, 128], bf16, name=f"sTpg{g}")
            for g in range(S_TILES // 4)
        ]
        maxima = big.tile([128, S_TILES, BI], bf16)  # per (word, wtile, img) region-max
        scoresT_sb = big.tile([BI, S_TILES, 4], f32)  # [img, wt, jj] == [img, sent]
        scores_sb = big.tile([128, 2, BI], f32)       # [sent%128, sent//128, img]

        # fp8 views: im pair-split for the deinterleave, s raw for SwInterleave
        imTq8_g = [
            t.bitcast(fp8).rearrange("p m (t b) -> p m b t", b=2) for t in imTq_g
        ]
        sTraw_g = [t.bitcast(fp8) for t in sTp_g]     # each [128, 4, 4, 256]

        with ExitStack() as mid:
            stage = mid.enter_context(
                tc.tile_pool(name="stage", bufs=opts["sf_bufs"])
            )
            pk = mid.enter_context(tc.tile_pool(name="pk", bufs=opts["pk_bufs"]))
            alp = mid.enter_context(
                tc.tile_pool(name="alp", bufs=opts["alp_bufs"], space="PSUM")
            )
            scp = mid.enter_context(tc.tile_pool(name="scp", bufs=1, space="PSUM"))
            scoresT_ps = scp.tile([BI, S_TILES, 4], f32)

            def stage_im_quad(g):
                ts = IMQ[g]
                win_g = IM_STAGE[ts[-1]][1]
                tf32s = []
                for t in ts:
                    toff, win, i0, ni = IM_STAGE[t]
                    nreal = 36 * ni
                    tf32 = stage.tile([128, D], f32, tag="sf32")
                    nc.sync.dma_start(
                        tf32[0:nreal, :],
                        im_ext[i0:i0 + ni, 1:1 + NREG, :].rearrange(
                            "i n d -> n i d"
                        ),
                    )
                    tf32s.append(tf32)
                ipkq = pk.tile([128, len(ts), D // 2], bf16, tag="spk")
                for k, t in enumerate(ts):
                    toff, win, i0, ni = IM_STAGE[t]
                    # masked cast on the (ramp-idle) vector engine
                    nc.vector.tensor_scalar(
                        ipkq[:, k, :].bitcast(fp8)[0:win, :], tf32s[k][0:win, :],
                        maskcol_im[0:win, t:t + 1], None, op0=ALU.mult,
                    )
                # one XBAR DMA for the whole quad (scalar queue: keeps the
                # sync queue free to prefetch s loads without blocking)
                nc.scalar.dma_start(
                    imTq_g[g][:, :, 0:win_g],
                    ipkq[0:win_g, :, :].rearrange("p a c -> p (a c)"),
                    transpose=True,
                )
            def im_deints(g):
                # deinterleave + compact + un-permute (region-major stage
                # order ni*n + i -> dense 36*i + n), one strided copy per tile
                for k, t in enumerate(IMQ[g]):
                    toff, win, i0, ni = IM_STAGE[t]
                    nreal = 36 * ni
                    nc.vector.tensor_copy(
                        imP[:, :, :, toff:toff + nreal].rearrange(
                            "p q b (i n) -> p q b i n", n=NREG
                        ),
                        imTq8_g[g][:, 4 * k:4 * k + 4, :, 0:nreal].rearrange(
                            "p q b (n i) -> p q b n i", i=ni
                        ).rearrange("p q b n i -> p q b i n"),
                    )

            def s_load(gq):
                # four per-tile loads (the DMA AP balancer caps at 3 dims,
                # so a quad can't be one DMA), word-major per tile
                tfs = []
                for a in range(4):
                    i = 4 * gq + a
                    tf32 = stage.tile([128, D], f32, tag="sf32")
                    nc.sync.dma_start(
                        tf32,
                        s_ext[4 * i:4 * i + 4, 1:1 + NWORD, :].rearrange(
                            "j w d -> w j d"
                        ),
                    )
                    tfs.append(tf32)
                return tfs

            def s_pack(gq, tfs):
                pkq = pk.tile([128, 4, D // 2], bf16, tag="spk")
                gsp = opts["gpsimd_cast"]
                for a in range(4):
                    i = 4 * gq + a
                    if gsp and (i % gsp == gsp - 1):
                        nc.vector.tensor_copy(
                            pkq[:, a, :].bitcast(fp8), tfs[a]
                        )
                    else:
                        nc.scalar.activation(
                            pkq[:, a, :].bitcast(fp8), tfs[a], ACTF.Copy
                        )
                nc.sync.dma_start(
                    sTp_g[gq].rearrange("p a q t -> p (a q) t"),
                    pkq.rearrange("p a c -> p (a c)"),
                    transpose=True,
                )

            # im head first (needed by rc0); s quad-loads run ahead of their
            # cast+transpose stages so no dispatch queue ever blocks
            PRE = opts["s_prefetch"]
            for g in range(len(IMQ)):
                stage_im_quad(g)
            for g in range(len(IMQ)):
                im_deints(g)
            pending = []
            for gq in range(S_TILES // 4):
                pending.append((gq, s_load(gq)))
                if len(pending) > PRE:
                    s_pack(*pending.pop(0))
            for it in pending:
                s_pack(*it)

            # ---------------- main matmul + region-max + word-sum ----------------
            def word_sum(wt):
                # scoresT[img, 4wt+jj] = sum_m maxima[(jj,m), wt, img] * wmask
                nc.tensor.matmul(
                    scoresT_ps[:, wt, :],
                    lhsT=maxima[:, wt, :],
                    rhs=w4[:, wt, :],
                    start=True, stop=True,
                )

            for wt in range(S_TILES):
                for rci, (toff, ntok, nimg) in enumerate(RCHUNKS):
                    pal = alp.tile([128, nimg, NREG], f32, tag="align")
                    for q in range(4):
                        nc.tensor.matmul(
                            pal.rearrange("p a b -> p (a b)"),
                            lhsT=sTraw_g[wt // 4][:, wt % 4, q, :],
                            rhs=imP[:, q, :, toff:toff + ntok],
                            start=(q == 0),
                            stop=(q == 3),
                            perf_mode=SWI,
                        )
                    nc.vector.tensor_reduce(
                        maxima[:, wt, toff // NREG:toff // NREG + nimg],
                        pal, axis=AX.X, op=ALU.max,
                    )
                    # emit the previous tile's word-sum between rc chunks so the
                    # PE never waits on the vector MAX of the current tile
                    if rci == 0 and wt > 0:
                        word_sum(wt - 1)
            word_sum(S_TILES - 1)

            # scoresT -> SBUF, then transpose back to [sent, img]
            nc.vector.tensor_copy(scoresT_sb, scoresT_ps)
            if opts.get("debug"):
                w4d = mpool.tile([128, S_TILES, 4], f32)
                nc.vector.tensor_copy(w4d, w4)
                nc.sync.dma_start(dbg_w4[:, :, :], w4d)
                nc.sync.dma_start(dbg_scT[:, :, :], scoresT_sb)
                mxd = mpool.tile([128, S_TILES, BI], f32)
                nc.vector.tensor_copy(mxd, maxima)
                nc.sync.dma_start(dbg_mx[:, :, :], mxd)
            sc_ps = scp.tile([128, 2, BI], f32)
            for t in range(2):
                nc.tensor.transpose(
                    sc_ps[:, t, :],
                    scoresT_sb[:, 32 * t:32 * (t + 1), :].rearrange(
                        "p a b -> p (a b)"
                    ),
                    ident_f32[:BI, :BI],
                )
                nc.vector.tensor_copy(scores_sb[:, t, :], sc_ps[:, t, :])

        # ---------------- loss tail ----------------
        with ExitStack() as tail:
            tp = tail.enter_context(tc.tile_pool(name="tailp", bufs=1, space="PSUM"))
            ts = tail.enter_context(tc.tile_pool(name="tails", bufs=1))

            # col-max over local images (diag excluded) + scattered diag
            masked = ts.tile([128, 2, BI], f32)
            nc.vector.scalar_tensor_tensor(
                masked, dmask_sb, -BIG, scores_sb, op0=ALU.mult, op1=ALU.add
            )
            colmax_p = ts.tile([128, 2], f32)
            nc.vector.tensor_reduce(colmax_p, masked, axis=AX.X, op=ALU.max)
            dtmp = ts.tile([128, 2, BI], f32)
            nc.vector.tensor_mul(dtmp, dmask_sb, scores_sb)
            dscat = ts.tile([128, 2], f32)
            nc.vector.tensor_reduce(dscat, dtmp, axis=AX.X, op=ALU.add)

            # row-max over sentences (diag excluded); scoresT_sb is [img, sent]
            scT_flat = scoresT_sb.rearrange("p a b -> p (a b)")
            dmaskT_flat = dmaskT_sb.rearrange("p a b -> p (a b)")
            maskedT = ts.tile([BI, B], f32)
            nc.vector.scalar_tensor_tensor(
                maskedT, dmaskT_flat, -BIG, scT_flat, op0=ALU.mult, op1=ALU.add
            )
            rowmax = ts.tile([BI, 1], f32)
            nc.vector.tensor_reduce(rowmax, maskedT, axis=AX.X, op=ALU.max)
            dT_tmp = ts.tile([BI, B], f32)
            nc.vector.tensor_mul(dT_tmp, dmaskT_flat, scT_flat)
            d_row = ts.tile([BI, 1], f32)
            nc.vector.tensor_reduce(d_row, dT_tmp, axis=AX.X, op=ALU.add)

            cost_s = ts.tile([BI, 1], f32)
            nc.vector.tensor_sub(cost_s, rowmax, d_row)
            nc.vector.tensor_scalar(
                cost_s, cost_s, MARGIN, 0.0, op0=ALU.add, op1=ALU.max
            )
            cs_ps = tp.tile([1, 1], f32)
            nc.tensor.matmul(cs_ps, lhsT=ones32, rhs=cost_s, start=True, stop=True)

            # one packed tile: [:,0:2]=colmax | [:,2:4]=dscat | [0,4]=cost_s
            pkt = ts.tile([128, 5], f32)
            nc.vector.tensor_copy(pkt[:, 0:2], colmax_p)
            nc.vector.tensor_copy(pkt[:, 2:4], dscat)
            nc.vector.tensor_copy(pkt[0:1, 4:5], cs_ps)
            blk = dram.tile([BLK], f32)
            nc.sync.dma_start(
                blk[0:640].rearrange("(a p) -> p a", p=128), pkt
            )
            gath = dram.tile([NCORES, BLK], f32, addr_space="Shared")
            nc.gpsimd.collective_compute(
                "AllGather",
                ALU.bypass,
                ins=[blk.opt()],
                outs=[gath.opt()],
                replica_groups=[list(range(NCORES))],
            )

            # redundant final reduction on every core; single unpack DMA
            # per-piece transposing unpacks (DMA APs cap at 3 dims with a
            # contiguous final dim), spread across both dispatch queues
            g5 = ts.tile([128, 5, NCORES], f32)
            for a in range(5):
                eng = nc.sync if a % 2 == 0 else nc.scalar
                eng.dma_start(
                    g5[:, a, :],
                    gath[:, 128 * a:128 * (a + 1)].rearrange("c p -> p c"),
                )
            colmax_g = ts.tile([128, 2], f32)
            nc.vector.tensor_reduce(colmax_g, g5[:, 0:2, :], axis=AX.X, op=ALU.max)
            d_all = ts.tile([128, 2], f32)
            nc.vector.tensor_reduce(d_all, g5[:, 2:4, :], axis=AX.X, op=ALU.add)
            cim = ts.tile([128, 2], f32)
            nc.vector.tensor_sub(cim, colmax_g, d_all)
            nc.vector.tensor_scalar(cim, cim, MARGIN, 0.0, op0=ALU.add, op1=ALU.max)
            cim_r = ts.tile([128, 1], f32)
            nc.vector.tensor_reduce(cim_r, cim, axis=AX.X, op=ALU.add)
            cs_tot = ts.tile([1, 1], f32)
            nc.vector.tensor_reduce(cs_tot, g5[0:1, 4, :], axis=AX.X, op=ALU.add)
            nc.vector.tensor_add(cim_r[0:1, :], cim_r[0:1, :], cs_tot)
            tot_ps = tp.tile([1, 1], f32)
            nc.tensor.matmul(tot_ps, lhsT=ones128, rhs=cim_r, start=True, stop=True)
            total = ts.tile([1, 1], f32)
            nc.vector.tensor_copy(total, tot_ps)
            nc.sync.dma_start(out_ext[0:1], total[0, :])

    fix_multiwaits(nc, mybir)
    return nc


_CACHE = {}


def _get_nc():
    if "nc" not in _CACHE:
        _CACHE["nc"] = build_graph()
    return _CACHE["nc"]


def make_in_maps(im_set, s_seq, im_len, s_len):
    im_set = np.ascontiguousarray(im_set, dtype=np.float32)
    s_seq = np.ascontiguousarray(s_seq, dtype=np.float32)
    im_len = np.ascontiguousarray(im_len, dtype=np.int32)
    s_len = np.ascontiguousarray(s_len, dtype=np.int32)
    in_maps = []
    for c in range(NCORES):
        dm = np.zeros((B, BI), dtype=np.float32)
        for i in range(BI):
            dm[BI * c + i, i] = 1.0
        in_maps.append({
            "im_set": im_set[BI * c:BI * (c + 1)],
            "s_seq": s_seq,
            "im_len": im_len[BI * c:BI * (c + 1)],
            "s_len": s_len,
            "diag_mask": dm,
            "diag_maskT": np.ascontiguousarray(dm.T),
        })
    return in_maps


def kernel(im_set, s_seq, im_len, s_len):
    import time
    from concourse.bass_utils import run_bass_kernel_spmd

    nc = _get_nc()
    in_maps = make_in_maps(im_set, s_seq, im_len, s_len)
    last = None
    for attempt in range(3):
        try:
            res = run_bass_kernel_spmd(nc, in_maps, core_ids=list(range(NCORES)))
            return np.asarray(
                res.results[0]["out"], dtype=np.float32
            ).reshape(())[()]
        except Exception as e:  # transient device-unrecoverable happens
            last = e
            time.sleep(30 * (attempt + 1))
    raise last


# revision 39
# speedup vs baseline: 1.6797x; 1.0145x over previous
"""Trainium2 Bass kernel for AlignmentContrastiveLoss (8 NeuronCores, SPMD).

Reference semantics:
  im = im_set[:, 1:, :]           [256, 36, 1024]
  s  = s_seq[:, 1:-2, :]          [256, 32, 1024]
  align[i,j,n,m] = im[i,n] . s[j,m], zeroed where n >= im_len[i]-1 or m >= s_len[j]-3
  scores[i,j] = sum_m max_n align[i,j,n,m]
  loss = sum_i relu(M + max_{j!=i} scores[i,j] - scores[i,i])
       + sum_j relu(M + max_{i!=j} scores[i,j] - scores[j,j])

Sharding: data-parallel over images (32 per core), s replicated.

v2 design:
  - f32 tokens are cast once to fp8e4 (im-mask fused as activation scale),
    bitcast to 16-bit fp8-pairs and transposed via the DMA XBAR (pure bit
    movement) into a packed layout: partition p of q-chunk q holds the d
    pair (256q+2p, 256q+2p+1) interleaved per token (HW-verified mapping).
  - s feeds the PE as RAW packed bytes via MatmulPerfMode.DoubleRowSwInterleave
    (stationary side accepts interleaved pairs; output partitions come out
    token-REVERSED, verified on HW). im (small) is deinterleaved to planar
    [128, q, 2, tok] fp8 by one gpsimd 4D copy per tile, which also compacts
    away the XBAR pad columns. No PE transposes, no PSUM->SBUF staging copies.
  - The s-token reversal is compensated in the word-sum weights (w4 built
    from a reversed word mask), so scoresT and the loss tail are unchanged.
  - wt-outer loop: per s-tile, 3 region-chunk matmul groups + vector MAX
    reduce; word-sum is a tiny PE matmul against s-mask-weighted block-ones
    (applies the s word mask for free and directly yields scoresT[img,sent]).
  - im staged as 11 tiles of 3 images (112-partition windows, 4-col overlap
    into the next tile's range which is later overwritten with real data).
  - Cross-core traffic: one 520-float AllGather of per-core column-max
    partials + scattered diagonals + local cost_s sum.
"""

import numpy as np

MARGIN = 0.2
B = 256          # global batch (images == sentences)
NCORES = 8
BI = B // NCORES  # images per core = 32
NREG = 36        # regions per image after stripping
NWORD = 32       # words per sentence after stripping
D = 1024
IM_TOK = BI * NREG      # 1152 dense im tokens
S_TOK = B * NWORD       # 8192 s tokens
S_TILES = S_TOK // 128  # 64
BIG = 1.0e30
# region chunks for the main matmul: (token offset, ntok, nimg)
RCHUNKS = [(0, 432, 12), (432, 432, 12), (864, 288, 8)]
# im staging tiles: (dense token offset, window (mult of 16), first image, n images)
IM_STAGE = [(108 * t, 112, 3 * t, 3) for t in range(10)] + [(1080, 112, 30, 2)]
IM_TP_COLS = 112 * 11  # padded XBAR destination: disjoint 112-col windows
BLK = 640  # allgather block floats: [p-major x5] colmax x2 | dscat x2 | cost_s


def fix_multiwaits(nc, mybir):
    """This toolchain's walrus accepts 1 wait per instruction (2 for
    EventSemaphore); Tile can emit more. Offload surplus waits onto
    inserted same-engine NoOps placed immediately before the instruction."""
    n_fix = 0
    for fn in nc.m.functions:
        for blk in fn.blocks:
            insts = blk.instructions
            i = 0
            while i < len(insts):
                inst = insts[i]
                si = inst.sync_info
                waits = list(si.on_wait) if si is not None and si.on_wait else []
                cap = 2 if isinstance(inst, mybir.InstEventSemaphore) else 1
                if len(waits) > cap:
                    surplus, keep = waits[:-cap], waits[-cap:]
                    si.on_wait = keep
                    for w in surplus:
                        nop = mybir.InstNoOp(
                            name=f"{inst.name}_wsplit{n_fix}",
                            engine=inst.engine,
                            ins=[],
                            outs=[],
                            sync_info=mybir.SyncInfo(on_wait=[w], on_update=[]),
                        )
                        insts.insert(i, nop)
                        n_fix += 1
                        i += 1
                i += 1
    return n_fix


DEFAULT_OPTS = {
    "sf_bufs": 16,     # f32 staging tiles
    "pk_bufs": 5,      # packed fp8-as-bf16 staging quad tiles
    "alp_bufs": 6,     # PSUM align buffers
    "gpsimd_cast": 0,  # every Nth s cast on vector (0 = all scalar)
    "s_prefetch": 3,   # s quad-loads dispatched ahead of their pack stage
    "im_head": 4,      # im tiles staged before the first s tile
}


def build_graph(opts=None):
    import concourse.bass as bass
    import concourse.mybir as mybir
    import concourse.tile as tile
    from concourse.masks import make_identity
    from contextlib import ExitStack

    opts = {**DEFAULT_OPTS, **(opts or {})}

    f32 = mybir.dt.float32
    bf16 = mybir.dt.bfloat16
    fp8 = mybir.dt.float8e4
    i32 = mybir.dt.int32
    ALU = mybir.AluOpType
    AX = mybir.AxisListType
    ACTF = mybir.ActivationFunctionType
    SWI = mybir.MatmulPerfMode.DoubleRowSwInterleave

    nc = bass.Bass()

    im_ext = nc.declare_dram_parameter("im_set", [BI, 37, D], f32, isOutput=False)
    s_ext = nc.declare_dram_parameter("s_seq", [B, 35, D], f32, isOutput=False)
    imlen_ext = nc.declare_dram_parameter("im_len", [BI], i32, isOutput=False)
    slen_ext = nc.declare_dram_parameter("s_len", [B], i32, isOutput=False)
    dmask_ext = nc.declare_dram_parameter("diag_mask", [B, BI], f32, isOutput=False)
    dmaskT_ext = nc.declare_dram_parameter("diag_maskT", [BI, B], f32, isOutput=False)
    out_ext = nc.declare_dram_parameter("out", [1], f32, isOutput=True)
    if opts.get("debug"):
        dbg_sel = nc.declare_dram_parameter("dbg_sel", [128, 128], f32, isOutput=True)
        dbg_pm = nc.declare_dram_parameter("dbg_pm", [128, 128], f32, isOutput=True)
        dbg_rb = nc.declare_dram_parameter("dbg_rb", [128, 128], f32, isOutput=True)
        dbg_tc = nc.declare_dram_parameter("dbg_tc", [128, 2], f32, isOutput=True)
        dbg_bm = nc.declare_dram_parameter("dbg_bm", [2, 128, 128], f32, isOutput=True)
        dbg_w4 = nc.declare_dram_parameter("dbg_w4", [128, S_TILES, 4], f32, isOutput=True)
        dbg_scT = nc.declare_dram_parameter("dbg_scT", [BI, S_TILES, 4], f32, isOutput=True)
        dbg_mx = nc.declare_dram_parameter("dbg_mx", [128, S_TILES, BI], f32, isOutput=True)

    with tile.TileContext(nc) as tc, ExitStack() as top:
        # ---------------- constants ----------------
        const = top.enter_context(tc.tile_pool(name="const", bufs=1))
        ident_f32 = const.tile([128, 128], f32)
        make_identity(nc, ident_f32)
        ones32 = const.tile([32, 1], f32)
        nc.gpsimd.memset(ones32, 1.0)
        ones128 = const.tile([128, 1], f32)
        nc.gpsimd.memset(ones128, 1.0)

        # ---------------- token masks (device-side) ----------------
        mpool = top.enter_context(tc.tile_pool(name="masks", bufs=1))
        dram = top.enter_context(tc.tile_pool(name="dram", bufs=1, space="DRAM"))

        # per-image region mask [BI, NREG]: n < im_len-1
        imlen_sb = mpool.tile([BI, 1], i32)
        nc.gpsimd.dma_start(imlen_sb, imlen_ext.rearrange("(p o) -> p o", o=1))
        il_sb = mpool.tile([BI, 1], f32)
        nc.gpsimd.tensor_scalar(il_sb, imlen_sb, 1, None, op0=ALU.subtract)
        iota_r = mpool.tile([BI, NREG], f32)
        nc.gpsimd.iota(
            iota_r, pattern=[[1, NREG]], base=0, channel_multiplier=0,
            allow_small_or_imprecise_dtypes=True,
        )
        mask_im = mpool.tile([BI, NREG], f32)
        nc.gpsimd.tensor_scalar(mask_im, iota_r, il_sb, None, op0=ALU.is_lt)
        # maskcol_im [128, 11]: per (partition, im stage tile) in REGION-major
        # stage order (p = ni*n + i); pad rows -> 0
        mask_im_dram = dram.tile([BI * NREG], f32)
        nc.gpsimd.dma_start(
            mask_im_dram.rearrange("(i n) -> i n", n=NREG), mask_im
        )
        maskcol_im = mpool.tile([128, len(IM_STAGE)], f32)
        nc.gpsimd.memset(maskcol_im, 0.0)
        for t, (toff, win, i0, ni) in enumerate(IM_STAGE):
            nc.gpsimd.dma_start(
                maskcol_im[0:36 * ni, t:t + 1],
                mask_im_dram[36 * i0:36 * (i0 + ni)].rearrange(
                    "(i n) -> n i", n=NREG
                ),
            )

        # s word masks -> w4 block-ones weights [128, 64, 4] bf16:
        #   w4[32*jj + m, wt, jj] = (m < s_len[4*wt+jj] - 3)
        slen_sb = mpool.tile([128, 2], i32)
        nc.gpsimd.dma_start(slen_sb, slen_ext.rearrange("(t p) -> p t", p=128))
        sl_sb = mpool.tile([128, 2], f32)
        nc.gpsimd.tensor_scalar(sl_sb, slen_sb, 3, None, op0=ALU.subtract)
        iota_w = mpool.tile([128, NWORD], f32)
        nc.gpsimd.iota(
            iota_w, pattern=[[1, NWORD]], base=0, channel_multiplier=0,
            allow_small_or_imprecise_dtypes=True,
        )
        # Word-sum weights for WORD-major stage order (p = 4w + j) combined
        # with the SwInterleave token reversal (partition p <-> raw col 127-p):
        #   w4[p, wt, jj] = [ (127-p)%4 == jj ] * ( (127-p)//4 < sl[4*wt+jj] )
        # Built transposed (partition c = sentence-within-half, free p) then
        # PE-transposed into place.
        rb = mpool.tile([128, 128], f32)     # rb[c, p] = (127-p)//4
        nc.gpsimd.iota(rb, pattern=[[-1, 32], [0, 4]], base=31,
                       channel_multiplier=0, allow_small_or_imprecise_dtypes=True)
        # sel[c, p] = (p%4 == 3 - c%4)  <=>  ((c + p + 1) & 3 == 0)
        cp_i = mpool.tile([128, 128], i32)
        nc.gpsimd.iota(cp_i, pattern=[[1, 128]], base=1, channel_multiplier=1)
        cp_a = mpool.tile([128, 128], i32)
        nc.vector.tensor_scalar(cp_a, cp_i, 3, None, op0=ALU.bitwise_and)
        sel = mpool.tile([128, 128], f32)
        nc.vector.tensor_scalar(sel, cp_a, 0, None, op0=ALU.is_equal)
        w4 = mpool.tile([128, S_TILES, 4], bf16)
        with tc.tile_pool(name="w4ps", bufs=2, space="PSUM") as wps:
            for h in range(2):
                bh = mpool.tile([128, 128], f32, tag=f"w4bh{h}")
                nc.vector.tensor_scalar(
                    bh, rb, sl_sb[:, h:h + 1], None, op0=ALU.is_lt
                )
                bm = mpool.tile([128, 128], f32, tag=f"w4bm{h}")
                nc.vector.tensor_mul(bm, bh, sel)
                wt_ps = wps.tile([128, 128], f32, tag=f"w4t{h}")
                nc.tensor.transpose(wt_ps, bm, ident_f32)
                nc.vector.tensor_copy(
                    w4[:, 32 * h:32 * (h + 1), :].rearrange("p a b -> p (a b)"),
                    wt_ps,
                )
                if opts.get("debug"):
                    nc.sync.dma_start(dbg_bm[h, :, :], bm)
        if opts.get("debug"):
            nc.sync.dma_start(dbg_sel[:, :], sel)
            nc.sync.dma_start(dbg_rb[:, :], rb)

        # diag masks (sharding metadata inputs)
        dmask_sb = mpool.tile([128, 2, BI], f32)
        nc.gpsimd.dma_start(dmask_sb, dmask_ext.rearrange("(t p) i -> p t i", p=128))
        dmaskT_sb = mpool.tile([BI, 2, 128], f32)
        nc.gpsimd.dma_start(
            dmaskT_sb, dmaskT_ext.rearrange("p (t f) -> p t f", f=128)
        )

        # ---------------- persistent big buffers ----------------
        big = top.enter_context(tc.tile_pool(name="big", bufs=1))
        # packed-transposed fp8 pairs, stored as bf16 bit containers.
        # One tile per staging unit so the (whole-tile) dependency tracking
        # of the XBAR DMA writes stays exactly per-tile.
        IMQ = [[0, 1, 2, 3], [4, 5, 6, 7], [8, 9], [10]]
        imTq_g = [
            big.tile([128, 4 * len(ts), 112], bf16, name=f"imTq{g}")
            for g, ts in enumerate(IMQ)
        ]
        imP = big.tile([128, 4, 2, IM_TOK], fp8)      # dense planar im
        sTp_g = [
            big.tile([128, 4, 4, 128], bf16, name=f"sTpg{g}")
            for g in range(S_TILES // 4)
        ]
        maxima = big.tile([128, S_TILES, BI], bf16)  # per (word, wtile, img) region-max
        scoresT_sb = big.tile([BI, S_TILES, 4], f32)  # [img, wt, jj] == [img, sent]
        scores_sb = big.tile([128, 2, BI], f32)       # [sent%128, sent//128, img]

        # fp8 views: im pair-split for the deinterleave, s raw for SwInterleave
        imTq8_g = [
            t.bitcast(fp8).rearrange("p m (t b) -> p m b t", b=2) for t in imTq_g
        ]
        sTraw_g = [t.bitcast(fp8) for t in sTp_g]     # each [128, 4, 4, 256]

        with ExitStack() as mid:
            stage = mid.enter_context(
                tc.tile_pool(name="stage", bufs=opts["sf_bufs"])
            )
            pk = mid.enter_context(tc.tile_pool(name="pk", bufs=opts["pk_bufs"]))
            alp = mid.enter_context(
                tc.tile_pool(name="alp", bufs=opts["alp_bufs"], space="PSUM")
            )
            scp = mid.enter_context(tc.tile_pool(name="scp", bufs=1, space="PSUM"))
            scoresT_ps = scp.tile([BI, S_TILES, 4], f32)

            def stage_im_quad(g):
                ts = IMQ[g]
                win_g = IM_STAGE[ts[-1]][1]
                tf32s = []
                for t in ts:
                    toff, win, i0, ni = IM_STAGE[t]
                    nreal = 36 * ni
                    tf32 = stage.tile([128, D], f32, tag="sf32")
                    nc.sync.dma_start(
                        tf32[0:nreal, :],
                        im_ext[i0:i0 + ni, 1:1 + NREG, :].rearrange(
                            "i n d -> n i d"
                        ),
                    )
                    tf32s.append(tf32)
                ipkq = pk.tile([128, len(ts), D // 2], bf16, tag="spk")
                for k, t in enumerate(ts):
                    toff, win, i0, ni = IM_STAGE[t]
                    # masked cast on the (ramp-idle) vector engine
                    nc.vector.tensor_scalar(
                        ipkq[:, k, :].bitcast(fp8)[0:win, :], tf32s[k][0:win, :],
                        maskcol_im[0:win, t:t + 1], None, op0=ALU.mult,
                    )
                # one XBAR DMA for the whole quad (scalar queue: keeps the
                # sync queue free to prefetch s loads without blocking)
                nc.scalar.dma_start(
                    imTq_g[g][:, :, 0:win_g],
                    ipkq[0:win_g, :, :].rearrange("p a c -> p (a c)"),
                    transpose=True,
                )
            def im_deints(g):
                # deinterleave + compact + un-permute (region-major stage
                # order ni*n + i -> dense 36*i + n), one strided copy per tile
                for k, t in enumerate(IMQ[g]):
                    toff, win, i0, ni = IM_STAGE[t]
                    nreal = 36 * ni
                    nc.vector.tensor_copy(
                        imP[:, :, :, toff:toff + nreal].rearrange(
                            "p q b (i n) -> p q b i n", n=NREG
                        ),
                        imTq8_g[g][:, 4 * k:4 * k + 4, :, 0:nreal].rearrange(
                            "p q b (n i) -> p q b n i", i=ni
                        ).rearrange("p q b n i -> p q b i n"),
                    )

            def s_load(gq):
                # four per-tile loads (the DMA AP balancer caps at 3 dims,
                # so a quad can't be one DMA), word-major per tile
                tfs = []
                for a in range(4):
                    i = 4 * gq + a
                    tf32 = stage.tile([128, D], f32, tag="sf32")
                    nc.sync.dma_start(
                        tf32,
                        s_ext[4 * i:4 * i + 4, 1:1 + NWORD, :].rearrange(
                            "j w d -> w j d"
                        ),
                    )
                    tfs.append(tf32)
                return tfs

            def s_pack(gq, tfs):
                pkq = pk.tile([128, 4, D // 2], bf16, tag="spk")
                gsp = opts["gpsimd_cast"]
                for a in range(4):
                    i = 4 * gq + a
                    if gsp and (i % gsp == gsp - 1):
                        nc.vector.tensor_copy(
                            pkq[:, a, :].bitcast(fp8), tfs[a]
                        )
                    else:
                        nc.scalar.activation(
                            pkq[:, a, :].bitcast(fp8), tfs[a], ACTF.Copy
                        )
                nc.sync.dma_start(
                    sTp_g[gq].rearrange("p a q t -> p (a q) t"),
                    pkq.rearrange("p a c -> p (a c)"),
                    transpose=True,
                )

            # im head first (needed by rc0); s quad-loads run ahead of their
            # cast+transpose stages so no dispatch queue ever blocks
            PRE = opts["s_prefetch"]
            for g in range(len(IMQ)):
                stage_im_quad(g)
            for g in range(len(IMQ)):
                im_deints(g)
            pending = []
            for gq in range(S_TILES // 4):
                pending.append((gq, s_load(gq)))
                if len(pending) > PRE:
                    s_pack(*pending.pop(0))
            for it in pending:
                s_pack(*it)

            # ---------------- main matmul + region-max + word-sum ----------------
            def word_sum(wt):
                # scoresT[img, 4wt+jj] = sum_m maxima[(jj,m), wt, img] * wmask
                nc.tensor.matmul(
                    scoresT_ps[:, wt, :],
                    lhsT=maxima[:, wt, :],
                    rhs=w4[:, wt, :],
                    start=True, stop=True,
                )

            for wt in range(S_TILES):
                for rci, (toff, ntok, nimg) in enumerate(RCHUNKS):
                    pal = alp.tile([128, nimg, NREG], f32, tag="align")
                    for q in range(4):
                        nc.tensor.matmul(
                            pal.rearrange("p a b -> p (a b)"),
                            lhsT=sTraw_g[wt // 4][:, wt % 4, q, :],
                            rhs=imP[:, q, :, toff:toff + ntok],
                            start=(q == 0),
                            stop=(q == 3),
                            perf_mode=SWI,
                        )
                    nc.vector.tensor_reduce(
                        maxima[:, wt, toff // NREG:toff // NREG + nimg],
                        pal, axis=AX.X, op=ALU.max,
                    )
                    # emit the previous tile's word-sum between rc chunks so the
                    # PE never waits on the vector MAX of the current tile
                    if rci == 0 and wt > 0:
                        word_sum(wt - 1)
            word_sum(S_TILES - 1)

            # scoresT -> SBUF, then transpose back to [sent, img]
            nc.vector.tensor_copy(scoresT_sb, scoresT_ps)
            if opts.get("debug"):
                w4d = mpool.tile([128, S_TILES, 4], f32)
                nc.vector.tensor_copy(w4d, w4)
                nc.sync.dma_start(dbg_w4[:, :, :], w4d)
                nc.sync.dma_start(dbg_scT[:, :, :], scoresT_sb)
                mxd = mpool.tile([128, S_TILES, BI], f32)
                nc.vector.tensor_copy(mxd, maxima)
                nc.sync.dma_start(dbg_mx[:, :, :], mxd)
            sc_ps = scp.tile([128, 2, BI], f32)
            for t in range(2):
                nc.tensor.transpose(
                    sc_ps[:, t, :],
                    scoresT_sb[:, 32 * t:32 * (t + 1), :].rearrange(
                        "p a b -> p (a b)"
                    ),
                    ident_f32[:BI, :BI],
                )
                nc.vector.tensor_copy(scores_sb[:, t, :], sc_ps[:, t, :])

        # ---------------- loss tail ----------------
        with ExitStack() as tail:
            tp = tail.enter_context(tc.tile_pool(name="tailp", bufs=1, space="PSUM"))
            ts = tail.enter_context(tc.tile_pool(name="tails", bufs=1))

            # col-max over local images (diag excluded) + scattered diag
            masked = ts.tile([128, 2, BI], f32)
            nc.vector.scalar_tensor_tensor(
                masked, dmask_sb, -BIG, scores_sb, op0=ALU.mult, op1=ALU.add
            )
            colmax_p = ts.tile([128, 2], f32)
            nc.vector.tensor_reduce(colmax_p, masked, axis=AX.X, op=ALU.max)
            dtmp = ts.tile([128, 2, BI], f32)
            nc.vector.tensor_mul(dtmp, dmask_sb, scores_sb)
            dscat = ts.tile([128, 2], f32)
            nc.vector.tensor_reduce(dscat, dtmp, axis=AX.X, op=ALU.add)

            # row-max over sentences (diag excluded); scoresT_sb is [img, sent]
            scT_flat = scoresT_sb.rearrange("p a b -> p (a b)")
            dmaskT_flat = dmaskT_sb.rearrange("p a b -> p (a b)")
            maskedT = ts.tile([BI, B], f32)
            nc.vector.scalar_tensor_tensor(
                maskedT, dmaskT_flat, -BIG, scT_flat, op0=ALU.mult, op1=ALU.add
            )
            rowmax = ts.tile([BI, 1], f32)
            nc.vector.tensor_reduce(rowmax, maskedT, axis=AX.X, op=ALU.max)
            dT_tmp = ts.tile([BI, B], f32)
            nc.vector.tensor_mul(dT_tmp, dmaskT_flat, scT_flat)
            d_row = ts.tile([BI, 1], f32)
            nc.vector.tensor_reduce(d_row, dT_tmp, axis=AX.X, op=ALU.add)

            cost_s = ts.tile([BI, 1], f32)
            nc.vector.tensor_sub(cost_s, rowmax, d_row)
            nc.vector.tensor_scalar(
                cost_s, cost_s, MARGIN, 0.0, op0=ALU.add, op1=ALU.max
            )
            cs_ps = tp.tile([1, 1], f32)
            nc.tensor.matmul(cs_ps, lhsT=ones32, rhs=cost_s, start=True, stop=True)

            # one packed tile: [:,0:2]=colmax | [:,2:4]=dscat | [0,4]=cost_s
            pkt = ts.tile([128, 5], f32)
            nc.vector.tensor_copy(pkt[:, 0:2], colmax_p)
            nc.vector.tensor_copy(pkt[:, 2:4], dscat)
            nc.vector.tensor_copy(pkt[0:1, 4:5], cs_ps)
            blk = dram.tile([BLK], f32)
            nc.sync.dma_start(
                blk[0:640].rearrange("(a p) -> p a", p=128), pkt
            )
            gath = dram.tile([NCORES, BLK], f32, addr_space="Shared")
            nc.gpsimd.collective_compute(
                "AllGather",
                ALU.bypass,
                ins=[blk.opt()],
                outs=[gath.opt()],
                replica_groups=[list(range(NCORES))],
            )

            # redundant final reduction on every core; single unpack DMA
            # per-piece transposing unpacks (DMA APs cap at 3 dims with a
            # contiguous final dim), spread across both dispatch queues
            g5 = ts.tile([128, 5, NCORES], f32)
            for a in range(5):
                eng = nc.sync if a % 2 == 0 else nc.scalar
                eng.dma_start(
                    g5[:, a, :],
                    gath[:, 128 * a:128 * (a + 1)].rearrange("c p -> p c"),
                )
            colmax_g = ts.tile([128, 2], f32)
            nc.vector.tensor_reduce(colmax_g, g5[:, 0:2, :], axis=AX.X, op=ALU.max)
            d_all = ts.tile([128, 2], f32)
            nc.vector.tensor_reduce(d_all, g5[:, 2:4, :], axis=AX.X, op=ALU.add)
            cim = ts.tile([128, 2], f32)
            nc.vector.tensor_sub(cim, colmax_g, d_all)
            nc.vector.tensor_scalar(cim, cim, MARGIN, 0.0, op0=ALU.add, op1=ALU.max)
            cim_r = ts.tile([128, 1], f32)
            nc.vector.tensor_reduce(cim_r, cim, axis=AX.X, op=ALU.add)
            cs_tot = ts.tile([1, 1], f32)
            nc.vector.tensor_reduce(cs_tot, g5[0:1, 4, :], axis=AX.X, op=ALU.add)
            nc.vector.tensor_add(cim_r[0:1, :], cim_r[0:1, :], cs_tot)
            tot_ps = tp.tile([1, 1], f32)
            nc.tensor.matmul(tot_ps, lhsT=ones128, rhs=cim_r, start=True, stop=True)
            total = ts.tile([1, 1], f32)
            nc.vector.tensor_copy(total, tot_ps)
            nc.sync.dma_start(out_ext[0:1], total[0, :])

    fix_multiwaits(nc, mybir)
    return nc


_CACHE = {}


def _get_nc():
    if "nc" not in _CACHE:
        _CACHE["nc"] = build_graph()
    return _CACHE["nc"]


def make_in_maps(im_set, s_seq, im_len, s_len):
    im_set = np.ascontiguousarray(im_set, dtype=np.float32)
    s_seq = np.ascontiguousarray(s_seq, dtype=np.float32)
    im_len = np.ascontiguousarray(im_len, dtype=np.int32)
    s_len = np.ascontiguousarray(s_len, dtype=np.int32)
    in_maps = []
    for c in range(NCORES):
        dm = np.zeros((B, BI), dtype=np.float32)
        for i in range(BI):
            dm[BI * c + i, i] = 1.0
        in_maps.append({
            "im_set": im_set[BI * c:BI * (c + 1)],
            "s_seq": s_seq,
            "im_len": im_len[BI * c:BI * (c + 1)],
            "s_len": s_len,
            "diag_mask": dm,
            "diag_maskT": np.ascontiguousarray(dm.T),
        })
    return in_maps


def kernel(im_set, s_seq, im_len, s_len):
    import time
    from concourse.bass_utils import run_bass_kernel_spmd

    nc = _get_nc()
    in_maps = make_in_maps(im_set, s_seq, im_len, s_len)
    last = None
    for attempt in range(3):
        try:
            res = run_bass_kernel_spmd(nc, in_maps, core_ids=list(range(NCORES)))
            return np.asarray(
                res.results[0]["out"], dtype=np.float32
            ).reshape(())[()]
        except Exception as e:  # transient device-unrecoverable happens
            last = e
            time.sleep(30 * (attempt + 1))
    raise last


# revision 40
# speedup vs baseline: 1.6811x; 1.0008x over previous
"""Trainium2 Bass kernel for AlignmentContrastiveLoss (8 NeuronCores, SPMD).

Reference semantics:
  im = im_set[:, 1:, :]           [256, 36, 1024]
  s  = s_seq[:, 1:-2, :]          [256, 32, 1024]
  align[i,j,n,m] = im[i,n] . s[j,m], zeroed where n >= im_len[i]-1 or m >= s_len[j]-3
  scores[i,j] = sum_m max_n align[i,j,n,m]
  loss = sum_i relu(M + max_{j!=i} scores[i,j] - scores[i,i])
       + sum_j relu(M + max_{i!=j} scores[i,j] - scores[j,j])

Sharding: data-parallel over images (32 per core), s replicated.

v2 design:
  - f32 tokens are cast once to fp8e4 (im-mask fused as activation scale),
    bitcast to 16-bit fp8-pairs and transposed via the DMA XBAR (pure bit
    movement) into a packed layout: partition p of q-chunk q holds the d
    pair (256q+2p, 256q+2p+1) interleaved per token (HW-verified mapping).
  - s feeds the PE as RAW packed bytes via MatmulPerfMode.DoubleRowSwInterleave
    (stationary side accepts interleaved pairs; output partitions come out
    token-REVERSED, verified on HW). im (small) is deinterleaved to planar
    [128, q, 2, tok] fp8 by one gpsimd 4D copy per tile, which also compacts
    away the XBAR pad columns. No PE transposes, no PSUM->SBUF staging copies.
  - The s-token reversal is compensated in the word-sum weights (w4 built
    from a reversed word mask), so scoresT and the loss tail are unchanged.
  - wt-outer loop: per s-tile, 3 region-chunk matmul groups + vector MAX
    reduce; word-sum is a tiny PE matmul against s-mask-weighted block-ones
    (applies the s word mask for free and directly yields scoresT[img,sent]).
  - im staged as 11 tiles of 3 images (112-partition windows, 4-col overlap
    into the next tile's range which is later overwritten with real data).
  - Cross-core traffic: one 520-float AllGather of per-core column-max
    partials + scattered diagonals + local cost_s sum.
"""

import numpy as np

MARGIN = 0.2
B = 256          # global batch (images == sentences)
NCORES = 8
BI = B // NCORES  # images per core = 32
NREG = 36        # regions per image after stripping
NWORD = 32       # words per sentence after stripping
D = 1024
IM_TOK = BI * NREG      # 1152 dense im tokens
S_TOK = B * NWORD       # 8192 s tokens
S_TILES = S_TOK // 128  # 64
BIG = 1.0e30
# region chunks for the main matmul: (token offset, ntok, nimg)
RCHUNKS = [(0, 432, 12), (432, 432, 12), (864, 288, 8)]
# im staging tiles: (dense token offset, window (mult of 16), first image, n images)
IM_STAGE = [(108 * t, 112, 3 * t, 3) for t in range(10)] + [(1080, 112, 30, 2)]
IM_TP_COLS = 112 * 11  # padded XBAR destination: disjoint 112-col windows
BLK = 640  # allgather block floats: [p-major x5] colmax x2 | dscat x2 | cost_s


def fix_multiwaits(nc, mybir):
    """This toolchain's walrus accepts 1 wait per instruction (2 for
    EventSemaphore); Tile can emit more. Offload surplus waits onto
    inserted same-engine NoOps placed immediately before the instruction."""
    n_fix = 0
    for fn in nc.m.functions:
        for blk in fn.blocks:
            insts = blk.instructions
            i = 0
            while i < len(insts):
                inst = insts[i]
                si = inst.sync_info
                waits = list(si.on_wait) if si is not None and si.on_wait else []
                cap = 2 if isinstance(inst, mybir.InstEventSemaphore) else 1
                if len(waits) > cap:
                    surplus, keep = waits[:-cap], waits[-cap:]
                    si.on_wait = keep
                    for w in surplus:
                        nop = mybir.InstNoOp(
                            name=f"{inst.name}_wsplit{n_fix}",
                            engine=inst.engine,
                            ins=[],
                            outs=[],
                            sync_info=mybir.SyncInfo(on_wait=[w], on_update=[]),
                        )
                        insts.insert(i, nop)
                        n_fix += 1
                        i += 1
                i += 1
    return n_fix


DEFAULT_OPTS = {
    "sf_bufs": 18,     # f32 staging tiles
    "pk_bufs": 6,      # packed fp8-as-bf16 staging quad tiles
    "alp_bufs": 6,     # PSUM align buffers
    "gpsimd_cast": 0,  # every Nth s cast on vector (0 = all scalar)
    "s_prefetch": 3,   # s quad-loads dispatched ahead of their pack stage
    "im_head": 4,      # im tiles staged before the first s tile
}


def build_graph(opts=None):
    import concourse.bass as bass
    import concourse.mybir as mybir
    import concourse.tile as tile
    from concourse.masks import make_identity
    from contextlib import ExitStack

    opts = {**DEFAULT_OPTS, **(opts or {})}

    f32 = mybir.dt.float32
    bf16 = mybir.dt.bfloat16
    fp8 = mybir.dt.float8e4
    i32 = mybir.dt.int32
    ALU = mybir.AluOpType
    AX = mybir.AxisListType
    ACTF = mybir.ActivationFunctionType
    SWI = mybir.MatmulPerfMode.DoubleRowSwInterleave

    nc = bass.Bass()

    im_ext = nc.declare_dram_parameter("im_set", [BI, 37, D], f32, isOutput=False)
    s_ext = nc.declare_dram_parameter("s_seq", [B, 35, D], f32, isOutput=False)
    imlen_ext = nc.declare_dram_parameter("im_len", [BI], i32, isOutput=False)
    slen_ext = nc.declare_dram_parameter("s_len", [B], i32, isOutput=False)
    dmask_ext = nc.declare_dram_parameter("diag_mask", [B, BI], f32, isOutput=False)
    dmaskT_ext = nc.declare_dram_parameter("diag_maskT", [BI, B], f32, isOutput=False)
    out_ext = nc.declare_dram_parameter("out", [1], f32, isOutput=True)
    if opts.get("debug"):
        dbg_sel = nc.declare_dram_parameter("dbg_sel", [128, 128], f32, isOutput=True)
        dbg_pm = nc.declare_dram_parameter("dbg_pm", [128, 128], f32, isOutput=True)
        dbg_rb = nc.declare_dram_parameter("dbg_rb", [128, 128], f32, isOutput=True)
        dbg_tc = nc.declare_dram_parameter("dbg_tc", [128, 2], f32, isOutput=True)
        dbg_bm = nc.declare_dram_parameter("dbg_bm", [2, 128, 128], f32, isOutput=True)
        dbg_w4 = nc.declare_dram_parameter("dbg_w4", [128, S_TILES, 4], f32, isOutput=True)
        dbg_scT = nc.declare_dram_parameter("dbg_scT", [BI, S_TILES, 4], f32, isOutput=True)
        dbg_mx = nc.declare_dram_parameter("dbg_mx", [128, S_TILES, BI], f32, isOutput=True)

    with tile.TileContext(nc) as tc, ExitStack() as top:
        # ---------------- constants ----------------
        const = top.enter_context(tc.tile_pool(name="const", bufs=1))
        ident_f32 = const.tile([128, 128], f32)
        make_identity(nc, ident_f32)
        ones32 = const.tile([32, 1], f32)
        nc.gpsimd.memset(ones32, 1.0)
        ones128 = const.tile([128, 1], f32)
        nc.gpsimd.memset(ones128, 1.0)

        # ---------------- token masks (device-side) ----------------
        mpool = top.enter_context(tc.tile_pool(name="masks", bufs=1))
        dram = top.enter_context(tc.tile_pool(name="dram", bufs=1, space="DRAM"))

        # per-image region mask [BI, NREG]: n < im_len-1
        imlen_sb = mpool.tile([BI, 1], i32)
        nc.gpsimd.dma_start(imlen_sb, imlen_ext.rearrange("(p o) -> p o", o=1))
        il_sb = mpool.tile([BI, 1], f32)
        nc.gpsimd.tensor_scalar(il_sb, imlen_sb, 1, None, op0=ALU.subtract)
        iota_r = mpool.tile([BI, NREG], f32)
        nc.gpsimd.iota(
            iota_r, pattern=[[1, NREG]], base=0, channel_multiplier=0,
            allow_small_or_imprecise_dtypes=True,
        )
        mask_im = mpool.tile([BI, NREG], f32)
        nc.gpsimd.tensor_scalar(mask_im, iota_r, il_sb, None, op0=ALU.is_lt)
        # maskcol_im [128, 11]: per (partition, im stage tile) in REGION-major
        # stage order (p = ni*n + i); pad rows -> 0
        mask_im_dram = dram.tile([BI * NREG], f32)
        nc.gpsimd.dma_start(
            mask_im_dram.rearrange("(i n) -> i n", n=NREG), mask_im
        )
        maskcol_im = mpool.tile([128, len(IM_STAGE)], f32)
        nc.gpsimd.memset(maskcol_im, 0.0)
        for t, (toff, win, i0, ni) in enumerate(IM_STAGE):
            nc.gpsimd.dma_start(
                maskcol_im[0:36 * ni, t:t + 1],
                mask_im_dram[36 * i0:36 * (i0 + ni)].rearrange(
                    "(i n) -> n i", n=NREG
                ),
            )

        # s word masks -> w4 block-ones weights [128, 64, 4] bf16:
        #   w4[32*jj + m, wt, jj] = (m < s_len[4*wt+jj] - 3)
        slen_sb = mpool.tile([128, 2], i32)
        nc.gpsimd.dma_start(slen_sb, slen_ext.rearrange("(t p) -> p t", p=128))
        sl_sb = mpool.tile([128, 2], f32)
        nc.gpsimd.tensor_scalar(sl_sb, slen_sb, 3, None, op0=ALU.subtract)
        iota_w = mpool.tile([128, NWORD], f32)
        nc.gpsimd.iota(
            iota_w, pattern=[[1, NWORD]], base=0, channel_multiplier=0,
            allow_small_or_imprecise_dtypes=True,
        )
        # Word-sum weights for WORD-major stage order (p = 4w + j) combined
        # with the SwInterleave token reversal (partition p <-> raw col 127-p):
        #   w4[p, wt, jj] = [ (127-p)%4 == jj ] * ( (127-p)//4 < sl[4*wt+jj] )
        # Built transposed (partition c = sentence-within-half, free p) then
        # PE-transposed into place.
        rb = mpool.tile([128, 128], f32)     # rb[c, p] = (127-p)//4
        nc.gpsimd.iota(rb, pattern=[[-1, 32], [0, 4]], base=31,
                       channel_multiplier=0, allow_small_or_imprecise_dtypes=True)
        # sel[c, p] = (p%4 == 3 - c%4)  <=>  ((c + p + 1) & 3 == 0)
        cp_i = mpool.tile([128, 128], i32)
        nc.gpsimd.iota(cp_i, pattern=[[1, 128]], base=1, channel_multiplier=1)
        cp_a = mpool.tile([128, 128], i32)
        nc.vector.tensor_scalar(cp_a, cp_i, 3, None, op0=ALU.bitwise_and)
        sel = mpool.tile([128, 128], f32)
        nc.vector.tensor_scalar(sel, cp_a, 0, None, op0=ALU.is_equal)
        w4 = mpool.tile([128, S_TILES, 4], bf16)
        with tc.tile_pool(name="w4ps", bufs=2, space="PSUM") as wps:
            for h in range(2):
                bh = mpool.tile([128, 128], f32, tag=f"w4bh{h}")
                nc.vector.tensor_scalar(
                    bh, rb, sl_sb[:, h:h + 1], None, op0=ALU.is_lt
                )
                bm = mpool.tile([128, 128], f32, tag=f"w4bm{h}")
                nc.vector.tensor_mul(bm, bh, sel)
                wt_ps = wps.tile([128, 128], f32, tag=f"w4t{h}")
                nc.tensor.transpose(wt_ps, bm, ident_f32)
                nc.vector.tensor_copy(
                    w4[:, 32 * h:32 * (h + 1), :].rearrange("p a b -> p (a b)"),
                    wt_ps,
                )
                if opts.get("debug"):
                    nc.sync.dma_start(dbg_bm[h, :, :], bm)
        if opts.get("debug"):
            nc.sync.dma_start(dbg_sel[:, :], sel)
            nc.sync.dma_start(dbg_rb[:, :], rb)

        # diag masks (sharding metadata inputs)
        dmask_sb = mpool.tile([128, 2, BI], f32)
        nc.gpsimd.dma_start(dmask_sb, dmask_ext.rearrange("(t p) i -> p t i", p=128))
        dmaskT_sb = mpool.tile([BI, 2, 128], f32)
        nc.gpsimd.dma_start(
            dmaskT_sb, dmaskT_ext.rearrange("p (t f) -> p t f", f=128)
        )

        # ---------------- persistent big buffers ----------------
        big = top.enter_context(tc.tile_pool(name="big", bufs=1))
        # packed-transposed fp8 pairs, stored as bf16 bit containers.
        # One tile per staging unit so the (whole-tile) dependency tracking
        # of the XBAR DMA writes stays exactly per-tile.
        IMQ = [[0, 1, 2, 3], [4, 5, 6, 7], [8, 9], [10]]
        imTq_g = [
            big.tile([128, 4 * len(ts), 112], bf16, name=f"imTq{g}")
            for g, ts in enumerate(IMQ)
        ]
        imP = big.tile([128, 4, 2, IM_TOK], fp8)      # dense planar im
        sTp_g = [
            big.tile([128, 4, 4, 128], bf16, name=f"sTpg{g}")
            for g in range(S_TILES // 4)
        ]
        maxima = big.tile([128, S_TILES, BI], bf16)  # per (word, wtile, img) region-max
        scoresT_sb = big.tile([BI, S_TILES, 4], f32)  # [img, wt, jj] == [img, sent]
        scores_sb = big.tile([128, 2, BI], f32)       # [sent%128, sent//128, img]

        # fp8 views: im pair-split for the deinterleave, s raw for SwInterleave
        imTq8_g = [
            t.bitcast(fp8).rearrange("p m (t b) -> p m b t", b=2) for t in imTq_g
        ]
        sTraw_g = [t.bitcast(fp8) for t in sTp_g]     # each [128, 4, 4, 256]

        with ExitStack() as mid:
            stage = mid.enter_context(
                tc.tile_pool(name="stage", bufs=opts["sf_bufs"])
            )
            pk = mid.enter_context(tc.tile_pool(name="pk", bufs=opts["pk_bufs"]))
            alp = mid.enter_context(
                tc.tile_pool(name="alp", bufs=opts["alp_bufs"], space="PSUM")
            )
            scp = mid.enter_context(tc.tile_pool(name="scp", bufs=1, space="PSUM"))
            scoresT_ps = scp.tile([BI, S_TILES, 4], f32)

            def stage_im_quad(g):
                ts = IMQ[g]
                win_g = IM_STAGE[ts[-1]][1]
                tf32s = []
                for t in ts:
                    toff, win, i0, ni = IM_STAGE[t]
                    nreal = 36 * ni
                    tf32 = stage.tile([128, D], f32, tag="sf32")
                    nc.sync.dma_start(
                        tf32[0:nreal, :],
                        im_ext[i0:i0 + ni, 1:1 + NREG, :].rearrange(
                            "i n d -> n i d"
                        ),
                    )
                    tf32s.append(tf32)
                ipkq = pk.tile([128, len(ts), D // 2], bf16, tag="spk")
                for k, t in enumerate(ts):
                    toff, win, i0, ni = IM_STAGE[t]
                    # masked cast on the (ramp-idle) vector engine
                    nc.vector.tensor_scalar(
                        ipkq[:, k, :].bitcast(fp8)[0:win, :], tf32s[k][0:win, :],
                        maskcol_im[0:win, t:t + 1], None, op0=ALU.mult,
                    )
                # one XBAR DMA for the whole quad (scalar queue: keeps the
                # sync queue free to prefetch s loads without blocking)
                nc.scalar.dma_start(
                    imTq_g[g][:, :, 0:win_g],
                    ipkq[0:win_g, :, :].rearrange("p a c -> p (a c)"),
                    transpose=True,
                )
            def im_deints(g):
                # deinterleave + compact + un-permute (region-major stage
                # order ni*n + i -> dense 36*i + n), one strided copy per tile
                for k, t in enumerate(IMQ[g]):
                    toff, win, i0, ni = IM_STAGE[t]
                    nreal = 36 * ni
                    nc.vector.tensor_copy(
                        imP[:, :, :, toff:toff + nreal].rearrange(
                            "p q b (i n) -> p q b i n", n=NREG
                        ),
                        imTq8_g[g][:, 4 * k:4 * k + 4, :, 0:nreal].rearrange(
                            "p q b (n i) -> p q b n i", i=ni
                        ).rearrange("p q b n i -> p q b i n"),
                    )

            def s_load(gq):
                # four per-tile loads (the DMA AP balancer caps at 3 dims,
                # so a quad can't be one DMA), word-major per tile
                tfs = []
                for a in range(4):
                    i = 4 * gq + a
                    tf32 = stage.tile([128, D], f32, tag="sf32")
                    nc.sync.dma_start(
                        tf32,
                        s_ext[4 * i:4 * i + 4, 1:1 + NWORD, :].rearrange(
                            "j w d -> w j d"
                        ),
                    )
                    tfs.append(tf32)
                return tfs

            def s_pack(gq, tfs):
                pkq = pk.tile([128, 4, D // 2], bf16, tag="spk")
                gsp = opts["gpsimd_cast"]
                for a in range(4):
                    i = 4 * gq + a
                    if gsp and (i % gsp == gsp - 1):
                        nc.vector.tensor_copy(
                            pkq[:, a, :].bitcast(fp8), tfs[a]
                        )
                    else:
                        nc.scalar.activation(
                            pkq[:, a, :].bitcast(fp8), tfs[a], ACTF.Copy
                        )
                nc.sync.dma_start(
                    sTp_g[gq].rearrange("p a q t -> p (a q) t"),
                    pkq.rearrange("p a c -> p (a c)"),
                    transpose=True,
                )

            # im head first (needed by rc0); s quad-loads run ahead of their
            # cast+transpose stages so no dispatch queue ever blocks
            # s staging starts immediately (its chain is mask-free); the im
            # quads stage in parallel behind the first three s quads so the
            # mask->cast->transpose chain never gates the s pipeline
            PRE = opts["s_prefetch"]
            HEADQ = 3
            pending = []
            for gq in range(HEADQ):
                pending.append((gq, s_load(gq)))
            for _ in range(HEADQ):
                s_pack(*pending.pop(0))
            for g in range(len(IMQ)):
                stage_im_quad(g)
            for g in range(len(IMQ)):
                im_deints(g)
            for gq in range(HEADQ, S_TILES // 4):
                pending.append((gq, s_load(gq)))
                if len(pending) > PRE:
                    s_pack(*pending.pop(0))
            for it in pending:
                s_pack(*it)

            # ---------------- main matmul + region-max + word-sum ----------------
            def word_sum(wt):
                # scoresT[img, 4wt+jj] = sum_m maxima[(jj,m), wt, img] * wmask
                nc.tensor.matmul(
                    scoresT_ps[:, wt, :],
                    lhsT=maxima[:, wt, :],
                    rhs=w4[:, wt, :],
                    start=True, stop=True,
                )

            for wt in range(S_TILES):
                for rci, (toff, ntok, nimg) in enumerate(RCHUNKS):
                    pal = alp.tile([128, nimg, NREG], f32, tag="align")
                    for q in range(4):
                        nc.tensor.matmul(
                            pal.rearrange("p a b -> p (a b)"),
                            lhsT=sTraw_g[wt // 4][:, wt % 4, q, :],
                            rhs=imP[:, q, :, toff:toff + ntok],
                            start=(q == 0),
                            stop=(q == 3),
                            perf_mode=SWI,
                        )
                    nc.vector.tensor_reduce(
                        maxima[:, wt, toff // NREG:toff // NREG + nimg],
                        pal, axis=AX.X, op=ALU.max,
                    )
                    # emit the previous tile's word-sum between rc chunks so the
                    # PE never waits on the vector MAX of the current tile
                    if rci == 0 and wt > 0:
                        word_sum(wt - 1)
            word_sum(S_TILES - 1)

            # scoresT -> SBUF, then transpose back to [sent, img]
            nc.vector.tensor_copy(scoresT_sb, scoresT_ps)
            if opts.get("debug"):
                w4d = mpool.tile([128, S_TILES, 4], f32)
                nc.vector.tensor_copy(w4d, w4)
                nc.sync.dma_start(dbg_w4[:, :, :], w4d)
                nc.sync.dma_start(dbg_scT[:, :, :], scoresT_sb)
                mxd = mpool.tile([128, S_TILES, BI], f32)
                nc.vector.tensor_copy(mxd, maxima)
                nc.sync.dma_start(dbg_mx[:, :, :], mxd)
            sc_ps = scp.tile([128, 2, BI], f32)
            for t in range(2):
                nc.tensor.transpose(
                    sc_ps[:, t, :],
                    scoresT_sb[:, 32 * t:32 * (t + 1), :].rearrange(
                        "p a b -> p (a b)"
                    ),
                    ident_f32[:BI, :BI],
                )
                nc.vector.tensor_copy(scores_sb[:, t, :], sc_ps[:, t, :])

        # ---------------- loss tail ----------------
        with ExitStack() as tail:
            tp = tail.enter_context(tc.tile_pool(name="tailp", bufs=1, space="PSUM"))
            ts = tail.enter_context(tc.tile_pool(name="tails", bufs=1))

            # col-max over local images (diag excluded) + scattered diag
            masked = ts.tile([128, 2, BI], f32)
            nc.vector.scalar_tensor_tensor(
                masked, dmask_sb, -BIG, scores_sb, op0=ALU.mult, op1=ALU.add
            )
            colmax_p = ts.tile([128, 2], f32)
            nc.vector.tensor_reduce(colmax_p, masked, axis=AX.X, op=ALU.max)
            dtmp = ts.tile([128, 2, BI], f32)
            nc.vector.tensor_mul(dtmp, dmask_sb, scores_sb)
            dscat = ts.tile([128, 2], f32)
            nc.vector.tensor_reduce(dscat, dtmp, axis=AX.X, op=ALU.add)

            # row-max over sentences (diag excluded); scoresT_sb is [img, sent]
            scT_flat = scoresT_sb.rearrange("p a b -> p (a b)")
            dmaskT_flat = dmaskT_sb.rearrange("p a b -> p (a b)")
            maskedT = ts.tile([BI, B], f32)
            nc.vector.scalar_tensor_tensor(
                maskedT, dmaskT_flat, -BIG, scT_flat, op0=ALU.mult, op1=ALU.add
            )
            rowmax = ts.tile([BI, 1], f32)
            nc.vector.tensor_reduce(rowmax, maskedT, axis=AX.X, op=ALU.max)
            dT_tmp = ts.tile([BI, B], f32)
            nc.vector.tensor_mul(dT_tmp, dmaskT_flat, scT_flat)
            d_row = ts.tile([BI, 1], f32)
            nc.vector.tensor_reduce(d_row, dT_tmp, axis=AX.X, op=ALU.add)

            cost_s = ts.tile([BI, 1], f32)
            nc.vector.tensor_sub(cost_s, rowmax, d_row)
            nc.vector.tensor_scalar(
                cost_s, cost_s, MARGIN, 0.0, op0=ALU.add, op1=ALU.max
            )
            cs_ps = tp.tile([1, 1], f32)
            nc.tensor.matmul(cs_ps, lhsT=ones32, rhs=cost_s, start=True, stop=True)

            # one packed tile: [:,0:2]=colmax | [:,2:4]=dscat | [0,4]=cost_s
            pkt = ts.tile([128, 5], f32)
            nc.vector.tensor_copy(pkt[:, 0:2], colmax_p)
            nc.vector.tensor_copy(pkt[:, 2:4], dscat)
            nc.vector.tensor_copy(pkt[0:1, 4:5], cs_ps)
            blk = dram.tile([BLK], f32)
            nc.sync.dma_start(
                blk[0:640].rearrange("(a p) -> p a", p=128), pkt
            )
            gath = dram.tile([NCORES, BLK], f32, addr_space="Shared")
            nc.gpsimd.collective_compute(
                "AllGather",
                ALU.bypass,
                ins=[blk.opt()],
                outs=[gath.opt()],
                replica_groups=[list(range(NCORES))],
            )

            # redundant final reduction on every core; single unpack DMA
            # per-piece transposing unpacks (DMA APs cap at 3 dims with a
            # contiguous final dim), spread across both dispatch queues
            g5 = ts.tile([128, 5, NCORES], f32)
            for a in range(5):
                eng = nc.sync if a % 2 == 0 else nc.scalar
                eng.dma_start(
                    g5[:, a, :],
                    gath[:, 128 * a:128 * (a + 1)].rearrange("c p -> p c"),
                )
            colmax_g = ts.tile([128, 2], f32)
            nc.vector.tensor_reduce(colmax_g, g5[:, 0:2, :], axis=AX.X, op=ALU.max)
            d_all = ts.tile([128, 2], f32)
            nc.vector.tensor_reduce(d_all, g5[:, 2:4, :], axis=AX.X, op=ALU.add)
            cim = ts.tile([128, 2], f32)
            nc.vector.tensor_sub(cim, colmax_g, d_all)
            nc.vector.tensor_scalar(cim, cim, MARGIN, 0.0, op0=ALU.add, op1=ALU.max)
            cim_r = ts.tile([128, 1], f32)
            nc.vector.tensor_reduce(cim_r, cim, axis=AX.X, op=ALU.add)
            cs_tot = ts.tile([1, 1], f32)
            nc.vector.tensor_reduce(cs_tot, g5[0:1, 4, :], axis=AX.X, op=ALU.add)
            nc.vector.tensor_add(cim_r[0:1, :], cim_r[0:1, :], cs_tot)
            tot_ps = tp.tile([1, 1], f32)
            nc.tensor.matmul(tot_ps, lhsT=ones128, rhs=cim_r, start=True, stop=True)
            total = ts.tile([1, 1], f32)
            nc.vector.tensor_copy(total, tot_ps)
            nc.sync.dma_start(out_ext[0:1], total[0, :])

    fix_multiwaits(nc, mybir)
    return nc


_CACHE = {}


def _get_nc():
    if "nc" not in _CACHE:
        _CACHE["nc"] = build_graph()
    return _CACHE["nc"]


def make_in_maps(im_set, s_seq, im_len, s_len):
    im_set = np.ascontiguousarray(im_set, dtype=np.float32)
    s_seq = np.ascontiguousarray(s_seq, dtype=np.float32)
    im_len = np.ascontiguousarray(im_len, dtype=np.int32)
    s_len = np.ascontiguousarray(s_len, dtype=np.int32)
    in_maps = []
    for c in range(NCORES):
        dm = np.zeros((B, BI), dtype=np.float32)
        for i in range(BI):
            dm[BI * c + i, i] = 1.0
        in_maps.append({
            "im_set": im_set[BI * c:BI * (c + 1)],
            "s_seq": s_seq,
            "im_len": im_len[BI * c:BI * (c + 1)],
            "s_len": s_len,
            "diag_mask": dm,
            "diag_maskT": np.ascontiguousarray(dm.T),
        })
    return in_maps


def kernel(im_set, s_seq, im_len, s_len):
    import time
    from concourse.bass_utils import run_bass_kernel_spmd

    nc = _get_nc()
    in_maps = make_in_maps(im_set, s_seq, im_len, s_len)
    last = None
    for attempt in range(3):
        try:
            res = run_bass_kernel_spmd(nc, in_maps, core_ids=list(range(NCORES)))
            return np.asarray(
                res.results[0]["out"], dtype=np.float32
            ).reshape(())[()]
        except Exception as e:  # transient device-unrecoverable happens
            last = e
            time.sleep(30 * (attempt + 1))
    raise last


# revision 41
# speedup vs baseline: 1.6966x; 1.0093x over previous
"""Trainium2 Bass kernel for AlignmentContrastiveLoss (8 NeuronCores, SPMD).

Reference semantics:
  im = im_set[:, 1:, :]           [256, 36, 1024]
  s  = s_seq[:, 1:-2, :]          [256, 32, 1024]
  align[i,j,n,m] = im[i,n] . s[j,m], zeroed where n >= im_len[i]-1 or m >= s_len[j]-3
  scores[i,j] = sum_m max_n align[i,j,n,m]
  loss = sum_i relu(M + max_{j!=i} scores[i,j] - scores[i,i])
       + sum_j relu(M + max_{i!=j} scores[i,j] - scores[j,j])

Sharding: data-parallel over images (32 per core), s replicated.

v2 design:
  - f32 tokens are cast once to fp8e4 (im-mask fused as activation scale),
    bitcast to 16-bit fp8-pairs and transposed via the DMA XBAR (pure bit
    movement) into a packed layout: partition p of q-chunk q holds the d
    pair (256q+2p, 256q+2p+1) interleaved per token (HW-verified mapping).
  - s feeds the PE as RAW packed bytes via MatmulPerfMode.DoubleRowSwInterleave
    (stationary side accepts interleaved pairs; output partitions come out
    token-REVERSED, verified on HW). im (small) is deinterleaved to planar
    [128, q, 2, tok] fp8 by one gpsimd 4D copy per tile, which also compacts
    away the XBAR pad columns. No PE transposes, no PSUM->SBUF staging copies.
  - The s-token reversal is compensated in the word-sum weights (w4 built
    from a reversed word mask), so scoresT and the loss tail are unchanged.
  - wt-outer loop: per s-tile, 3 region-chunk matmul groups + vector MAX
    reduce; word-sum is a tiny PE matmul against s-mask-weighted block-ones
    (applies the s word mask for free and directly yields scoresT[img,sent]).
  - im staged as 11 tiles of 3 images (112-partition windows, 4-col overlap
    into the next tile's range which is later overwritten with real data).
  - Cross-core traffic: one 520-float AllGather of per-core column-max
    partials + scattered diagonals + local cost_s sum.
"""

import numpy as np

MARGIN = 0.2
B = 256          # global batch (images == sentences)
NCORES = 8
BI = B // NCORES  # images per core = 32
NREG = 36        # regions per image after stripping
NWORD = 32       # words per sentence after stripping
D = 1024
IM_TOK = BI * NREG      # 1152 dense im tokens
S_TOK = B * NWORD       # 8192 s tokens
S_TILES = S_TOK // 128  # 64
BIG = 1.0e30
# region chunks for the main matmul: (token offset, ntok, nimg)
RCHUNKS = [(0, 432, 12), (432, 432, 12), (864, 288, 8)]
# im staging tiles: (dense token offset, window (mult of 16), first image, n images)
IM_STAGE = [(108 * t, 112, 3 * t, 3) for t in range(10)] + [(1080, 112, 30, 2)]
IM_TP_COLS = 112 * 11  # padded XBAR destination: disjoint 112-col windows
BLK = 640  # allgather block floats: [p-major x5] colmax x2 | dscat x2 | cost_s


def fix_multiwaits(nc, mybir):
    """This toolchain's walrus accepts 1 wait per instruction (2 for
    EventSemaphore); Tile can emit more. Offload surplus waits onto
    inserted same-engine NoOps placed immediately before the instruction."""
    n_fix = 0
    for fn in nc.m.functions:
        for blk in fn.blocks:
            insts = blk.instructions
            i = 0
            while i < len(insts):
                inst = insts[i]
                si = inst.sync_info
                waits = list(si.on_wait) if si is not None and si.on_wait else []
                cap = 2 if isinstance(inst, mybir.InstEventSemaphore) else 1
                if len(waits) > cap:
                    surplus, keep = waits[:-cap], waits[-cap:]
                    si.on_wait = keep
                    for w in surplus:
                        nop = mybir.InstNoOp(
                            name=f"{inst.name}_wsplit{n_fix}",
                            engine=inst.engine,
                            ins=[],
                            outs=[],
                            sync_info=mybir.SyncInfo(on_wait=[w], on_update=[]),
                        )
                        insts.insert(i, nop)
                        n_fix += 1
                        i += 1
                i += 1
    return n_fix


DEFAULT_OPTS = {
    "sf_bufs": 18,     # f32 staging tiles
    "pk_bufs": 6,      # packed fp8-as-bf16 staging quad tiles
    "alp_bufs": 7,     # PSUM align buffers
    "gpsimd_cast": 0,  # every Nth s cast on vector (0 = all scalar)
    "s_prefetch": 3,   # s quad-loads dispatched ahead of their pack stage
    "im_head": 4,      # im tiles staged before the first s tile
}


def build_graph(opts=None):
    import concourse.bass as bass
    import concourse.mybir as mybir
    import concourse.tile as tile
    from concourse.masks import make_identity
    from contextlib import ExitStack

    opts = {**DEFAULT_OPTS, **(opts or {})}

    f32 = mybir.dt.float32
    bf16 = mybir.dt.bfloat16
    fp8 = mybir.dt.float8e4
    i32 = mybir.dt.int32
    ALU = mybir.AluOpType
    AX = mybir.AxisListType
    ACTF = mybir.ActivationFunctionType
    SWI = mybir.MatmulPerfMode.DoubleRowSwInterleave

    nc = bass.Bass()

    im_ext = nc.declare_dram_parameter("im_set", [BI, 37, D], f32, isOutput=False)
    s_ext = nc.declare_dram_parameter("s_seq", [B, 35, D], f32, isOutput=False)
    imlen_ext = nc.declare_dram_parameter("im_len", [BI], i32, isOutput=False)
    slen_ext = nc.declare_dram_parameter("s_len", [B], i32, isOutput=False)
    dmask_ext = nc.declare_dram_parameter("diag_mask", [B, BI], f32, isOutput=False)
    dmaskT_ext = nc.declare_dram_parameter("diag_maskT", [BI, B], f32, isOutput=False)
    out_ext = nc.declare_dram_parameter("out", [1], f32, isOutput=True)
    if opts.get("debug"):
        dbg_sel = nc.declare_dram_parameter("dbg_sel", [128, 128], f32, isOutput=True)
        dbg_pm = nc.declare_dram_parameter("dbg_pm", [128, 128], f32, isOutput=True)
        dbg_rb = nc.declare_dram_parameter("dbg_rb", [128, 128], f32, isOutput=True)
        dbg_tc = nc.declare_dram_parameter("dbg_tc", [128, 2], f32, isOutput=True)
        dbg_bm = nc.declare_dram_parameter("dbg_bm", [2, 128, 128], f32, isOutput=True)
        dbg_w4 = nc.declare_dram_parameter("dbg_w4", [128, S_TILES, 4], f32, isOutput=True)
        dbg_scT = nc.declare_dram_parameter("dbg_scT", [BI, S_TILES, 4], f32, isOutput=True)
        dbg_mx = nc.declare_dram_parameter("dbg_mx", [128, S_TILES, BI], f32, isOutput=True)

    with tile.TileContext(nc) as tc, ExitStack() as top:
        # ---------------- constants ----------------
        const = top.enter_context(tc.tile_pool(name="const", bufs=1))
        ident_f32 = const.tile([128, 128], f32)
        make_identity(nc, ident_f32)
        ones32 = const.tile([32, 1], f32)
        nc.gpsimd.memset(ones32, 1.0)
        ones128 = const.tile([128, 1], f32)
        nc.gpsimd.memset(ones128, 1.0)

        # ---------------- token masks (device-side) ----------------
        mpool = top.enter_context(tc.tile_pool(name="masks", bufs=1))
        dram = top.enter_context(tc.tile_pool(name="dram", bufs=1, space="DRAM"))

        # per-image region mask [BI, NREG]: n < im_len-1
        imlen_sb = mpool.tile([BI, 1], i32)
        nc.gpsimd.dma_start(imlen_sb, imlen_ext.rearrange("(p o) -> p o", o=1))
        il_sb = mpool.tile([BI, 1], f32)
        nc.gpsimd.tensor_scalar(il_sb, imlen_sb, 1, None, op0=ALU.subtract)
        iota_r = mpool.tile([BI, NREG], f32)
        nc.gpsimd.iota(
            iota_r, pattern=[[1, NREG]], base=0, channel_multiplier=0,
            allow_small_or_imprecise_dtypes=True,
        )
        mask_im = mpool.tile([BI, NREG], f32)
        nc.gpsimd.tensor_scalar(mask_im, iota_r, il_sb, None, op0=ALU.is_lt)
        # maskcol_im [128, 11]: per (partition, im stage tile) in REGION-major
        # stage order (p = ni*n + i); pad rows -> 0
        mask_im_dram = dram.tile([BI * NREG], f32)
        nc.gpsimd.dma_start(
            mask_im_dram.rearrange("(i n) -> i n", n=NREG), mask_im
        )
        maskcol_im = mpool.tile([128, len(IM_STAGE)], f32)
        nc.gpsimd.memset(maskcol_im, 0.0)
        for t, (toff, win, i0, ni) in enumerate(IM_STAGE):
            nc.gpsimd.dma_start(
                maskcol_im[0:36 * ni, t:t + 1],
                mask_im_dram[36 * i0:36 * (i0 + ni)].rearrange(
                    "(i n) -> n i", n=NREG
                ),
            )

        # s word masks -> w4 block-ones weights [128, 64, 4] bf16:
        #   w4[32*jj + m, wt, jj] = (m < s_len[4*wt+jj] - 3)
        slen_sb = mpool.tile([128, 2], i32)
        nc.gpsimd.dma_start(slen_sb, slen_ext.rearrange("(t p) -> p t", p=128))
        sl_sb = mpool.tile([128, 2], f32)
        nc.gpsimd.tensor_scalar(sl_sb, slen_sb, 3, None, op0=ALU.subtract)
        iota_w = mpool.tile([128, NWORD], f32)
        nc.gpsimd.iota(
            iota_w, pattern=[[1, NWORD]], base=0, channel_multiplier=0,
            allow_small_or_imprecise_dtypes=True,
        )
        # Word-sum weights for WORD-major stage order (p = 4w + j) combined
        # with the SwInterleave token reversal (partition p <-> raw col 127-p):
        #   w4[p, wt, jj] = [ (127-p)%4 == jj ] * ( (127-p)//4 < sl[4*wt+jj] )
        # Built transposed (partition c = sentence-within-half, free p) then
        # PE-transposed into place.
        rb = mpool.tile([128, 128], f32)     # rb[c, p] = (127-p)//4
        nc.gpsimd.iota(rb, pattern=[[-1, 32], [0, 4]], base=31,
                       channel_multiplier=0, allow_small_or_imprecise_dtypes=True)
        # sel[c, p] = (p%4 == 3 - c%4)  <=>  ((c + p + 1) & 3 == 0)
        cp_i = mpool.tile([128, 128], i32)
        nc.gpsimd.iota(cp_i, pattern=[[1, 128]], base=1, channel_multiplier=1)
        cp_a = mpool.tile([128, 128], i32)
        nc.vector.tensor_scalar(cp_a, cp_i, 3, None, op0=ALU.bitwise_and)
        sel = mpool.tile([128, 128], f32)
        nc.vector.tensor_scalar(sel, cp_a, 0, None, op0=ALU.is_equal)
        w4 = mpool.tile([128, S_TILES, 4], bf16)
        with tc.tile_pool(name="w4ps", bufs=2, space="PSUM") as wps:
            for h in range(2):
                bh = mpool.tile([128, 128], f32, tag=f"w4bh{h}")
                nc.vector.tensor_scalar(
                    bh, rb, sl_sb[:, h:h + 1], None, op0=ALU.is_lt
                )
                bm = mpool.tile([128, 128], f32, tag=f"w4bm{h}")
                nc.vector.tensor_mul(bm, bh, sel)
                wt_ps = wps.tile([128, 128], f32, tag=f"w4t{h}")
                nc.tensor.transpose(wt_ps, bm, ident_f32)
                nc.vector.tensor_copy(
                    w4[:, 32 * h:32 * (h + 1), :].rearrange("p a b -> p (a b)"),
                    wt_ps,
                )
                if opts.get("debug"):
                    nc.sync.dma_start(dbg_bm[h, :, :], bm)
        if opts.get("debug"):
            nc.sync.dma_start(dbg_sel[:, :], sel)
            nc.sync.dma_start(dbg_rb[:, :], rb)

        # diag masks (sharding metadata inputs)
        dmask_sb = mpool.tile([128, 2, BI], f32)
        nc.gpsimd.dma_start(dmask_sb, dmask_ext.rearrange("(t p) i -> p t i", p=128))
        dmaskT_sb = mpool.tile([BI, 2, 128], f32)
        nc.gpsimd.dma_start(
            dmaskT_sb, dmaskT_ext.rearrange("p (t f) -> p t f", f=128)
        )

        # ---------------- persistent big buffers ----------------
        big = top.enter_context(tc.tile_pool(name="big", bufs=1))
        # packed-transposed fp8 pairs, stored as bf16 bit containers.
        # One tile per staging unit so the (whole-tile) dependency tracking
        # of the XBAR DMA writes stays exactly per-tile.
        IMQ = [[0, 1, 2, 3], [4, 5, 6, 7], [8, 9], [10]]
        imTq_g = [
            big.tile([128, 4 * len(ts), 112], bf16, name=f"imTq{g}")
            for g, ts in enumerate(IMQ)
        ]
        imP = big.tile([128, 4, 2, IM_TOK], fp8)      # dense planar im
        sTp_g = [
            big.tile([128, 4, 4, 128], bf16, name=f"sTpg{g}")
            for g in range(S_TILES // 4)
        ]
        maxima = big.tile([128, S_TILES, BI], bf16)  # per (word, wtile, img) region-max
        scoresT_sb = big.tile([BI, S_TILES, 4], f32)  # [img, wt, jj] == [img, sent]
        scores_sb = big.tile([128, 2, BI], f32)       # [sent%128, sent//128, img]

        # fp8 views: im pair-split for the deinterleave, s raw for SwInterleave
        imTq8_g = [
            t.bitcast(fp8).rearrange("p m (t b) -> p m b t", b=2) for t in imTq_g
        ]
        sTraw_g = [t.bitcast(fp8) for t in sTp_g]     # each [128, 4, 4, 256]

        with ExitStack() as mid:
            stage = mid.enter_context(
                tc.tile_pool(name="stage", bufs=opts["sf_bufs"])
            )
            pk = mid.enter_context(tc.tile_pool(name="pk", bufs=opts["pk_bufs"]))
            alp = mid.enter_context(
                tc.tile_pool(name="alp", bufs=opts["alp_bufs"], space="PSUM")
            )
            scp = mid.enter_context(tc.tile_pool(name="scp", bufs=1, space="PSUM"))
            scoresT_ps = scp.tile([BI, S_TILES, 4], f32)

            def stage_im_quad(g):
                ts = IMQ[g]
                win_g = IM_STAGE[ts[-1]][1]
                tf32s = []
                for t in ts:
                    toff, win, i0, ni = IM_STAGE[t]
                    nreal = 36 * ni
                    tf32 = stage.tile([128, D], f32, tag="sf32")
                    nc.sync.dma_start(
                        tf32[0:nreal, :],
                        im_ext[i0:i0 + ni, 1:1 + NREG, :].rearrange(
                            "i n d -> n i d"
                        ),
                    )
                    tf32s.append(tf32)
                ipkq = pk.tile([128, len(ts), D // 2], bf16, tag="spk")
                for k, t in enumerate(ts):
                    toff, win, i0, ni = IM_STAGE[t]
                    # masked cast on the (ramp-idle) vector engine
                    nc.vector.tensor_scalar(
                        ipkq[:, k, :].bitcast(fp8)[0:win, :], tf32s[k][0:win, :],
                        maskcol_im[0:win, t:t + 1], None, op0=ALU.mult,
                    )
                # one XBAR DMA for the whole quad (scalar queue: keeps the
                # sync queue free to prefetch s loads without blocking)
                nc.scalar.dma_start(
                    imTq_g[g][:, :, 0:win_g],
                    ipkq[0:win_g, :, :].rearrange("p a c -> p (a c)"),
                    transpose=True,
                )
            def im_deints(g):
                # deinterleave + compact + un-permute (region-major stage
                # order ni*n + i -> dense 36*i + n), one strided copy per tile
                for k, t in enumerate(IMQ[g]):
                    toff, win, i0, ni = IM_STAGE[t]
                    nreal = 36 * ni
                    nc.vector.tensor_copy(
                        imP[:, :, :, toff:toff + nreal].rearrange(
                            "p q b (i n) -> p q b i n", n=NREG
                        ),
                        imTq8_g[g][:, 4 * k:4 * k + 4, :, 0:nreal].rearrange(
                            "p q b (n i) -> p q b n i", i=ni
                        ).rearrange("p q b n i -> p q b i n"),
                    )

            def s_load(gq):
                # four per-tile loads (the DMA AP balancer caps at 3 dims,
                # so a quad can't be one DMA), word-major per tile
                tfs = []
                for a in range(4):
                    i = 4 * gq + a
                    tf32 = stage.tile([128, D], f32, tag="sf32")
                    nc.sync.dma_start(
                        tf32,
                        s_ext[4 * i:4 * i + 4, 1:1 + NWORD, :].rearrange(
                            "j w d -> w j d"
                        ),
                    )
                    tfs.append(tf32)
                return tfs

            def s_pack(gq, tfs):
                pkq = pk.tile([128, 4, D // 2], bf16, tag="spk")
                gsp = opts["gpsimd_cast"]
                for a in range(4):
                    i = 4 * gq + a
                    if gsp and (i % gsp == gsp - 1):
                        nc.vector.tensor_copy(
                            pkq[:, a, :].bitcast(fp8), tfs[a]
                        )
                    else:
                        nc.scalar.activation(
                            pkq[:, a, :].bitcast(fp8), tfs[a], ACTF.Copy
                        )
                nc.sync.dma_start(
                    sTp_g[gq].rearrange("p a q t -> p (a q) t"),
                    pkq.rearrange("p a c -> p (a c)"),
                    transpose=True,
                )

            # im head first (needed by rc0); s quad-loads run ahead of their
            # cast+transpose stages so no dispatch queue ever blocks
            # s staging starts immediately (its chain is mask-free); the im
            # quads stage in parallel behind the first three s quads so the
            # mask->cast->transpose chain never gates the s pipeline
            PRE = opts["s_prefetch"]
            HEADQ = 3
            pending = []
            for gq in range(HEADQ):
                pending.append((gq, s_load(gq)))
            for _ in range(HEADQ):
                s_pack(*pending.pop(0))
            for g in range(len(IMQ)):
                stage_im_quad(g)
            for g in range(len(IMQ)):
                im_deints(g)
            for gq in range(HEADQ, S_TILES // 4):
                pending.append((gq, s_load(gq)))
                if len(pending) > PRE:
                    s_pack(*pending.pop(0))
            for it in pending:
                s_pack(*it)

            # ---------------- main matmul + region-max + word-sum ----------------
            def word_sum(wt):
                # scoresT[img, 4wt+jj] = sum_m maxima[(jj,m), wt, img] * wmask
                nc.tensor.matmul(
                    scoresT_ps[:, wt, :],
                    lhsT=maxima[:, wt, :],
                    rhs=w4[:, wt, :],
                    start=True, stop=True,
                )

            for wt in range(S_TILES):
                for rci, (toff, ntok, nimg) in enumerate(RCHUNKS):
                    pal = alp.tile([128, nimg, NREG], f32, tag="align")
                    for q in range(4):
                        nc.tensor.matmul(
                            pal.rearrange("p a b -> p (a b)"),
                            lhsT=sTraw_g[wt // 4][:, wt % 4, q, :],
                            rhs=imP[:, q, :, toff:toff + ntok],
                            start=(q == 0),
                            stop=(q == 3),
                            perf_mode=SWI,
                        )
                    nc.vector.tensor_reduce(
                        maxima[:, wt, toff // NREG:toff // NREG + nimg],
                        pal, axis=AX.X, op=ALU.max,
                    )
                    # emit the previous tile's word-sum between rc chunks so the
                    # PE never waits on the vector MAX of the current tile
                    if rci == 0 and wt > 0:
                        word_sum(wt - 1)
            word_sum(S_TILES - 1)

            # scoresT -> SBUF (transposes to [sent, img] happen in the tail
            # scope so scp stays a single PSUM bank and alp gets one more)
            nc.vector.tensor_copy(scoresT_sb, scoresT_ps)
            if opts.get("debug"):
                w4d = mpool.tile([128, S_TILES, 4], f32)
                nc.vector.tensor_copy(w4d, w4)
                nc.sync.dma_start(dbg_w4[:, :, :], w4d)
                nc.sync.dma_start(dbg_scT[:, :, :], scoresT_sb)
                mxd = mpool.tile([128, S_TILES, BI], f32)
                nc.vector.tensor_copy(mxd, maxima)
                nc.sync.dma_start(dbg_mx[:, :, :], mxd)

        # ---------------- loss tail ----------------
        with ExitStack() as tail:
            tp = tail.enter_context(tc.tile_pool(name="tailp", bufs=1, space="PSUM"))
            ts = tail.enter_context(tc.tile_pool(name="tails", bufs=1))

            sc_ps = tp.tile([128, 2, BI], f32)
            for t in range(2):
                nc.tensor.transpose(
                    sc_ps[:, t, :],
                    scoresT_sb[:, 32 * t:32 * (t + 1), :].rearrange(
                        "p a b -> p (a b)"
                    ),
                    ident_f32[:BI, :BI],
                )
                nc.vector.tensor_copy(scores_sb[:, t, :], sc_ps[:, t, :])

            # col-max over local images (diag excluded) + scattered diag
            masked = ts.tile([128, 2, BI], f32)
            nc.vector.scalar_tensor_tensor(
                masked, dmask_sb, -BIG, scores_sb, op0=ALU.mult, op1=ALU.add
            )
            colmax_p = ts.tile([128, 2], f32)
            nc.vector.tensor_reduce(colmax_p, masked, axis=AX.X, op=ALU.max)
            dtmp = ts.tile([128, 2, BI], f32)
            nc.vector.tensor_mul(dtmp, dmask_sb, scores_sb)
            dscat = ts.tile([128, 2], f32)
            nc.vector.tensor_reduce(dscat, dtmp, axis=AX.X, op=ALU.add)

            # row-max over sentences (diag excluded); scoresT_sb is [img, sent]
            scT_flat = scoresT_sb.rearrange("p a b -> p (a b)")
            dmaskT_flat = dmaskT_sb.rearrange("p a b -> p (a b)")
            maskedT = ts.tile([BI, B], f32)
            nc.vector.scalar_tensor_tensor(
                maskedT, dmaskT_flat, -BIG, scT_flat, op0=ALU.mult, op1=ALU.add
            )
            rowmax = ts.tile([BI, 1], f32)
            nc.vector.tensor_reduce(rowmax, maskedT, axis=AX.X, op=ALU.max)
            dT_tmp = ts.tile([BI, B], f32)
            nc.vector.tensor_mul(dT_tmp, dmaskT_flat, scT_flat)
            d_row = ts.tile([BI, 1], f32)
            nc.vector.tensor_reduce(d_row, dT_tmp, axis=AX.X, op=ALU.add)

            cost_s = ts.tile([BI, 1], f32)
            nc.vector.tensor_sub(cost_s, rowmax, d_row)
            nc.vector.tensor_scalar(
                cost_s, cost_s, MARGIN, 0.0, op0=ALU.add, op1=ALU.max
            )
            cs_ps = tp.tile([1, 1], f32)
            nc.tensor.matmul(cs_ps, lhsT=ones32, rhs=cost_s, start=True, stop=True)

            # one packed tile: [:,0:2]=colmax | [:,2:4]=dscat | [0,4]=cost_s
            pkt = ts.tile([128, 5], f32)
            nc.vector.tensor_copy(pkt[:, 0:2], colmax_p)
            nc.vector.tensor_copy(pkt[:, 2:4], dscat)
            nc.vector.tensor_copy(pkt[0:1, 4:5], cs_ps)
            blk = dram.tile([BLK], f32)
            nc.sync.dma_start(
                blk[0:640].rearrange("(a p) -> p a", p=128), pkt
            )
            gath = dram.tile([NCORES, BLK], f32, addr_space="Shared")
            nc.gpsimd.collective_compute(
                "AllGather",
                ALU.bypass,
                ins=[blk.opt()],
                outs=[gath.opt()],
                replica_groups=[list(range(NCORES))],
            )

            # redundant final reduction on every core; single unpack DMA
            # per-piece transposing unpacks (DMA APs cap at 3 dims with a
            # contiguous final dim), spread across both dispatch queues
            g5 = ts.tile([128, 5, NCORES], f32)
            for a in range(5):
                eng = nc.sync if a % 2 == 0 else nc.scalar
                eng.dma_start(
                    g5[:, a, :],
                    gath[:, 128 * a:128 * (a + 1)].rearrange("c p -> p c"),
                )
            colmax_g = ts.tile([128, 2], f32)
            nc.vector.tensor_reduce(colmax_g, g5[:, 0:2, :], axis=AX.X, op=ALU.max)
            d_all = ts.tile([128, 2], f32)
            nc.vector.tensor_reduce(d_all, g5[:, 2:4, :], axis=AX.X, op=ALU.add)
            cim = ts.tile([128, 2], f32)
            nc.vector.tensor_sub(cim, colmax_g, d_all)
            nc.vector.tensor_scalar(cim, cim, MARGIN, 0.0, op0=ALU.add, op1=ALU.max)
            cim_r = ts.tile([128, 1], f32)
            nc.vector.tensor_reduce(cim_r, cim, axis=AX.X, op=ALU.add)
            cs_tot = ts.tile([1, 1], f32)
            nc.vector.tensor_reduce(cs_tot, g5[0:1, 4, :], axis=AX.X, op=ALU.add)
            nc.vector.tensor_add(cim_r[0:1, :], cim_r[0:1, :], cs_tot)
            tot_ps = tp.tile([1, 1], f32)
            nc.tensor.matmul(tot_ps, lhsT=ones128, rhs=cim_r, start=True, stop=True)
            total = ts.tile([1, 1], f32)
            nc.vector.tensor_copy(total, tot_ps)
            nc.sync.dma_start(out_ext[0:1], total[0, :])

    fix_multiwaits(nc, mybir)
    return nc


_CACHE = {}


def _get_nc():
    if "nc" not in _CACHE:
        _CACHE["nc"] = build_graph()
    return _CACHE["nc"]


def make_in_maps(im_set, s_seq, im_len, s_len):
    im_set = np.ascontiguousarray(im_set, dtype=np.float32)
    s_seq = np.ascontiguousarray(s_seq, dtype=np.float32)
    im_len = np.ascontiguousarray(im_len, dtype=np.int32)
    s_len = np.ascontiguousarray(s_len, dtype=np.int32)
    in_maps = []
    for c in range(NCORES):
        dm = np.zeros((B, BI), dtype=np.float32)
        for i in range(BI):
            dm[BI * c + i, i] = 1.0
        in_maps.append({
            "im_set": im_set[BI * c:BI * (c + 1)],
            "s_seq": s_seq,
            "im_len": im_len[BI * c:BI * (c + 1)],
            "s_len": s_len,
            "diag_mask": dm,
            "diag_maskT": np.ascontiguousarray(dm.T),
        })
    return in_maps


def kernel(im_set, s_seq, im_len, s_len):
    import time
    from concourse.bass_utils import run_bass_kernel_spmd

    nc = _get_nc()
    in_maps = make_in_maps(im_set, s_seq, im_len, s_len)
    last = None
    for attempt in range(3):
        try:
            res = run_bass_kernel_spmd(nc, in_maps, core_ids=list(range(NCORES)))
            return np.asarray(
                res.results[0]["out"], dtype=np.float32
            ).reshape(())[()]
        except Exception as e:  # transient device-unrecoverable happens
            last = e
            time.sleep(30 * (attempt + 1))
    raise last


# revision 42
# speedup vs baseline: 1.7754x; 1.0464x over previous
"""Trainium2 Bass kernel for AlignmentContrastiveLoss (8 NeuronCores, SPMD).

Reference semantics:
  im = im_set[:, 1:, :]           [256, 36, 1024]
  s  = s_seq[:, 1:-2, :]          [256, 32, 1024]
  align[i,j,n,m] = im[i,n] . s[j,m], zeroed where n >= im_len[i]-1 or m >= s_len[j]-3
  scores[i,j] = sum_m max_n align[i,j,n,m]
  loss = sum_i relu(M + max_{j!=i} scores[i,j] - scores[i,i])
       + sum_j relu(M + max_{i!=j} scores[i,j] - scores[j,j])

Sharding: data-parallel over images (32 per core), s replicated.

v2 design:
  - f32 tokens are cast once to fp8e4 (im-mask fused as activation scale),
    bitcast to 16-bit fp8-pairs and transposed via the DMA XBAR (pure bit
    movement) into a packed layout: partition p of q-chunk q holds the d
    pair (256q+2p, 256q+2p+1) interleaved per token (HW-verified mapping).
  - s feeds the PE as RAW packed bytes via MatmulPerfMode.DoubleRowSwInterleave
    (stationary side accepts interleaved pairs; output partitions come out
    token-REVERSED, verified on HW). im (small) is deinterleaved to planar
    [128, q, 2, tok] fp8 by one gpsimd 4D copy per tile, which also compacts
    away the XBAR pad columns. No PE transposes, no PSUM->SBUF staging copies.
  - The s-token reversal is compensated in the word-sum weights (w4 built
    from a reversed word mask), so scoresT and the loss tail are unchanged.
  - wt-outer loop: per s-tile, 3 region-chunk matmul groups + vector MAX
    reduce; word-sum is a tiny PE matmul against s-mask-weighted block-ones
    (applies the s word mask for free and directly yields scoresT[img,sent]).
  - im staged as 11 tiles of 3 images (112-partition windows, 4-col overlap
    into the next tile's range which is later overwritten with real data).
  - Cross-core traffic: one 520-float AllGather of per-core column-max
    partials + scattered diagonals + local cost_s sum.
"""

import numpy as np

MARGIN = 0.2
B = 256          # global batch (images == sentences)
NCORES = 8
BI = B // NCORES  # images per core = 32
NREG = 36        # regions per image after stripping
NWORD = 32       # words per sentence after stripping
D = 1024
IM_TOK = BI * NREG      # 1152 dense im tokens
S_TOK = B * NWORD       # 8192 s tokens
S_TILES = S_TOK // 128  # 64
BIG = 1.0e30
# region chunks for the main matmul: (token offset, ntok, nimg)
RCHUNKS = [(0, 432, 12), (432, 432, 12), (864, 288, 8)]
# im staging tiles: (dense token offset, window (mult of 16), first image, n images)
IM_STAGE = [(108 * t, 112, 3 * t, 3) for t in range(10)] + [(1080, 112, 30, 2)]
IM_TP_COLS = 112 * 11  # padded XBAR destination: disjoint 112-col windows
BLK = 640  # allgather block floats: [p-major x5] colmax x2 | dscat x2 | cost_s


def fix_multiwaits(nc, mybir):
    """This toolchain's walrus accepts 1 wait per instruction (2 for
    EventSemaphore); Tile can emit more. Offload surplus waits onto
    inserted same-engine NoOps placed immediately before the instruction."""
    n_fix = 0
    for fn in nc.m.functions:
        for blk in fn.blocks:
            insts = blk.instructions
            i = 0
            while i < len(insts):
                inst = insts[i]
                si = inst.sync_info
                waits = list(si.on_wait) if si is not None and si.on_wait else []
                cap = 2 if isinstance(inst, mybir.InstEventSemaphore) else 1
                if len(waits) > cap:
                    surplus, keep = waits[:-cap], waits[-cap:]
                    si.on_wait = keep
                    for w in surplus:
                        nop = mybir.InstNoOp(
                            name=f"{inst.name}_wsplit{n_fix}",
                            engine=inst.engine,
                            ins=[],
                            outs=[],
                            sync_info=mybir.SyncInfo(on_wait=[w], on_update=[]),
                        )
                        insts.insert(i, nop)
                        n_fix += 1
                        i += 1
                i += 1
    return n_fix


DEFAULT_OPTS = {
    "sf_bufs": 18,     # f32 staging tiles
    "pk_bufs": 8,      # packed fp8-as-bf16 staging quad tiles
    "alp_bufs": 7,     # PSUM align buffers
    "gpsimd_cast": 2,  # every Nth s cast on vector (0 = all scalar)
    "s_prefetch": 3,   # s quad-loads dispatched ahead of their pack stage
    "im_head": 4,      # im tiles staged before the first s tile
}


def build_graph(opts=None):
    import concourse.bass as bass
    import concourse.mybir as mybir
    import concourse.tile as tile
    from concourse.masks import make_identity
    from contextlib import ExitStack

    opts = {**DEFAULT_OPTS, **(opts or {})}

    f32 = mybir.dt.float32
    bf16 = mybir.dt.bfloat16
    fp8 = mybir.dt.float8e4
    i32 = mybir.dt.int32
    ALU = mybir.AluOpType
    AX = mybir.AxisListType
    ACTF = mybir.ActivationFunctionType
    SWI = mybir.MatmulPerfMode.DoubleRowSwInterleave

    nc = bass.Bass()

    im_ext = nc.declare_dram_parameter("im_set", [BI, 37, D], f32, isOutput=False)
    s_ext = nc.declare_dram_parameter("s_seq", [B, 35, D], f32, isOutput=False)
    imlen_ext = nc.declare_dram_parameter("im_len", [BI], i32, isOutput=False)
    slen_ext = nc.declare_dram_parameter("s_len", [B], i32, isOutput=False)
    dmask_ext = nc.declare_dram_parameter("diag_mask", [B, BI], f32, isOutput=False)
    dmaskT_ext = nc.declare_dram_parameter("diag_maskT", [BI, B], f32, isOutput=False)
    out_ext = nc.declare_dram_parameter("out", [1], f32, isOutput=True)
    if opts.get("debug"):
        dbg_sel = nc.declare_dram_parameter("dbg_sel", [128, 128], f32, isOutput=True)
        dbg_pm = nc.declare_dram_parameter("dbg_pm", [128, 128], f32, isOutput=True)
        dbg_rb = nc.declare_dram_parameter("dbg_rb", [128, 128], f32, isOutput=True)
        dbg_tc = nc.declare_dram_parameter("dbg_tc", [128, 2], f32, isOutput=True)
        dbg_bm = nc.declare_dram_parameter("dbg_bm", [2, 128, 128], f32, isOutput=True)
        dbg_w4 = nc.declare_dram_parameter("dbg_w4", [128, S_TILES, 4], f32, isOutput=True)
        dbg_scT = nc.declare_dram_parameter("dbg_scT", [BI, S_TILES, 4], f32, isOutput=True)
        dbg_mx = nc.declare_dram_parameter("dbg_mx", [128, S_TILES, BI], f32, isOutput=True)

    with tile.TileContext(nc) as tc, ExitStack() as top:
        # ---------------- constants ----------------
        const = top.enter_context(tc.tile_pool(name="const", bufs=1))
        ident_f32 = const.tile([128, 128], f32)
        make_identity(nc, ident_f32)
        ones32 = const.tile([32, 1], f32)
        nc.gpsimd.memset(ones32, 1.0)
        ones128 = const.tile([128, 1], f32)
        nc.gpsimd.memset(ones128, 1.0)

        # ---------------- token masks (device-side) ----------------
        mpool = top.enter_context(tc.tile_pool(name="masks", bufs=1))
        dram = top.enter_context(tc.tile_pool(name="dram", bufs=1, space="DRAM"))

        # per-image region mask [BI, NREG]: n < im_len-1
        imlen_sb = mpool.tile([BI, 1], i32)
        nc.gpsimd.dma_start(imlen_sb, imlen_ext.rearrange("(p o) -> p o", o=1))
        il_sb = mpool.tile([BI, 1], f32)
        nc.gpsimd.tensor_scalar(il_sb, imlen_sb, 1, None, op0=ALU.subtract)
        iota_r = mpool.tile([BI, NREG], f32)
        nc.gpsimd.iota(
            iota_r, pattern=[[1, NREG]], base=0, channel_multiplier=0,
            allow_small_or_imprecise_dtypes=True,
        )
        mask_im = mpool.tile([BI, NREG], f32)
        nc.gpsimd.tensor_scalar(mask_im, iota_r, il_sb, None, op0=ALU.is_lt)
        # maskcol_im [128, 11]: per (partition, im stage tile) in REGION-major
        # stage order (p = ni*n + i); pad rows -> 0
        mask_im_dram = dram.tile([BI * NREG], f32)
        nc.gpsimd.dma_start(
            mask_im_dram.rearrange("(i n) -> i n", n=NREG), mask_im
        )
        maskcol_im = mpool.tile([128, len(IM_STAGE)], f32)
        nc.gpsimd.memset(maskcol_im, 0.0)
        for t, (toff, win, i0, ni) in enumerate(IM_STAGE):
            nc.gpsimd.dma_start(
                maskcol_im[0:36 * ni, t:t + 1],
                mask_im_dram[36 * i0:36 * (i0 + ni)].rearrange(
                    "(i n) -> n i", n=NREG
                ),
            )

        # s word masks -> w4 block-ones weights [128, 64, 4] bf16:
        #   w4[32*jj + m, wt, jj] = (m < s_len[4*wt+jj] - 3)
        slen_sb = mpool.tile([128, 2], i32)
        nc.gpsimd.dma_start(slen_sb, slen_ext.rearrange("(t p) -> p t", p=128))
        sl_sb = mpool.tile([128, 2], f32)
        nc.gpsimd.tensor_scalar(sl_sb, slen_sb, 3, None, op0=ALU.subtract)
        iota_w = mpool.tile([128, NWORD], f32)
        nc.gpsimd.iota(
            iota_w, pattern=[[1, NWORD]], base=0, channel_multiplier=0,
            allow_small_or_imprecise_dtypes=True,
        )
        # Word-sum weights for WORD-major stage order (p = 4w + j) combined
        # with the SwInterleave token reversal (partition p <-> raw col 127-p):
        #   w4[p, wt, jj] = [ (127-p)%4 == jj ] * ( (127-p)//4 < sl[4*wt+jj] )
        # Built transposed (partition c = sentence-within-half, free p) then
        # PE-transposed into place.
        rb = mpool.tile([128, 128], f32)     # rb[c, p] = (127-p)//4
        nc.gpsimd.iota(rb, pattern=[[-1, 32], [0, 4]], base=31,
                       channel_multiplier=0, allow_small_or_imprecise_dtypes=True)
        # sel[c, p] = (p%4 == 3 - c%4)  <=>  ((c + p + 1) & 3 == 0)
        cp_i = mpool.tile([128, 128], i32)
        nc.gpsimd.iota(cp_i, pattern=[[1, 128]], base=1, channel_multiplier=1)
        cp_a = mpool.tile([128, 128], i32)
        nc.vector.tensor_scalar(cp_a, cp_i, 3, None, op0=ALU.bitwise_and)
        sel = mpool.tile([128, 128], f32)
        nc.vector.tensor_scalar(sel, cp_a, 0, None, op0=ALU.is_equal)
        w4 = mpool.tile([128, S_TILES, 4], bf16)
        with tc.tile_pool(name="w4ps", bufs=2, space="PSUM") as wps:
            for h in range(2):
                bh = mpool.tile([128, 128], f32, tag=f"w4bh{h}")
                nc.vector.tensor_scalar(
                    bh, rb, sl_sb[:, h:h + 1], None, op0=ALU.is_lt
                )
                bm = mpool.tile([128, 128], f32, tag=f"w4bm{h}")
                nc.vector.tensor_mul(bm, bh, sel)
                wt_ps = wps.tile([128, 128], f32, tag=f"w4t{h}")
                nc.tensor.transpose(wt_ps, bm, ident_f32)
                nc.vector.tensor_copy(
                    w4[:, 32 * h:32 * (h + 1), :].rearrange("p a b -> p (a b)"),
                    wt_ps,
                )
                if opts.get("debug"):
                    nc.sync.dma_start(dbg_bm[h, :, :], bm)
        if opts.get("debug"):
            nc.sync.dma_start(dbg_sel[:, :], sel)
            nc.sync.dma_start(dbg_rb[:, :], rb)

        # diag masks (sharding metadata inputs)
        dmask_sb = mpool.tile([128, 2, BI], f32)
        nc.gpsimd.dma_start(dmask_sb, dmask_ext.rearrange("(t p) i -> p t i", p=128))
        dmaskT_sb = mpool.tile([BI, 2, 128], f32)
        nc.gpsimd.dma_start(
            dmaskT_sb, dmaskT_ext.rearrange("p (t f) -> p t f", f=128)
        )

        # ---------------- persistent big buffers ----------------
        big = top.enter_context(tc.tile_pool(name="big", bufs=1))
        # packed-transposed fp8 pairs, stored as bf16 bit containers.
        # One tile per staging unit so the (whole-tile) dependency tracking
        # of the XBAR DMA writes stays exactly per-tile.
        IMQ = [[0, 1, 2, 3], [4, 5, 6, 7], [8, 9], [10]]
        imTq_g = [
            big.tile([128, 4 * len(ts), 112], bf16, name=f"imTq{g}")
            for g, ts in enumerate(IMQ)
        ]
        imP = big.tile([128, 4, 2, IM_TOK], fp8)      # dense planar im
        sTp_g = [
            big.tile([128, 4, 4, 128], bf16, name=f"sTpg{g}")
            for g in range(S_TILES // 4)
        ]
        maxima = big.tile([128, S_TILES, BI], bf16)  # per (word, wtile, img) region-max
        scoresT_sb = big.tile([BI, S_TILES, 4], f32)  # [img, wt, jj] == [img, sent]
        scores_sb = big.tile([128, 2, BI], f32)       # [sent%128, sent//128, img]

        # fp8 views: im pair-split for the deinterleave, s raw for SwInterleave
        imTq8_g = [
            t.bitcast(fp8).rearrange("p m (t b) -> p m b t", b=2) for t in imTq_g
        ]
        sTraw_g = [t.bitcast(fp8) for t in sTp_g]     # each [128, 4, 4, 256]

        with ExitStack() as mid:
            stage = mid.enter_context(
                tc.tile_pool(name="stage", bufs=opts["sf_bufs"])
            )
            pk = mid.enter_context(tc.tile_pool(name="pk", bufs=opts["pk_bufs"]))
            alp = mid.enter_context(
                tc.tile_pool(name="alp", bufs=opts["alp_bufs"], space="PSUM")
            )
            scp = mid.enter_context(tc.tile_pool(name="scp", bufs=1, space="PSUM"))
            scoresT_ps = scp.tile([BI, S_TILES, 4], f32)

            def stage_im_quad(g):
                ts = IMQ[g]
                win_g = IM_STAGE[ts[-1]][1]
                tf32s = []
                for t in ts:
                    toff, win, i0, ni = IM_STAGE[t]
                    nreal = 36 * ni
                    tf32 = stage.tile([128, D], f32, tag="sf32")
                    nc.sync.dma_start(
                        tf32[0:nreal, :],
                        im_ext[i0:i0 + ni, 1:1 + NREG, :].rearrange(
                            "i n d -> n i d"
                        ),
                    )
                    tf32s.append(tf32)
                ipkq = pk.tile([128, len(ts), D // 2], bf16, tag="spk")
                for k, t in enumerate(ts):
                    toff, win, i0, ni = IM_STAGE[t]
                    # masked cast on the (ramp-idle) vector engine
                    nc.vector.tensor_scalar(
                        ipkq[:, k, :].bitcast(fp8)[0:win, :], tf32s[k][0:win, :],
                        maskcol_im[0:win, t:t + 1], None, op0=ALU.mult,
                    )
                # one XBAR DMA for the whole quad (scalar queue: keeps the
                # sync queue free to prefetch s loads without blocking)
                nc.scalar.dma_start(
                    imTq_g[g][:, :, 0:win_g],
                    ipkq[0:win_g, :, :].rearrange("p a c -> p (a c)"),
                    transpose=True,
                )
            def im_deints(g):
                # deinterleave + compact + un-permute (region-major stage
                # order ni*n + i -> dense 36*i + n), one strided copy per tile
                for k, t in enumerate(IMQ[g]):
                    toff, win, i0, ni = IM_STAGE[t]
                    nreal = 36 * ni
                    nc.vector.tensor_copy(
                        imP[:, :, :, toff:toff + nreal].rearrange(
                            "p q b (i n) -> p q b i n", n=NREG
                        ),
                        imTq8_g[g][:, 4 * k:4 * k + 4, :, 0:nreal].rearrange(
                            "p q b (n i) -> p q b n i", i=ni
                        ).rearrange("p q b n i -> p q b i n"),
                    )

            def s_load(gq):
                # four per-tile loads (the DMA AP balancer caps at 3 dims,
                # so a quad can't be one DMA), word-major per tile
                tfs = []
                for a in range(4):
                    i = 4 * gq + a
                    tf32 = stage.tile([128, D], f32, tag="sf32")
                    nc.sync.dma_start(
                        tf32,
                        s_ext[4 * i:4 * i + 4, 1:1 + NWORD, :].rearrange(
                            "j w d -> w j d"
                        ),
                    )
                    tfs.append(tf32)
                return tfs

            def s_pack(gq, tfs):
                pkq = pk.tile([128, 4, D // 2], bf16, tag="spk")
                gsp = opts["gpsimd_cast"]
                for a in range(4):
                    i = 4 * gq + a
                    if gsp and (i % gsp == gsp - 1):
                        nc.vector.tensor_copy(
                            pkq[:, a, :].bitcast(fp8), tfs[a]
                        )
                    else:
                        nc.scalar.activation(
                            pkq[:, a, :].bitcast(fp8), tfs[a], ACTF.Copy
                        )
                nc.sync.dma_start(
                    sTp_g[gq].rearrange("p a q t -> p (a q) t"),
                    pkq.rearrange("p a c -> p (a c)"),
                    transpose=True,
                )

            # im head first (needed by rc0); s quad-loads run ahead of their
            # cast+transpose stages so no dispatch queue ever blocks
            # s staging starts immediately (its chain is mask-free); the im
            # quads stage in parallel behind the first three s quads so the
            # mask->cast->transpose chain never gates the s pipeline
            PRE = opts["s_prefetch"]
            HEADQ = 3
            pending = []
            for gq in range(HEADQ):
                pending.append((gq, s_load(gq)))
            for _ in range(HEADQ):
                s_pack(*pending.pop(0))
            for g in range(len(IMQ)):
                stage_im_quad(g)
            for g in range(len(IMQ)):
                im_deints(g)
            for gq in range(HEADQ, S_TILES // 4):
                pending.append((gq, s_load(gq)))
                if len(pending) > PRE:
                    s_pack(*pending.pop(0))
            for it in pending:
                s_pack(*it)

            # ---------------- main matmul + region-max + word-sum ----------------
            def word_sum(wt):
                # scoresT[img, 4wt+jj] = sum_m maxima[(jj,m), wt, img] * wmask
                nc.tensor.matmul(
                    scoresT_ps[:, wt, :],
                    lhsT=maxima[:, wt, :],
                    rhs=w4[:, wt, :],
                    start=True, stop=True,
                )

            for wt in range(S_TILES):
                for rci, (toff, ntok, nimg) in enumerate(RCHUNKS):
                    pal = alp.tile([128, nimg, NREG], f32, tag="align")
                    for q in range(4):
                        nc.tensor.matmul(
                            pal.rearrange("p a b -> p (a b)"),
                            lhsT=sTraw_g[wt // 4][:, wt % 4, q, :],
                            rhs=imP[:, q, :, toff:toff + ntok],
                            start=(q == 0),
                            stop=(q == 3),
                            perf_mode=SWI,
                        )
                    nc.vector.tensor_reduce(
                        maxima[:, wt, toff // NREG:toff // NREG + nimg],
                        pal, axis=AX.X, op=ALU.max,
                    )
                    # emit the previous tile's word-sum between rc chunks so the
                    # PE never waits on the vector MAX of the current tile
                    if rci == 0 and wt > 0:
                        word_sum(wt - 1)
            word_sum(S_TILES - 1)

            # scoresT -> SBUF (transposes to [sent, img] happen in the tail
            # scope so scp stays a single PSUM bank and alp gets one more)
            nc.vector.tensor_copy(scoresT_sb, scoresT_ps)
            if opts.get("debug"):
                w4d = mpool.tile([128, S_TILES, 4], f32)
                nc.vector.tensor_copy(w4d, w4)
                nc.sync.dma_start(dbg_w4[:, :, :], w4d)
                nc.sync.dma_start(dbg_scT[:, :, :], scoresT_sb)
                mxd = mpool.tile([128, S_TILES, BI], f32)
                nc.vector.tensor_copy(mxd, maxima)
                nc.sync.dma_start(dbg_mx[:, :, :], mxd)

        # ---------------- loss tail ----------------
        with ExitStack() as tail:
            tp = tail.enter_context(tc.tile_pool(name="tailp", bufs=1, space="PSUM"))
            ts = tail.enter_context(tc.tile_pool(name="tails", bufs=1))

            sc_ps = tp.tile([128, 2, BI], f32)
            for t in range(2):
                nc.tensor.transpose(
                    sc_ps[:, t, :],
                    scoresT_sb[:, 32 * t:32 * (t + 1), :].rearrange(
                        "p a b -> p (a b)"
                    ),
                    ident_f32[:BI, :BI],
                )
                nc.vector.tensor_copy(scores_sb[:, t, :], sc_ps[:, t, :])

            # col-max over local images (diag excluded) + scattered diag
            masked = ts.tile([128, 2, BI], f32)
            nc.vector.scalar_tensor_tensor(
                masked, dmask_sb, -BIG, scores_sb, op0=ALU.mult, op1=ALU.add
            )
            colmax_p = ts.tile([128, 2], f32)
            nc.vector.tensor_reduce(colmax_p, masked, axis=AX.X, op=ALU.max)
            dtmp = ts.tile([128, 2, BI], f32)
            nc.vector.tensor_mul(dtmp, dmask_sb, scores_sb)
            dscat = ts.tile([128, 2], f32)
            nc.vector.tensor_reduce(dscat, dtmp, axis=AX.X, op=ALU.add)

            # row-max over sentences (diag excluded); scoresT_sb is [img, sent]
            scT_flat = scoresT_sb.rearrange("p a b -> p (a b)")
            dmaskT_flat = dmaskT_sb.rearrange("p a b -> p (a b)")
            maskedT = ts.tile([BI, B], f32)
            nc.vector.scalar_tensor_tensor(
                maskedT, dmaskT_flat, -BIG, scT_flat, op0=ALU.mult, op1=ALU.add
            )
            rowmax = ts.tile([BI, 1], f32)
            nc.vector.tensor_reduce(rowmax, maskedT, axis=AX.X, op=ALU.max)
            dT_tmp = ts.tile([BI, B], f32)
            nc.vector.tensor_mul(dT_tmp, dmaskT_flat, scT_flat)
            d_row = ts.tile([BI, 1], f32)
            nc.vector.tensor_reduce(d_row, dT_tmp, axis=AX.X, op=ALU.add)

            cost_s = ts.tile([BI, 1], f32)
            nc.vector.tensor_sub(cost_s, rowmax, d_row)
            nc.vector.tensor_scalar(
                cost_s, cost_s, MARGIN, 0.0, op0=ALU.add, op1=ALU.max
            )
            cs_ps = tp.tile([1, 1], f32)
            nc.tensor.matmul(cs_ps, lhsT=ones32, rhs=cost_s, start=True, stop=True)

            # one packed tile: [:,0:2]=colmax | [:,2:4]=dscat | [0,4]=cost_s
            pkt = ts.tile([128, 5], f32)
            nc.vector.tensor_copy(pkt[:, 0:2], colmax_p)
            nc.vector.tensor_copy(pkt[:, 2:4], dscat)
            nc.vector.tensor_copy(pkt[0:1, 4:5], cs_ps)
            blk = dram.tile([BLK], f32)
            nc.sync.dma_start(
                blk[0:640].rearrange("(a p) -> p a", p=128), pkt
            )
            gath = dram.tile([NCORES, BLK], f32, addr_space="Shared")
            nc.gpsimd.collective_compute(
                "AllGather",
                ALU.bypass,
                ins=[blk.opt()],
                outs=[gath.opt()],
                replica_groups=[list(range(NCORES))],
            )

            # redundant final reduction on every core; single unpack DMA
            # per-piece transposing unpacks (DMA APs cap at 3 dims with a
            # contiguous final dim), spread across both dispatch queues
            g5 = ts.tile([128, 5, NCORES], f32)
            for a in range(5):
                eng = nc.sync if a % 2 == 0 else nc.scalar
                eng.dma_start(
                    g5[:, a, :],
                    gath[:, 128 * a:128 * (a + 1)].rearrange("c p -> p c"),
                )
            colmax_g = ts.tile([128, 2], f32)
            nc.vector.tensor_reduce(colmax_g, g5[:, 0:2, :], axis=AX.X, op=ALU.max)
            d_all = ts.tile([128, 2], f32)
            nc.vector.tensor_reduce(d_all, g5[:, 2:4, :], axis=AX.X, op=ALU.add)
            cim = ts.tile([128, 2], f32)
            nc.vector.tensor_sub(cim, colmax_g, d_all)
            nc.vector.tensor_scalar(cim, cim, MARGIN, 0.0, op0=ALU.add, op1=ALU.max)
            cim_r = ts.tile([128, 1], f32)
            nc.vector.tensor_reduce(cim_r, cim, axis=AX.X, op=ALU.add)
            cs_tot = ts.tile([1, 1], f32)
            nc.vector.tensor_reduce(cs_tot, g5[0:1, 4, :], axis=AX.X, op=ALU.add)
            nc.vector.tensor_add(cim_r[0:1, :], cim_r[0:1, :], cs_tot)
            tot_ps = tp.tile([1, 1], f32)
            nc.tensor.matmul(tot_ps, lhsT=ones128, rhs=cim_r, start=True, stop=True)
            total = ts.tile([1, 1], f32)
            nc.vector.tensor_copy(total, tot_ps)
            nc.sync.dma_start(out_ext[0:1], total[0, :])

    fix_multiwaits(nc, mybir)
    return nc


_CACHE = {}


def _get_nc():
    if "nc" not in _CACHE:
        _CACHE["nc"] = build_graph()
    return _CACHE["nc"]


def make_in_maps(im_set, s_seq, im_len, s_len):
    im_set = np.ascontiguousarray(im_set, dtype=np.float32)
    s_seq = np.ascontiguousarray(s_seq, dtype=np.float32)
    im_len = np.ascontiguousarray(im_len, dtype=np.int32)
    s_len = np.ascontiguousarray(s_len, dtype=np.int32)
    in_maps = []
    for c in range(NCORES):
        dm = np.zeros((B, BI), dtype=np.float32)
        for i in range(BI):
            dm[BI * c + i, i] = 1.0
        in_maps.append({
            "im_set": im_set[BI * c:BI * (c + 1)],
            "s_seq": s_seq,
            "im_len": im_len[BI * c:BI * (c + 1)],
            "s_len": s_len,
            "diag_mask": dm,
            "diag_maskT": np.ascontiguousarray(dm.T),
        })
    return in_maps


def kernel(im_set, s_seq, im_len, s_len):
    import time
    from concourse.bass_utils import run_bass_kernel_spmd

    nc = _get_nc()
    in_maps = make_in_maps(im_set, s_seq, im_len, s_len)
    last = None
    for attempt in range(3):
        try:
            res = run_bass_kernel_spmd(nc, in_maps, core_ids=list(range(NCORES)))
            return np.asarray(
                res.results[0]["out"], dtype=np.float32
            ).reshape(())[()]
        except Exception as e:  # transient device-unrecoverable happens
            last = e
            time.sleep(30 * (attempt + 1))
    raise last
